# revision 3
# baseline (speedup 1.0000x reference)
"""Trainium2 Bass kernel for the FD synapse layer — v3 (engine-rebalanced).

Math (per lane h, substeps s = 4t+k, dt = 1/4):
    y_{s+1} = c1*y_s + I_t          y = (Ca-mu)/(dt*alpha), y_0 = 0
    sig_s   = sigmoid(SSC*y_s)      SSC = dt*alpha/sigma
    P_s     = cp + V'_t*sig_s       V' = -dt*Prm*I - dt*k_delta, cp = 1-dt*k_min
    Q_s     = QM*sig_s + QA         QM = dt*k_delta, QA = dt*k_min
    R_{s+1} = P_s*R_s + Q_s         R_0 = 1
    sacc_t  = sum_k e1^{3-k} sig_{t,k} R_{t,k}
    E_{t+1} = e1^4*E_t + W2_t*sacc_t    W2 = -dt*beta*Prm*I, E_0 = 0

Engine assignment (per (lb, blk) unit, tb=512 timesteps, S=2048 substeps):
    DVE : y-scan at substep granularity (raw 3-D-AP scan, d1 = I broadcast
          over k), R-scan (d1 = Q in PSUM), sr = sig*R (bf16 2x), EPSC scan
    ACT : the single sigmoid over S, V'/W2 affines of I, P = vsig + cp,
          sacc PSUM->SBUF evacuation
    Pool: vsig = sig * V'-broadcast, racc = W2*sacc
    PE  : Q = diag(QM)@sig + QA x ones -> PSUM, Horner sacc = sum_k
          diag(e1^{3-k}) @ sr-plane-k -> PSUM (accumulating diag matmuls)

I/O is bf16 (host converts); output EPSC returned as f32.
Sharding: batch 32 -> 4 samples/core, pure data parallel on 8 cores.
"""

import numpy as np
import ml_dtypes
from contextlib import ExitStack

import concourse.bass as bass
import concourse.mybir as mybir
import concourse.tile as tile
from concourse.bass_utils import run_bass_kernel_spmd

f32 = mybir.dt.float32
bf16 = mybir.dt.bfloat16
AF = mybir.ActivationFunctionType
OP = mybir.AluOpType

B, T, H = 32, 2048, 512
K = 4
NCORES = 8
BPC = B // NCORES     # 4 samples per core
GH = H // 128         # 4 h-groups
NLB = BPC * GH        # 16 lane batches per core
PD = 128
TB = 512              # timesteps per block
S = K * TB            # substeps per block
NP = 13               # param columns per lb

(C1, SSC, VC, VA, W2S, CP, E14, QMC, QAC, SG0, SG1, SG2, SG3) = range(NP)


def _raw_scan(eng, out3, d0, initial, d1):
    """tensor_tensor_scan with multi-free-dim APs (verified on HW): the
    recurrence chains across slice boundaries in AP iteration order."""
    nc = eng.bass
    return eng.add_instruction(
        mybir.InstTensorScalarPtr(
            name=nc.get_next_instruction_name(),
            is_tensor_tensor_scan=True,
            is_scalar_tensor_tensor=True,
            op0=OP.mult,
            op1=OP.add,
            ins=[eng.lower_ap(d0), eng.lower_ap_or_imm(initial),
                 eng.lower_ap(d1)],
            outs=[eng.lower_ap(out3)],
        )
    )


BUFS = int(__import__('os').environ.get('V3_BUFS', '3'))
SKEW = [int(x) for x in __import__('os').environ.get('V3_SKEW', '1,2,3,4,6,10').split(',')]
QMODE = __import__('os').environ.get('V3_QMODE', 'pe')   # 'pe' | 'dve'


def build_program(Tn=T, tb=TB, nlb=NLB, n_devices=NCORES):
    nblk = Tn // tb
    s_ = K * tb
    nc = bass.Bass("TRN2", target_bir_lowering=False, debug=False,
                   num_devices=n_devices)
    I_d = nc.dram_tensor("i_ca", [nlb, PD, Tn], bf16, kind="ExternalInput").ap()
    par_d = nc.dram_tensor("par", [PD, nlb * NP], f32,
                           kind="ExternalInput").ap()
    wh_d = nc.dram_tensor("wh", [PD, GH * K * PD], bf16,
                          kind="ExternalInput").ap()
    wi_d = nc.dram_tensor("wi", [PD, GH * 3 * PD], bf16,
                          kind="ExternalInput").ap()
    wid_d = nc.dram_tensor("wid", [PD, GH * 3 * PD], bf16,
                           kind="ExternalInput").ap()
    O_d = [[nc.dram_tensor(f"epsc_{lb}_{blk}", [PD, tb], bf16,
                           kind="ExternalOutput").ap()
            for blk in range(nblk)] for lb in range(nlb)]

    with ExitStack() as ctx:
        tc = ctx.enter_context(tile.TileContext(nc))
        import os as _os
        TAGB = {"zsh": 5, "sig": 5, "vp": 6, "w2": 6, "vsig": 5, "pt": 5,
                "qts": 5, "rsh": 5, "sr": 5, "saccs": 6, "racc": 6,
                "etile": 5, "plane": 6}
        for kv in _os.environ.get('V3_TAGB', '').split(';'):
            if kv:
                k, v = kv.split('='); TAGB[k] = int(v)
        apool = ctx.enter_context(tc.tile_pool(name="asig", bufs=BUFS))
        bpool = ctx.enter_context(tc.tile_pool(name="bmid", bufs=BUFS))
        cpool = ctx.enter_context(tc.tile_pool(name="ccar", bufs=BUFS))
        spool = ctx.enter_context(tc.tile_pool(name="small", bufs=BUFS + 1))
        ipool = ctx.enter_context(tc.tile_pool(name="inp", bufs=2))
        ppool = ctx.enter_context(tc.tile_pool(name="par", bufs=1))
        plpool = ctx.enter_context(tc.psum_pool(name="plps", bufs=1))
        hpool = ctx.enter_context(tc.psum_pool(name="hps", bufs=2))

        par = ppool.tile([PD, nlb * NP], f32, tag="par")
        wh = ppool.tile([PD, GH * K * PD], bf16, tag="wh")
        wi = ppool.tile([PD, GH * 3 * PD], bf16, tag="wi")
        wid = ppool.tile([PD, GH * 3 * PD], bf16, tag="wid")
        nc.sync.dma_start(par[:], par_d)
        nc.sync.dma_start(wh[:], wh_d)
        nc.sync.dma_start(wi[:], wi_d)
        nc.sync.dma_start(wid[:], wid_d)

        itile_lbs = {}
        prev_z = {}
        prev_rsh = {}
        prev_e = {}

        def pcol_of(lb):
            return lambda i: par[:, lb * NP + i:lb * NP + i + 1]

        def stage_a(lb, blk):
            """z-scan (timestep granularity), V', W2 on DVE."""
            pcol = pcol_of(lb)
            t0 = blk * tb
            if blk == 0:
                itile_lb = ipool.tile([PD, Tn], bf16, tag="itile")
                nc.sync.dma_start(itile_lb[:], I_d[lb])
                itile_lbs[lb] = itile_lb
            it = itile_lbs[lb][:, t0:t0 + tb]

            zsh = apool.tile([PD, tb + 1], bf16, tag="zsh", bufs=TAGB["zsh"])
            init = 0.0 if blk == 0 else prev_z[lb][:, tb:tb + 1]
            nc.vector.tensor_tensor_scan(
                zsh[:, 1:tb + 1], pcol(C1).to_broadcast((PD, tb)), it,
                init, OP.mult, OP.add)
            if blk == 0:
                nc.vector.memset(zsh[:, 0:1], 0.0)
            else:
                nc.vector.tensor_copy(zsh[:, 0:1], prev_z[lb][:, tb:tb + 1])
            prev_z[lb] = zsh

            vp = spool.tile([PD, tb], bf16, tag="vp", bufs=TAGB["vp"])
            nc.vector.tensor_scalar(vp[:], it, pcol(VC), pcol(VA),
                                    OP.mult, OP.add)
            w2 = spool.tile([PD, tb], bf16, tag="w2", bufs=TAGB["w2"])
            nc.vector.tensor_scalar(w2[:], it, pcol(W2S), 0.0,
                                    OP.mult, OP.add)
            return zsh, it, vp, w2

        def stage_a2(lb, blk, zsh, it, vp, w2):
            """sigmoid-argument planes k=1..3 on PE -> PSUM."""
            g = lb % GH
            planes = []
            for j in range(3):
                pl = plpool.tile([PD, tb], f32, tag="plane", name="pl",
                                 bufs=TAGB["plane"])
                nc.tensor.matmul(pl[:], wi[:, (g * 3 + j) * PD:
                                            (g * 3 + j + 1) * PD],
                                 it, start=True, stop=False)
                nc.tensor.matmul(pl[:], wid[:, (g * 3 + j) * PD:
                                            (g * 3 + j + 1) * PD],
                                 zsh[:, 0:tb], start=False, stop=True)
                planes.append(pl)
            return zsh, planes, vp, w2

        def stage_a3(lb, blk, zsh, planes, vp, w2):
            """sigmoids: k=0 from z (SBUF), k=1..3 from PE planes (scales
            folded into the PE diags)."""
            pcol = pcol_of(lb)
            sig = apool.tile([PD, s_], bf16, tag="sig", bufs=TAGB["sig"])
            sig3 = sig[:].rearrange("p (t k) -> p t k", k=K)
            nc.scalar.activation(sig3[:, :, 0], zsh[:, 0:tb], AF.Sigmoid,
                                 bias=0.0, scale=pcol(SG0))
            for j, k in enumerate((1, 2, 3)):
                nc.scalar.activation(sig3[:, :, k], planes[j][:], AF.Sigmoid,
                                     bias=0.0, scale=1.0)
            return sig, vp, w2

        def stage_b1(lb, blk, sig, vp, w2):
            """vsig halves (Pool), P halves (ACT), Q (DVE ts 4x)."""
            pcol = pcol_of(lb)
            vsig = bpool.tile([PD, s_], bf16, tag="vsig", bufs=TAGB["vsig"])
            pt = bpool.tile([PD, s_], bf16, tag="pt", bufs=TAGB["pt"])
            NSPL = int(_os.environ.get('V3_VSPL', '2'))
            th = tb // NSPL
            sh = s_ // NSPL
            PTQ = _os.environ.get('V3_PTQ', 'dve_act')
            for h in range(NSPL):
                nc.gpsimd.tensor_mul(
                    vsig[:, h * sh:(h + 1) * sh].rearrange(
                        "p (t k) -> p t k", k=K),
                    sig[:, h * sh:(h + 1) * sh].rearrange(
                        "p (t k) -> p t k", k=K),
                    vp[:, h * th:(h + 1) * th].unsqueeze(2).broadcast_to(
                        (PD, th, K)))
                if PTQ == 'act_dve':
                    nc.scalar.activation(pt[:, h * sh:(h + 1) * sh],
                                         vsig[:, h * sh:(h + 1) * sh],
                                         AF.Identity, bias=pcol(CP), scale=1.0)
            qt = bpool.tile([PD, s_], bf16, tag="qts", bufs=TAGB["qts"])
            if PTQ == 'act_dve':
                nc.vector.tensor_scalar(qt[:], sig[:], pcol(QMC), pcol(QAC),
                                        OP.mult, OP.add)
            else:
                nc.vector.tensor_scalar(pt[:], vsig[:], 1.0, pcol(CP),
                                        OP.mult, OP.add)
                nc.scalar.activation(qt[:], sig[:], AF.Identity,
                                     bias=pcol(QAC), scale=pcol(QMC))
            return sig, pt, qt, w2

        def stage_b2(lb, blk, sig, pt, qt, w2):
            """R-scan (all-SBUF operands), sr."""
            rsh = cpool.tile([PD, s_ + 1], bf16, tag="rsh", bufs=TAGB["rsh"])
            init = 1.0 if blk == 0 else prev_rsh[lb][:, s_:s_ + 1]
            nc.vector.tensor_tensor_scan(rsh[:, 1:s_ + 1], pt[:], qt[:],
                                         init, OP.mult, OP.add)
            if blk == 0:
                nc.vector.memset(rsh[:, 0:1], 1.0)
            else:
                nc.vector.tensor_copy(rsh[:, 0:1], prev_rsh[lb][:, s_:s_ + 1])
            prev_rsh[lb] = rsh
            sr = bpool.tile([PD, s_], bf16, tag="sr", bufs=TAGB["sr"])
            nc.vector.tensor_mul(sr[:], sig[:], rsh[:, 0:s_])
            return sr, w2

        def stage_c1(lb, blk, sr, w2):
            """Horner on PE (4 accumulating diag matmuls), evacuate."""
            g = lb % GH
            srk = sr[:].rearrange("p (t k) -> p t k", k=K)
            sacc = hpool.tile([PD, tb], f32, tag="sacc")
            for k in range(K):
                w = wh[:, (g * K + k) * PD:(g * K + k + 1) * PD]
                nc.tensor.matmul(sacc[:], w, srk[:, :, k],
                                 start=(k == 0), stop=(k == K - 1))
            saccs = spool.tile([PD, tb], bf16, tag="saccs",
                               bufs=TAGB["saccs"])
            nc.scalar.copy(saccs[:], sacc[:])
            return saccs, w2

        def stage_c2(lb, blk, saccs, w2):
            """racc."""
            racc = spool.tile([PD, tb], bf16, tag="racc", bufs=TAGB["racc"])
            reng = nc.gpsimd if _os.environ.get('V3_RACC', 'pool') == 'pool' \
                else nc.vector
            reng.tensor_mul(racc[:], w2[:], saccs[:])
            return racc

        def stage_c3(lb, blk, racc):
            """EPSC scan, out DMA."""
            pcol = pcol_of(lb)
            etile = cpool.tile([PD, tb], bf16, tag="etile",
                               bufs=TAGB["etile"])
            einit = 0.0 if blk == 0 else prev_e[lb][:, tb - 1:tb]
            nc.vector.tensor_tensor_scan(
                etile[:], pcol(E14).to_broadcast((PD, tb)), racc[:],
                einit, OP.mult, OP.add)
            prev_e[lb] = etile
            nc.sync.dma_start(O_d[lb][blk][:], etile[:])

        units = [(lb, blk) for lb in range(nlb) for blk in range(nblk)]
        n = len(units)
        sa2, sa3, sb1, sb2, sc1, sc2 = SKEW
        a_out, a2_out, a3_out, b1_out, b2_out, c1_out, c2_out = \
            {}, {}, {}, {}, {}, {}, {}
        for i in range(n + sc2):
            if sc2 <= i < n + sc2:
                c2_out[i - sc2] = stage_c2(*units[i - sc2],
                                           *c1_out.pop(i - sc2))
            if sb2 <= i < n + sb2:
                b2_out[i - sb2] = stage_b2(*units[i - sb2],
                                           *b1_out.pop(i - sb2))
            if sb1 <= i < n + sb1:
                b1_out[i - sb1] = stage_b1(*units[i - sb1],
                                           *a3_out.pop(i - sb1))
            if sc1 <= i < n + sc1:
                c1_out[i - sc1] = stage_c1(*units[i - sc1],
                                           *b2_out.pop(i - sc1))
            if sa3 <= i < n + sa3:
                a3_out[i - sa3] = stage_a3(*units[i - sa3],
                                           *a2_out.pop(i - sa3))
            if sa2 <= i < n + sa2:
                a2_out[i - sa2] = stage_a2(*units[i - sa2],
                                           *a_out.pop(i - sa2))
            if i < n:
                a_out[i] = stage_a(*units[i])
            if sc2 <= i < n + sc2:
                stage_c3(*units[i - sc2], c2_out.pop(i - sc2))

    import bass_rust
    bass_rust.generate_event_semaphores(nc)
    return nc


def derive_params(log_Ca_mu, log_Ca_sigma, log_tau_Ca, log_alpha, log_tau_EPSC,
                  log_beta, presigmoid_P_rel_max, log_k_recov_min,
                  log_k_recov_delta, ode_steps):
    d = np.float64
    dt = 1.0 / int(ode_steps)
    sigma = np.exp(log_Ca_sigma.astype(d))
    tau_Ca = np.exp(log_tau_Ca.astype(d))
    alpha = np.exp(log_alpha.astype(d))
    tau_E = np.exp(log_tau_EPSC.astype(d))
    beta = np.exp(log_beta.astype(d))
    Prm = 1.0 / (1.0 + np.exp(-presigmoid_P_rel_max.astype(d)))
    k_min = np.exp(log_k_recov_min.astype(d))
    k_delta = np.exp(log_k_recov_delta.astype(d))

    e1 = 1.0 - dt / tau_E
    c1 = 1.0 - dt / tau_Ca
    S_k = np.stack([np.zeros_like(c1), np.ones_like(c1), 1.0 + c1,
                    1.0 + c1 + c1 ** 2], 0)          # [K, H]
    S4 = S_k[3] + c1 ** 3
    n = log_Ca_mu.shape[0]
    par = np.zeros((n, NP), np.float64)
    par[:, C1] = c1 ** 4                 # z-scan coefficient (timesteps)
    par[:, SSC] = dt * alpha / sigma
    par[:, VC] = -dt * Prm
    par[:, VA] = -dt * k_delta
    par[:, W2S] = -dt * beta * Prm
    par[:, CP] = 1.0 - dt * k_min
    par[:, E14] = e1 ** 4
    par[:, QMC] = dt * k_delta
    par[:, QAC] = dt * k_min
    # sig_0 = sigmoid(SG0*z); k>=1: plane_k = SGk*(z + (S_k/(c1^k S4)) I) on
    # PE with the scale folded into both diags; sig_k = sigmoid(plane_k)
    ssc = dt * alpha / sigma
    for k in range(K):
        par[:, SG0 + k] = ssc * (c1 ** k) * S4
    sg = np.stack([ssc * (c1 ** k) * S4 for k in range(K)], 0)      # [K,H]
    wi = np.stack([sg[k] * S_k[k] / ((c1 ** k) * S4)
                   for k in range(1, K)], 0)                        # [3,H]
    wz = sg[1:4]                                                    # [3,H]
    qm = dt * k_delta
    qa = dt * k_min
    hw_ = np.stack([e1 ** (3 - k) for k in range(K)], 0)   # [K, H]
    return par.astype(np.float32), qm.astype(np.float32), \
        qa.astype(np.float32), hw_.astype(np.float32), \
        wi.astype(np.float32), wz.astype(np.float32)


_PROG = None
LAST_RESULTS = None


def _get_program():
    global _PROG
    if _PROG is None:
        _PROG = build_program()
    return _PROG


def kernel(I_Ca, log_Ca_mu, log_Ca_sigma, log_tau_Ca, log_alpha, log_tau_EPSC,
           log_beta, presigmoid_P_rel_max, log_k_recov_min, log_k_recov_delta,
           ode_steps):
    assert int(ode_steps) == K
    I_Ca = np.asarray(I_Ca, np.float32)
    assert I_Ca.shape == (B, T, H)

    par_h, qm, qa, hw_, wi, wz = derive_params(
        np.asarray(log_Ca_mu), np.asarray(log_Ca_sigma), np.asarray(log_tau_Ca),
        np.asarray(log_alpha), np.asarray(log_tau_EPSC), np.asarray(log_beta),
        np.asarray(presigmoid_P_rel_max), np.asarray(log_k_recov_min),
        np.asarray(log_k_recov_delta), ode_steps)

    # params: lane-batch lb = b_local*GH + g holds lanes h = g*128 + p
    par_lb = par_h.reshape(GH, PD, NP)
    par_core = np.ascontiguousarray(
        np.broadcast_to(par_lb[None], (BPC, GH, PD, NP)).reshape(
            NLB, PD, NP).transpose(1, 0, 2).reshape(PD, NLB * NP))

    # PE weights per h-group g
    bf = ml_dtypes.bfloat16
    wh_h = np.zeros((PD, GH * K * PD), bf)
    wi_h = np.zeros((PD, GH * 3 * PD), bf)
    wid_h = np.zeros((PD, GH * 3 * PD), bf)
    for g in range(GH):
        lanes = slice(g * PD, (g + 1) * PD)
        for k in range(K):
            blockh = wh_h[:, (g * K + k) * PD:(g * K + k + 1) * PD]
            np.fill_diagonal(blockh, hw_[k, lanes].astype(bf))
        for j in range(3):
            blockw = wi_h[:, (g * 3 + j) * PD:(g * 3 + j + 1) * PD]
            np.fill_diagonal(blockw, wi[j, lanes].astype(bf))
            blockz = wid_h[:, (g * 3 + j) * PD:(g * 3 + j + 1) * PD]
            np.fill_diagonal(blockz, wz[j, lanes].astype(bf))

    nc = _get_program()
    in_maps = []
    for c in range(NCORES):
        Ic = I_Ca[c * BPC:(c + 1) * BPC]
        Ic = Ic.reshape(BPC, T, GH, PD).transpose(0, 2, 3, 1)
        in_maps.append({
            "i_ca": np.ascontiguousarray(Ic.reshape(NLB, PD, T)).astype(bf),
            "par": par_core,
            "wh": wh_h, "wi": wi_h, "wid": wid_h,
        })

    res = run_bass_kernel_spmd(nc, in_maps, core_ids=list(range(NCORES)))
    global LAST_RESULTS
    LAST_RESULTS = res
    nblk = T // TB
    out = np.empty((B, T, H), np.float32)
    for c in range(NCORES):
        Oc = np.stack([
            np.concatenate([res.results[c][f"epsc_{lb}_{blk}"].astype(np.float32)
                            for blk in range(nblk)], axis=1)
            for lb in range(NLB)])
        Oc = Oc.reshape(BPC, GH, PD, T)
        out[c * BPC:(c + 1) * BPC] = Oc.transpose(0, 3, 1, 2).reshape(BPC, T, H)
    return out


# revision 4
# speedup vs baseline: 1.0047x; 1.0047x over previous
"""Trainium2 Bass kernel for the FD synapse layer — v3 (engine-rebalanced).

Math (per lane h, substeps s = 4t+k, dt = 1/4):
    y_{s+1} = c1*y_s + I_t          y = (Ca-mu)/(dt*alpha), y_0 = 0
    sig_s   = sigmoid(SSC*y_s)      SSC = dt*alpha/sigma
    P_s     = cp + V'_t*sig_s       V' = -dt*Prm*I - dt*k_delta, cp = 1-dt*k_min
    Q_s     = QM*sig_s + QA         QM = dt*k_delta, QA = dt*k_min
    R_{s+1} = P_s*R_s + Q_s         R_0 = 1
    sacc_t  = sum_k e1^{3-k} sig_{t,k} R_{t,k}
    E_{t+1} = e1^4*E_t + W2_t*sacc_t    W2 = -dt*beta*Prm*I, E_0 = 0

Engine assignment (per (lb, blk) unit, tb=512 timesteps, S=2048 substeps):
    DVE : y-scan at substep granularity (raw 3-D-AP scan, d1 = I broadcast
          over k), R-scan (d1 = Q in PSUM), sr = sig*R (bf16 2x), EPSC scan
    ACT : the single sigmoid over S, V'/W2 affines of I, P = vsig + cp,
          sacc PSUM->SBUF evacuation
    Pool: vsig = sig * V'-broadcast, racc = W2*sacc
    PE  : Q = diag(QM)@sig + QA x ones -> PSUM, Horner sacc = sum_k
          diag(e1^{3-k}) @ sr-plane-k -> PSUM (accumulating diag matmuls)

I/O is bf16 (host converts); output EPSC returned as f32.
Sharding: batch 32 -> 4 samples/core, pure data parallel on 8 cores.
"""

import numpy as np
import ml_dtypes
from contextlib import ExitStack

import concourse.bass as bass
import concourse.mybir as mybir
import concourse.tile as tile
from concourse.bass_utils import run_bass_kernel_spmd

f32 = mybir.dt.float32
bf16 = mybir.dt.bfloat16
AF = mybir.ActivationFunctionType
OP = mybir.AluOpType

B, T, H = 32, 2048, 512
K = 4
NCORES = 8
BPC = B // NCORES     # 4 samples per core
GH = H // 128         # 4 h-groups
NLB = BPC * GH        # 16 lane batches per core
PD = 128
TB = int(__import__('os').environ.get('V3_TB', '512'))  # timesteps per block
S = K * TB            # substeps per block
NP = 13               # param columns per lb

(C1, SSC, VC, VA, W2S, CP, E14, QMC, QAC, SG0, SG1, SG2, SG3) = range(NP)


def _raw_scan(eng, out3, d0, initial, d1):
    """tensor_tensor_scan with multi-free-dim APs (verified on HW): the
    recurrence chains across slice boundaries in AP iteration order."""
    nc = eng.bass
    return eng.add_instruction(
        mybir.InstTensorScalarPtr(
            name=nc.get_next_instruction_name(),
            is_tensor_tensor_scan=True,
            is_scalar_tensor_tensor=True,
            op0=OP.mult,
            op1=OP.add,
            ins=[eng.lower_ap(d0), eng.lower_ap_or_imm(initial),
                 eng.lower_ap(d1)],
            outs=[eng.lower_ap(out3)],
        )
    )


BUFS = int(__import__('os').environ.get('V3_BUFS', '3'))
SKEW = [int(x) for x in __import__('os').environ.get('V3_SKEW', '1,2,3,4,6,10').split(',')]
QMODE = __import__('os').environ.get('V3_QMODE', 'pe')   # 'pe' | 'dve'


def build_program(Tn=T, tb=TB, nlb=NLB, n_devices=NCORES):
    nblk = Tn // tb
    s_ = K * tb
    nc = bass.Bass("TRN2", target_bir_lowering=False, debug=False,
                   num_devices=n_devices)
    I_d = nc.dram_tensor("i_ca", [nlb, PD, Tn], bf16, kind="ExternalInput").ap()
    par_d = nc.dram_tensor("par", [PD, nlb * NP], f32,
                           kind="ExternalInput").ap()
    wh_d = nc.dram_tensor("wh", [PD, GH * K * PD], bf16,
                          kind="ExternalInput").ap()
    wi_d = nc.dram_tensor("wi", [PD, GH * 3 * PD], bf16,
                          kind="ExternalInput").ap()
    wid_d = nc.dram_tensor("wid", [PD, GH * 3 * PD], bf16,
                           kind="ExternalInput").ap()
    O_d = [[nc.dram_tensor(f"epsc_{lb}_{blk}", [PD, tb], bf16,
                           kind="ExternalOutput").ap()
            for blk in range(nblk)] for lb in range(nlb)]

    with ExitStack() as ctx:
        tc = ctx.enter_context(tile.TileContext(nc))
        import os as _os
        TAGB = {"zsh": 5, "sig": 5, "vp": 6, "w2": 6, "vsig": 5, "pt": 5,
                "qts": 5, "rsh": 5, "sr": 5, "saccs": 6, "racc": 6,
                "etile": 5, "plane": 6}
        for kv in _os.environ.get('V3_TAGB', '').split(';'):
            if kv:
                k, v = kv.split('='); TAGB[k] = int(v)
        apool = ctx.enter_context(tc.tile_pool(name="asig", bufs=BUFS))
        bpool = ctx.enter_context(tc.tile_pool(name="bmid", bufs=BUFS))
        cpool = ctx.enter_context(tc.tile_pool(name="ccar", bufs=BUFS))
        spool = ctx.enter_context(tc.tile_pool(name="small", bufs=BUFS + 1))
        ipool = ctx.enter_context(tc.tile_pool(name="inp", bufs=2))
        ppool = ctx.enter_context(tc.tile_pool(name="par", bufs=1))
        plpool = ctx.enter_context(tc.psum_pool(name="plps", bufs=1))
        hpool = ctx.enter_context(tc.psum_pool(name="hps", bufs=2))

        par = ppool.tile([PD, nlb * NP], f32, tag="par")
        wh = ppool.tile([PD, GH * K * PD], bf16, tag="wh")
        wi = ppool.tile([PD, GH * 3 * PD], bf16, tag="wi")
        wid = ppool.tile([PD, GH * 3 * PD], bf16, tag="wid")
        nc.sync.dma_start(par[:], par_d)

        itile_lbs = {}
        prev_z = {}
        prev_rsh = {}
        prev_e = {}

        def pcol_of(lb):
            return lambda i: par[:, lb * NP + i:lb * NP + i + 1]

        def stage_a0(lb, blk):
            """input-chunk prefetch DMA."""
            t0 = blk * tb
            if blk == 0:
                itile_lb = ipool.tile([PD, Tn], bf16, tag="itile")
                itile_lbs[lb] = itile_lb
            nc.sync.dma_start(itile_lbs[lb][:, t0:t0 + tb],
                              I_d[lb][:, t0:t0 + tb])

        def stage_a(lb, blk):
            """z-scan (timestep granularity), V', W2 on DVE."""
            pcol = pcol_of(lb)
            t0 = blk * tb
            it = itile_lbs[lb][:, t0:t0 + tb]

            zsh = apool.tile([PD, tb + 1], bf16, tag="zsh", bufs=TAGB["zsh"])
            init = 0.0 if blk == 0 else prev_z[lb][:, tb:tb + 1]
            nc.vector.tensor_tensor_scan(
                zsh[:, 1:tb + 1], pcol(C1).to_broadcast((PD, tb)), it,
                init, OP.mult, OP.add)
            CARRY_ACT = _os.environ.get('V3_CARRY', 'dve') == 'act'
            if blk == 0:
                if CARRY_ACT:
                    nc.scalar.mul(zsh[:, 0:1], zsh[:, 1:2], 0.0)
                else:
                    nc.vector.memset(zsh[:, 0:1], 0.0)
            elif CARRY_ACT:
                nc.scalar.copy(zsh[:, 0:1], prev_z[lb][:, tb:tb + 1])
            else:
                nc.vector.tensor_copy(zsh[:, 0:1], prev_z[lb][:, tb:tb + 1])
            prev_z[lb] = zsh

            vp = spool.tile([PD, tb], bf16, tag="vp", bufs=TAGB["vp"])
            nc.vector.tensor_scalar(vp[:], it, pcol(VC), pcol(VA),
                                    OP.mult, OP.add)
            w2 = spool.tile([PD, tb], bf16, tag="w2", bufs=TAGB["w2"])
            nc.vector.tensor_scalar(w2[:], it, pcol(W2S), 0.0,
                                    OP.mult, OP.add)
            return zsh, it, vp, w2

        def stage_a2(lb, blk, zsh, it, vp, w2):
            """sigmoid-argument planes k=1..3 on PE -> PSUM."""
            g = lb % GH
            planes = []
            for j in range(3):
                pl = plpool.tile([PD, tb], f32, tag="plane", name="pl",
                                 bufs=TAGB["plane"])
                for c0 in range(0, tb, 512):
                    nc.tensor.matmul(pl[:, c0:c0 + 512],
                                     wi[:, (g * 3 + j) * PD:
                                        (g * 3 + j + 1) * PD],
                                     it[:, c0:c0 + 512],
                                     start=True, stop=False)
                    nc.tensor.matmul(pl[:, c0:c0 + 512],
                                     wid[:, (g * 3 + j) * PD:
                                         (g * 3 + j + 1) * PD],
                                     zsh[:, c0:c0 + 512],
                                     start=False, stop=True)
                planes.append(pl)
            return zsh, planes, vp, w2

        def stage_a3(lb, blk, zsh, planes, vp, w2):
            """sigmoids: k=0 from z (SBUF), k=1..3 from PE planes (scales
            folded into the PE diags)."""
            pcol = pcol_of(lb)
            sig = apool.tile([PD, s_], bf16, tag="sig", bufs=TAGB["sig"])
            sig3 = sig[:].rearrange("p (t k) -> p t k", k=K)
            nc.scalar.activation(sig3[:, :, 0], zsh[:, 0:tb], AF.Sigmoid,
                                 bias=0.0, scale=pcol(SG0))
            for j, k in enumerate((1, 2, 3)):
                nc.scalar.activation(sig3[:, :, k], planes[j][:], AF.Sigmoid,
                                     bias=0.0, scale=1.0)
            return sig, vp, w2

        def stage_b1(lb, blk, sig, vp, w2):
            """vsig halves (Pool), P halves (ACT), Q (DVE ts 4x)."""
            pcol = pcol_of(lb)
            vsig = bpool.tile([PD, s_], bf16, tag="vsig", bufs=TAGB["vsig"])
            pt = bpool.tile([PD, s_], bf16, tag="pt", bufs=TAGB["pt"])
            NSPL = int(_os.environ.get('V3_VSPL', '2'))
            th = tb // NSPL
            sh = s_ // NSPL
            PTQ = _os.environ.get('V3_PTQ', 'dve_act')
            for h in range(NSPL):
                nc.gpsimd.tensor_mul(
                    vsig[:, h * sh:(h + 1) * sh].rearrange(
                        "p (t k) -> p t k", k=K),
                    sig[:, h * sh:(h + 1) * sh].rearrange(
                        "p (t k) -> p t k", k=K),
                    vp[:, h * th:(h + 1) * th].unsqueeze(2).broadcast_to(
                        (PD, th, K)))
                if PTQ == 'act_dve':
                    nc.scalar.activation(pt[:, h * sh:(h + 1) * sh],
                                         vsig[:, h * sh:(h + 1) * sh],
                                         AF.Identity, bias=pcol(CP), scale=1.0)
            qt = bpool.tile([PD, s_], bf16, tag="qts", bufs=TAGB["qts"])
            if PTQ == 'act_dve':
                nc.vector.tensor_scalar(qt[:], sig[:], pcol(QMC), pcol(QAC),
                                        OP.mult, OP.add)
            else:
                nc.vector.tensor_scalar(pt[:], vsig[:], 1.0, pcol(CP),
                                        OP.mult, OP.add)
                nc.scalar.activation(qt[:], sig[:], AF.Identity,
                                     bias=pcol(QAC), scale=pcol(QMC))
            return sig, pt, qt, w2

        def stage_b2(lb, blk, sig, pt, qt, w2):
            """R-scan (all-SBUF operands), sr."""
            rsh = cpool.tile([PD, s_ + 1], bf16, tag="rsh", bufs=TAGB["rsh"])
            init = 1.0 if blk == 0 else prev_rsh[lb][:, s_:s_ + 1]
            nc.vector.tensor_tensor_scan(rsh[:, 1:s_ + 1], pt[:], qt[:],
                                         init, OP.mult, OP.add)
            CARRY_ACT = _os.environ.get('V3_CARRY', 'dve') == 'act'
            if blk == 0:
                if CARRY_ACT:
                    nc.scalar.activation(rsh[:, 0:1], rsh[:, 1:2], AF.Identity,
                                         bias=1.0, scale=0.0)
                else:
                    nc.vector.memset(rsh[:, 0:1], 1.0)
            elif CARRY_ACT:
                nc.scalar.copy(rsh[:, 0:1], prev_rsh[lb][:, s_:s_ + 1])
            else:
                nc.vector.tensor_copy(rsh[:, 0:1], prev_rsh[lb][:, s_:s_ + 1])
            prev_rsh[lb] = rsh
            sr = bpool.tile([PD, s_], bf16, tag="sr", bufs=TAGB["sr"])
            nc.vector.tensor_mul(sr[:], sig[:], rsh[:, 0:s_])
            return sr, w2

        def stage_c1(lb, blk, sr, w2):
            """Horner on PE (4 accumulating diag matmuls), evacuate."""
            g = lb % GH
            srk = sr[:].rearrange("p (t k) -> p t k", k=K)
            sacc = hpool.tile([PD, tb], f32, tag="sacc", bufs=TAGB.get("sacc", 2))
            for c0 in range(0, tb, 512):
                for k in range(K):
                    w = wh[:, (g * K + k) * PD:(g * K + k + 1) * PD]
                    nc.tensor.matmul(sacc[:, c0:c0 + 512], w,
                                     srk[:, c0:c0 + 512, k],
                                     start=(k == 0), stop=(k == K - 1))
            saccs = spool.tile([PD, tb], bf16, tag="saccs",
                               bufs=TAGB["saccs"])
            nc.scalar.copy(saccs[:], sacc[:])
            return saccs, w2

        def stage_c2(lb, blk, saccs, w2):
            """racc."""
            racc = spool.tile([PD, tb], bf16, tag="racc", bufs=TAGB["racc"])
            reng = nc.gpsimd if _os.environ.get('V3_RACC', 'pool') == 'pool' \
                else nc.vector
            reng.tensor_mul(racc[:], w2[:], saccs[:])
            return racc

        def stage_c3(lb, blk, racc):
            """EPSC scan, out DMA."""
            pcol = pcol_of(lb)
            etile = cpool.tile([PD, tb], bf16, tag="etile",
                               bufs=TAGB["etile"])
            einit = 0.0 if blk == 0 else prev_e[lb][:, tb - 1:tb]
            nc.vector.tensor_tensor_scan(
                etile[:], pcol(E14).to_broadcast((PD, tb)), racc[:],
                einit, OP.mult, OP.add)
            prev_e[lb] = etile
            nc.sync.dma_start(O_d[lb][blk][:], etile[:])

        units = [(lb, blk) for lb in range(nlb) for blk in range(nblk)]
        n = len(units)
        sa2, sa3, sb1, sb2, sc1, sc2 = SKEW
        a_out, a2_out, a3_out, b1_out, b2_out, c1_out, c2_out = \
            {}, {}, {}, {}, {}, {}, {}
        for i in range(n + sc2):
            if sc2 <= i < n + sc2:
                c2_out[i - sc2] = stage_c2(*units[i - sc2],
                                           *c1_out.pop(i - sc2))
            if sb2 <= i < n + sb2:
                b2_out[i - sb2] = stage_b2(*units[i - sb2],
                                           *b1_out.pop(i - sb2))
            if sb1 <= i < n + sb1:
                b1_out[i - sb1] = stage_b1(*units[i - sb1],
                                           *a3_out.pop(i - sb1))
            if sc1 <= i < n + sc1:
                c1_out[i - sc1] = stage_c1(*units[i - sc1],
                                           *b2_out.pop(i - sc1))
            if sa3 <= i < n + sa3:
                a3_out[i - sa3] = stage_a3(*units[i - sa3],
                                           *a2_out.pop(i - sa3))
            if sa2 <= i < n + sa2:
                a2_out[i - sa2] = stage_a2(*units[i - sa2],
                                           *a_out.pop(i - sa2))
            if i == 0:
                stage_a0(*units[0])
                nc.sync.dma_start(wi[:], wi_d)
                nc.sync.dma_start(wid[:], wid_d)
            if i + 1 < n:
                stage_a0(*units[i + 1])
            if i < n:
                a_out[i] = stage_a(*units[i])
            if i == 0:
                nc.sync.dma_start(wh[:], wh_d)
            if sc2 <= i < n + sc2:
                stage_c3(*units[i - sc2], c2_out.pop(i - sc2))

    import bass_rust
    bass_rust.generate_event_semaphores(nc)
    return nc


def derive_params(log_Ca_mu, log_Ca_sigma, log_tau_Ca, log_alpha, log_tau_EPSC,
                  log_beta, presigmoid_P_rel_max, log_k_recov_min,
                  log_k_recov_delta, ode_steps):
    d = np.float64
    dt = 1.0 / int(ode_steps)
    sigma = np.exp(log_Ca_sigma.astype(d))
    tau_Ca = np.exp(log_tau_Ca.astype(d))
    alpha = np.exp(log_alpha.astype(d))
    tau_E = np.exp(log_tau_EPSC.astype(d))
    beta = np.exp(log_beta.astype(d))
    Prm = 1.0 / (1.0 + np.exp(-presigmoid_P_rel_max.astype(d)))
    k_min = np.exp(log_k_recov_min.astype(d))
    k_delta = np.exp(log_k_recov_delta.astype(d))

    e1 = 1.0 - dt / tau_E
    c1 = 1.0 - dt / tau_Ca
    S_k = np.stack([np.zeros_like(c1), np.ones_like(c1), 1.0 + c1,
                    1.0 + c1 + c1 ** 2], 0)          # [K, H]
    S4 = S_k[3] + c1 ** 3
    n = log_Ca_mu.shape[0]
    par = np.zeros((n, NP), np.float64)
    par[:, C1] = c1 ** 4                 # z-scan coefficient (timesteps)
    par[:, SSC] = dt * alpha / sigma
    par[:, VC] = -dt * Prm
    par[:, VA] = -dt * k_delta
    par[:, W2S] = -dt * beta * Prm
    par[:, CP] = 1.0 - dt * k_min
    par[:, E14] = e1 ** 4
    par[:, QMC] = dt * k_delta
    par[:, QAC] = dt * k_min
    # sig_0 = sigmoid(SG0*z); k>=1: plane_k = SGk*(z + (S_k/(c1^k S4)) I) on
    # PE with the scale folded into both diags; sig_k = sigmoid(plane_k)
    ssc = dt * alpha / sigma
    for k in range(K):
        par[:, SG0 + k] = ssc * (c1 ** k) * S4
    sg = np.stack([ssc * (c1 ** k) * S4 for k in range(K)], 0)      # [K,H]
    wi = np.stack([sg[k] * S_k[k] / ((c1 ** k) * S4)
                   for k in range(1, K)], 0)                        # [3,H]
    wz = sg[1:4]                                                    # [3,H]
    qm = dt * k_delta
    qa = dt * k_min
    hw_ = np.stack([e1 ** (3 - k) for k in range(K)], 0)   # [K, H]
    return par.astype(np.float32), qm.astype(np.float32), \
        qa.astype(np.float32), hw_.astype(np.float32), \
        wi.astype(np.float32), wz.astype(np.float32)


_PROG = None
LAST_RESULTS = None


def _get_program():
    global _PROG
    if _PROG is None:
        _PROG = build_program()
    return _PROG


def kernel(I_Ca, log_Ca_mu, log_Ca_sigma, log_tau_Ca, log_alpha, log_tau_EPSC,
           log_beta, presigmoid_P_rel_max, log_k_recov_min, log_k_recov_delta,
           ode_steps):
    assert int(ode_steps) == K
    I_Ca = np.asarray(I_Ca, np.float32)
    assert I_Ca.shape == (B, T, H)

    par_h, qm, qa, hw_, wi, wz = derive_params(
        np.asarray(log_Ca_mu), np.asarray(log_Ca_sigma), np.asarray(log_tau_Ca),
        np.asarray(log_alpha), np.asarray(log_tau_EPSC), np.asarray(log_beta),
        np.asarray(presigmoid_P_rel_max), np.asarray(log_k_recov_min),
        np.asarray(log_k_recov_delta), ode_steps)

    # params: lane-batch lb = b_local*GH + g holds lanes h = g*128 + p
    par_lb = par_h.reshape(GH, PD, NP)
    par_core = np.ascontiguousarray(
        np.broadcast_to(par_lb[None], (BPC, GH, PD, NP)).reshape(
            NLB, PD, NP).transpose(1, 0, 2).reshape(PD, NLB * NP))

    # PE weights per h-group g
    bf = ml_dtypes.bfloat16
    wh_h = np.zeros((PD, GH * K * PD), bf)
    wi_h = np.zeros((PD, GH * 3 * PD), bf)
    wid_h = np.zeros((PD, GH * 3 * PD), bf)
    for g in range(GH):
        lanes = slice(g * PD, (g + 1) * PD)
        for k in range(K):
            blockh = wh_h[:, (g * K + k) * PD:(g * K + k + 1) * PD]
            np.fill_diagonal(blockh, hw_[k, lanes].astype(bf))
        for j in range(3):
            blockw = wi_h[:, (g * 3 + j) * PD:(g * 3 + j + 1) * PD]
            np.fill_diagonal(blockw, wi[j, lanes].astype(bf))
            blockz = wid_h[:, (g * 3 + j) * PD:(g * 3 + j + 1) * PD]
            np.fill_diagonal(blockz, wz[j, lanes].astype(bf))

    nc = _get_program()
    in_maps = []
    for c in range(NCORES):
        Ic = I_Ca[c * BPC:(c + 1) * BPC]
        Ic = Ic.reshape(BPC, T, GH, PD).transpose(0, 2, 3, 1)
        in_maps.append({
            "i_ca": np.ascontiguousarray(Ic.reshape(NLB, PD, T)).astype(bf),
            "par": par_core,
            "wh": wh_h, "wi": wi_h, "wid": wid_h,
        })

    res = run_bass_kernel_spmd(nc, in_maps, core_ids=list(range(NCORES)))
    global LAST_RESULTS
    LAST_RESULTS = res
    nblk = T // TB
    out = np.empty((B, T, H), np.float32)
    for c in range(NCORES):
        Oc = np.stack([
            np.concatenate([res.results[c][f"epsc_{lb}_{blk}"].astype(np.float32)
                            for blk in range(nblk)], axis=1)
            for lb in range(NLB)])
        Oc = Oc.reshape(BPC, GH, PD, T)
        out[c * BPC:(c + 1) * BPC] = Oc.transpose(0, 3, 1, 2).reshape(BPC, T, H)
    return out


# revision 5
# speedup vs baseline: 1.0193x; 1.0145x over previous
"""Trainium2 Bass kernel for the FD synapse layer — v3 (engine-rebalanced).

Math (per lane h, substeps s = 4t+k, dt = 1/4):
    y_{s+1} = c1*y_s + I_t          y = (Ca-mu)/(dt*alpha), y_0 = 0
    sig_s   = sigmoid(SSC*y_s)      SSC = dt*alpha/sigma
    P_s     = cp + V'_t*sig_s       V' = -dt*Prm*I - dt*k_delta, cp = 1-dt*k_min
    Q_s     = QM*sig_s + QA         QM = dt*k_delta, QA = dt*k_min
    R_{s+1} = P_s*R_s + Q_s         R_0 = 1
    sacc_t  = sum_k e1^{3-k} sig_{t,k} R_{t,k}
    E_{t+1} = e1^4*E_t + W2_t*sacc_t    W2 = -dt*beta*Prm*I, E_0 = 0

Engine assignment (per (lb, blk) unit, tb=512 timesteps, S=2048 substeps):
    DVE : y-scan at substep granularity (raw 3-D-AP scan, d1 = I broadcast
          over k), R-scan (d1 = Q in PSUM), sr = sig*R (bf16 2x), EPSC scan
    ACT : the single sigmoid over S, V'/W2 affines of I, P = vsig + cp,
          sacc PSUM->SBUF evacuation
    Pool: vsig = sig * V'-broadcast, racc = W2*sacc
    PE  : Q = diag(QM)@sig + QA x ones -> PSUM, Horner sacc = sum_k
          diag(e1^{3-k}) @ sr-plane-k -> PSUM (accumulating diag matmuls)

I/O is bf16 (host converts); output EPSC returned as f32.
Sharding: batch 32 -> 4 samples/core, pure data parallel on 8 cores.
"""

import numpy as np
import ml_dtypes
from contextlib import ExitStack

import concourse.bass as bass
import concourse.mybir as mybir
import concourse.tile as tile
from concourse.bass_utils import run_bass_kernel_spmd

f32 = mybir.dt.float32
bf16 = mybir.dt.bfloat16
AF = mybir.ActivationFunctionType
OP = mybir.AluOpType

B, T, H = 32, 2048, 512
K = 4
NCORES = 8
BPC = B // NCORES     # 4 samples per core
GH = H // 128         # 4 h-groups
NLB = BPC * GH        # 16 lane batches per core
PD = 128
TB = int(__import__('os').environ.get('V3_TB', '512'))  # timesteps per block
S = K * TB            # substeps per block
NP = 13               # param columns per lb

(C1, SSC, VC, VA, W2S, CP, E14, QMC, QAC, SG0, SG1, SG2, SG3) = range(NP)


def _raw_scan(eng, out3, d0, initial, d1):
    """tensor_tensor_scan with multi-free-dim APs (verified on HW): the
    recurrence chains across slice boundaries in AP iteration order."""
    nc = eng.bass
    return eng.add_instruction(
        mybir.InstTensorScalarPtr(
            name=nc.get_next_instruction_name(),
            is_tensor_tensor_scan=True,
            is_scalar_tensor_tensor=True,
            op0=OP.mult,
            op1=OP.add,
            ins=[eng.lower_ap(d0), eng.lower_ap_or_imm(initial),
                 eng.lower_ap(d1)],
            outs=[eng.lower_ap(out3)],
        )
    )


BUFS = int(__import__('os').environ.get('V3_BUFS', '3'))
SKEW = [int(x) for x in __import__('os').environ.get('V3_SKEW', '1,2,3,4,6,10').split(',')]
QMODE = __import__('os').environ.get('V3_QMODE', 'pe')   # 'pe' | 'dve'


def build_program(Tn=T, tb=TB, nlb=NLB, n_devices=NCORES):
    nblk = Tn // tb
    s_ = K * tb
    nc = bass.Bass("TRN2", target_bir_lowering=False, debug=False,
                   num_devices=n_devices)
    I_d = nc.dram_tensor("i_ca", [nlb, PD, Tn], bf16, kind="ExternalInput").ap()
    par_d = nc.dram_tensor("par", [PD, nlb * NP], f32,
                           kind="ExternalInput").ap()
    wh_d = nc.dram_tensor("wh", [PD, GH * K * PD], bf16,
                          kind="ExternalInput").ap()
    wi_d = nc.dram_tensor("wi", [PD, GH * 3 * PD], bf16,
                          kind="ExternalInput").ap()
    wid_d = nc.dram_tensor("wid", [PD, GH * 3 * PD], bf16,
                           kind="ExternalInput").ap()
    wie_d = nc.dram_tensor("wie", [PD, PD], bf16, kind="ExternalInput").ap()
    cpr_d = nc.dram_tensor("cpr", [1, GH * PD], bf16, kind="ExternalInput").ap()
    O_d = [[nc.dram_tensor(f"epsc_{lb}_{blk}", [PD, tb], bf16,
                           kind="ExternalOutput").ap()
            for blk in range(nblk)] for lb in range(nlb)]

    with ExitStack() as ctx:
        tc = ctx.enter_context(tile.TileContext(nc))
        import os as _os
        TAGB = {"zsh": 5, "sig": 5, "vp": 6, "w2": 6, "vsig": 5, "pt": 5,
                "qts": 5, "rsh": 5, "sr": 5, "saccs": 6, "racc": 6,
                "etile": 5, "plane": 3, "ptps": 2, "sacc": 1}
        for kv in _os.environ.get('V3_TAGB', '').split(';'):
            if kv:
                k, v = kv.split('='); TAGB[k] = int(v)
        apool = ctx.enter_context(tc.tile_pool(name="asig", bufs=BUFS))
        bpool = ctx.enter_context(tc.tile_pool(name="bmid", bufs=BUFS))
        cpool = ctx.enter_context(tc.tile_pool(name="ccar", bufs=BUFS))
        spool = ctx.enter_context(tc.tile_pool(name="small", bufs=BUFS + 1))
        ipool = ctx.enter_context(tc.tile_pool(name="inp", bufs=2))
        ppool = ctx.enter_context(tc.tile_pool(name="par", bufs=1))
        plpool = ctx.enter_context(tc.psum_pool(name="plps", bufs=1))
        ptpool = ctx.enter_context(tc.psum_pool(name="ptps", bufs=1))
        hpool = ctx.enter_context(tc.psum_pool(name="hps", bufs=2))

        par = ppool.tile([PD, nlb * NP], f32, tag="par")
        wh = ppool.tile([PD, GH * K * PD], bf16, tag="wh")
        wi = ppool.tile([PD, GH * 3 * PD], bf16, tag="wi")
        wid = ppool.tile([PD, GH * 3 * PD], bf16, tag="wid")
        wie = ppool.tile([PD, PD], bf16, tag="wie")
        cpr = ppool.tile([1, GH * PD], bf16, tag="cpr")
        ones = ppool.tile([1, 512], bf16, tag="ones")
        nc.vector.memset(ones[:], 1.0)
        nc.sync.dma_start(wie[:], wie_d)
        nc.sync.dma_start(cpr[:], cpr_d)
        nc.sync.dma_start(par[:], par_d)

        itile_lbs = {}
        prev_z = {}
        prev_rsh = {}
        prev_e = {}

        def pcol_of(lb):
            return lambda i: par[:, lb * NP + i:lb * NP + i + 1]

        def stage_a0(lb, blk):
            """input-chunk prefetch DMA."""
            t0 = blk * tb
            if blk == 0:
                itile_lb = ipool.tile([PD, Tn], bf16, tag="itile")
                itile_lbs[lb] = itile_lb
            nc.sync.dma_start(itile_lbs[lb][:, t0:t0 + tb],
                              I_d[lb][:, t0:t0 + tb])

        def stage_a(lb, blk):
            """z-scan (timestep granularity), V', W2 on DVE."""
            pcol = pcol_of(lb)
            t0 = blk * tb
            it = itile_lbs[lb][:, t0:t0 + tb]

            zsh = apool.tile([PD, tb + 1], bf16, tag="zsh", bufs=TAGB["zsh"])
            init = 0.0 if blk == 0 else prev_z[lb][:, tb:tb + 1]
            nc.vector.tensor_tensor_scan(
                zsh[:, 1:tb + 1], pcol(C1).to_broadcast((PD, tb)), it,
                init, OP.mult, OP.add)
            CARRY_ACT = _os.environ.get('V3_CARRY', 'dve') == 'act'
            if blk == 0:
                if CARRY_ACT:
                    nc.scalar.mul(zsh[:, 0:1], zsh[:, 1:2], 0.0)
                else:
                    nc.vector.memset(zsh[:, 0:1], 0.0)
            elif CARRY_ACT:
                nc.scalar.copy(zsh[:, 0:1], prev_z[lb][:, tb:tb + 1])
            else:
                nc.vector.tensor_copy(zsh[:, 0:1], prev_z[lb][:, tb:tb + 1])
            prev_z[lb] = zsh

            vp = spool.tile([PD, tb], bf16, tag="vp", bufs=TAGB["vp"])
            nc.vector.tensor_scalar(vp[:], it, pcol(VC), pcol(VA),
                                    OP.mult, OP.add)
            w2 = spool.tile([PD, tb], bf16, tag="w2", bufs=TAGB["w2"])
            nc.vector.tensor_scalar(w2[:], it, pcol(W2S), 0.0,
                                    OP.mult, OP.add)
            return zsh, it, vp, w2

        def stage_a2(lb, blk, zsh, it, vp, w2):
            """sigmoid-argument planes k=1..3 on PE -> PSUM."""
            g = lb % GH
            planes = []
            for j in range(3):
                pl = plpool.tile([PD, tb], f32, tag="plane", name="pl",
                                 bufs=TAGB["plane"])
                for c0 in range(0, tb, 512):
                    nc.tensor.matmul(pl[:, c0:c0 + 512],
                                     wi[:, (g * 3 + j) * PD:
                                        (g * 3 + j + 1) * PD],
                                     it[:, c0:c0 + 512],
                                     start=True, stop=False)
                    nc.tensor.matmul(pl[:, c0:c0 + 512],
                                     wid[:, (g * 3 + j) * PD:
                                         (g * 3 + j + 1) * PD],
                                     zsh[:, c0:c0 + 512],
                                     start=False, stop=True)
                planes.append(pl)
            return zsh, planes, vp, w2

        def stage_a3(lb, blk, zsh, planes, vp, w2):
            """sigmoids: k=0 from z (SBUF), k=1..3 from PE planes (scales
            folded into the PE diags)."""
            pcol = pcol_of(lb)
            sig = apool.tile([PD, s_], bf16, tag="sig", bufs=TAGB["sig"])
            sig3 = sig[:].rearrange("p (t k) -> p t k", k=K)
            nc.scalar.activation(sig3[:, :, 0], zsh[:, 0:tb], AF.Sigmoid,
                                 bias=0.0, scale=pcol(SG0))
            for j, k in enumerate((1, 2, 3)):
                nc.scalar.activation(sig3[:, :, k], planes[j][:], AF.Sigmoid,
                                     bias=0.0, scale=1.0)
            return sig, vp, w2

        def stage_b1(lb, blk, sig, vp, w2):
            """vsig halves (Pool), P halves (ACT), Q (DVE ts 4x)."""
            pcol = pcol_of(lb)
            vsig = bpool.tile([PD, s_], bf16, tag="vsig", bufs=TAGB["vsig"])
            pt = bpool.tile([PD, s_], bf16, tag="pt", bufs=TAGB["pt"])
            NSPL = int(_os.environ.get('V3_VSPL', '2'))
            th = tb // NSPL
            sh = s_ // NSPL
            PTQ = _os.environ.get('V3_PTQ', 'pe')
            for h in range(NSPL):
                nc.gpsimd.tensor_mul(
                    vsig[:, h * sh:(h + 1) * sh].rearrange(
                        "p (t k) -> p t k", k=K),
                    sig[:, h * sh:(h + 1) * sh].rearrange(
                        "p (t k) -> p t k", k=K),
                    vp[:, h * th:(h + 1) * th].unsqueeze(2).broadcast_to(
                        (PD, th, K)))
                if PTQ == 'act_dve':
                    nc.scalar.activation(pt[:, h * sh:(h + 1) * sh],
                                         vsig[:, h * sh:(h + 1) * sh],
                                         AF.Identity, bias=pcol(CP), scale=1.0)
            qt = bpool.tile([PD, s_], bf16, tag="qts", bufs=TAGB["qts"])
            if PTQ == 'act_dve':
                nc.vector.tensor_scalar(qt[:], sig[:], pcol(QMC), pcol(QAC),
                                        OP.mult, OP.add)
            elif PTQ == 'pe':
                g_ = lb % GH
                pth = []
                for h in range(2):
                    ph = ptpool.tile([PD, s_ // 2], f32, tag="ptps",
                                     name="ph", bufs=TAGB.get("ptps", 2))
                    for c0 in range(0, s_ // 2, 512):
                        o0 = h * (s_ // 2) + c0
                        nc.tensor.matmul(ph[:, c0:c0 + 512], wie[:],
                                         vsig[:, o0:o0 + 512],
                                         start=True, stop=False)
                        nc.tensor.matmul(ph[:, c0:c0 + 512],
                                         cpr[:, g_ * PD:(g_ + 1) * PD],
                                         ones[:], start=False, stop=True)
                    pth.append(ph)
                pt = pth
                nc.scalar.activation(qt[:], sig[:], AF.Identity,
                                     bias=pcol(QAC), scale=pcol(QMC))
            else:
                nc.vector.tensor_scalar(pt[:], vsig[:], 1.0, pcol(CP),
                                        OP.mult, OP.add)
                nc.scalar.activation(qt[:], sig[:], AF.Identity,
                                     bias=pcol(QAC), scale=pcol(QMC))
            return sig, pt, qt, w2

        def stage_b2(lb, blk, sig, pt, qt, w2):
            """R-scan (all-SBUF operands), sr."""
            rsh = cpool.tile([PD, s_ + 1], bf16, tag="rsh", bufs=TAGB["rsh"])
            init = 1.0 if blk == 0 else prev_rsh[lb][:, s_:s_ + 1]
            if isinstance(pt, list):
                h_ = s_ // 2
                nc.vector.tensor_tensor_scan(rsh[:, 1:h_ + 1], pt[0][:],
                                             qt[:, 0:h_], init,
                                             OP.mult, OP.add)
                nc.vector.tensor_tensor_scan(rsh[:, h_ + 1:s_ + 1], pt[1][:],
                                             qt[:, h_:s_], rsh[:, h_:h_ + 1],
                                             OP.mult, OP.add)
            else:
                nc.vector.tensor_tensor_scan(rsh[:, 1:s_ + 1], pt[:], qt[:],
                                             init, OP.mult, OP.add)
            CARRY_ACT = _os.environ.get('V3_CARRY', 'dve') == 'act'
            if blk == 0:
                if CARRY_ACT:
                    nc.scalar.activation(rsh[:, 0:1], rsh[:, 1:2], AF.Identity,
                                         bias=1.0, scale=0.0)
                else:
                    nc.vector.memset(rsh[:, 0:1], 1.0)
            elif CARRY_ACT:
                nc.scalar.copy(rsh[:, 0:1], prev_rsh[lb][:, s_:s_ + 1])
            else:
                nc.vector.tensor_copy(rsh[:, 0:1], prev_rsh[lb][:, s_:s_ + 1])
            prev_rsh[lb] = rsh
            sr = bpool.tile([PD, s_], bf16, tag="sr", bufs=TAGB["sr"])
            nc.vector.tensor_mul(sr[:], sig[:], rsh[:, 0:s_])
            return sr, w2

        def stage_c1(lb, blk, sr, w2):
            """Horner on PE (4 accumulating diag matmuls), evacuate."""
            g = lb % GH
            srk = sr[:].rearrange("p (t k) -> p t k", k=K)
            sacc = hpool.tile([PD, tb], f32, tag="sacc", bufs=TAGB.get("sacc", 2))
            for c0 in range(0, tb, 512):
                for k in range(K):
                    w = wh[:, (g * K + k) * PD:(g * K + k + 1) * PD]
                    nc.tensor.matmul(sacc[:, c0:c0 + 512], w,
                                     srk[:, c0:c0 + 512, k],
                                     start=(k == 0), stop=(k == K - 1))
            saccs = spool.tile([PD, tb], bf16, tag="saccs",
                               bufs=TAGB["saccs"])
            nc.scalar.copy(saccs[:], sacc[:])
            return saccs, w2

        rc_i = [0]

        def stage_c2(lb, blk, saccs, w2):
            """racc."""
            racc = spool.tile([PD, tb], bf16, tag="racc", bufs=TAGB["racc"])
            mode = _os.environ.get('V3_RACC', 'pool')
            if mode == 'alt':
                reng = nc.gpsimd if rc_i[0] % 2 == 0 else nc.vector
                rc_i[0] += 1
            elif mode == 'pool':
                reng = nc.gpsimd
            else:
                reng = nc.vector
            reng.tensor_mul(racc[:], w2[:], saccs[:])
            return racc

        def stage_c3(lb, blk, racc):
            """EPSC scan, out DMA."""
            pcol = pcol_of(lb)
            etile = cpool.tile([PD, tb], bf16, tag="etile",
                               bufs=TAGB["etile"])
            einit = 0.0 if blk == 0 else prev_e[lb][:, tb - 1:tb]
            nc.vector.tensor_tensor_scan(
                etile[:], pcol(E14).to_broadcast((PD, tb)), racc[:],
                einit, OP.mult, OP.add)
            prev_e[lb] = etile
            nc.sync.dma_start(O_d[lb][blk][:], etile[:])

        units = [(lb, blk) for lb in range(nlb) for blk in range(nblk)]
        n = len(units)
        sa2, sa3, sb1, sb2, sc1, sc2 = SKEW
        a_out, a2_out, a3_out, b1_out, b2_out, c1_out, c2_out = \
            {}, {}, {}, {}, {}, {}, {}
        for i in range(n + sc2):
            if sc2 <= i < n + sc2:
                c2_out[i - sc2] = stage_c2(*units[i - sc2],
                                           *c1_out.pop(i - sc2))
            if sb2 <= i < n + sb2:
                b2_out[i - sb2] = stage_b2(*units[i - sb2],
                                           *b1_out.pop(i - sb2))
            if sb1 <= i < n + sb1:
                b1_out[i - sb1] = stage_b1(*units[i - sb1],
                                           *a3_out.pop(i - sb1))
            if sc1 <= i < n + sc1:
                c1_out[i - sc1] = stage_c1(*units[i - sc1],
                                           *b2_out.pop(i - sc1))
            if sa3 <= i < n + sa3:
                a3_out[i - sa3] = stage_a3(*units[i - sa3],
                                           *a2_out.pop(i - sa3))
            if sa2 <= i < n + sa2:
                a2_out[i - sa2] = stage_a2(*units[i - sa2],
                                           *a_out.pop(i - sa2))
            if i == 0:
                stage_a0(*units[0])
                nc.sync.dma_start(wi[:], wi_d)
                nc.sync.dma_start(wid[:], wid_d)
            if i + 1 < n:
                stage_a0(*units[i + 1])
            if i < n:
                a_out[i] = stage_a(*units[i])
            if i == 0:
                nc.sync.dma_start(wh[:], wh_d)
            if sc2 <= i < n + sc2:
                stage_c3(*units[i - sc2], c2_out.pop(i - sc2))

    import bass_rust
    bass_rust.generate_event_semaphores(nc)
    return nc


def derive_params(log_Ca_mu, log_Ca_sigma, log_tau_Ca, log_alpha, log_tau_EPSC,
                  log_beta, presigmoid_P_rel_max, log_k_recov_min,
                  log_k_recov_delta, ode_steps):
    d = np.float64
    dt = 1.0 / int(ode_steps)
    sigma = np.exp(log_Ca_sigma.astype(d))
    tau_Ca = np.exp(log_tau_Ca.astype(d))
    alpha = np.exp(log_alpha.astype(d))
    tau_E = np.exp(log_tau_EPSC.astype(d))
    beta = np.exp(log_beta.astype(d))
    Prm = 1.0 / (1.0 + np.exp(-presigmoid_P_rel_max.astype(d)))
    k_min = np.exp(log_k_recov_min.astype(d))
    k_delta = np.exp(log_k_recov_delta.astype(d))

    e1 = 1.0 - dt / tau_E
    c1 = 1.0 - dt / tau_Ca
    S_k = np.stack([np.zeros_like(c1), np.ones_like(c1), 1.0 + c1,
                    1.0 + c1 + c1 ** 2], 0)          # [K, H]
    S4 = S_k[3] + c1 ** 3
    n = log_Ca_mu.shape[0]
    par = np.zeros((n, NP), np.float64)
    par[:, C1] = c1 ** 4                 # z-scan coefficient (timesteps)
    par[:, SSC] = dt * alpha / sigma
    par[:, VC] = -dt * Prm
    par[:, VA] = -dt * k_delta
    par[:, W2S] = -dt * beta * Prm
    par[:, CP] = 1.0 - dt * k_min
    par[:, E14] = e1 ** 4
    par[:, QMC] = dt * k_delta
    par[:, QAC] = dt * k_min
    # sig_0 = sigmoid(SG0*z); k>=1: plane_k = SGk*(z + (S_k/(c1^k S4)) I) on
    # PE with the scale folded into both diags; sig_k = sigmoid(plane_k)
    ssc = dt * alpha / sigma
    for k in range(K):
        par[:, SG0 + k] = ssc * (c1 ** k) * S4
    sg = np.stack([ssc * (c1 ** k) * S4 for k in range(K)], 0)      # [K,H]
    wi = np.stack([sg[k] * S_k[k] / ((c1 ** k) * S4)
                   for k in range(1, K)], 0)                        # [3,H]
    wz = sg[1:4]                                                    # [3,H]
    qm = dt * k_delta
    qa = dt * k_min
    hw_ = np.stack([e1 ** (3 - k) for k in range(K)], 0)   # [K, H]
    return par.astype(np.float32), qm.astype(np.float32), \
        qa.astype(np.float32), hw_.astype(np.float32), \
        wi.astype(np.float32), wz.astype(np.float32)


_PROG = None
LAST_RESULTS = None


def _get_program():
    global _PROG
    if _PROG is None:
        _PROG = build_program()
    return _PROG


def kernel(I_Ca, log_Ca_mu, log_Ca_sigma, log_tau_Ca, log_alpha, log_tau_EPSC,
           log_beta, presigmoid_P_rel_max, log_k_recov_min, log_k_recov_delta,
           ode_steps):
    assert int(ode_steps) == K
    I_Ca = np.asarray(I_Ca, np.float32)
    assert I_Ca.shape == (B, T, H)

    par_h, qm, qa, hw_, wi, wz = derive_params(
        np.asarray(log_Ca_mu), np.asarray(log_Ca_sigma), np.asarray(log_tau_Ca),
        np.asarray(log_alpha), np.asarray(log_tau_EPSC), np.asarray(log_beta),
        np.asarray(presigmoid_P_rel_max), np.asarray(log_k_recov_min),
        np.asarray(log_k_recov_delta), ode_steps)

    # params: lane-batch lb = b_local*GH + g holds lanes h = g*128 + p
    par_lb = par_h.reshape(GH, PD, NP)
    par_core = np.ascontiguousarray(
        np.broadcast_to(par_lb[None], (BPC, GH, PD, NP)).reshape(
            NLB, PD, NP).transpose(1, 0, 2).reshape(PD, NLB * NP))

    # PE weights per h-group g
    bf = ml_dtypes.bfloat16
    wh_h = np.zeros((PD, GH * K * PD), bf)
    wi_h = np.zeros((PD, GH * 3 * PD), bf)
    wid_h = np.zeros((PD, GH * 3 * PD), bf)
    wie_h = np.zeros((PD, PD), bf)
    np.fill_diagonal(wie_h, np.ones(PD, bf))
    cp_full = (1.0 - (1.0 / K) * np.exp(np.asarray(log_k_recov_min,
                                                   np.float64)))
    cpr_h = np.zeros((1, GH * PD), bf)
    cpr_h[0, :] = cp_full.astype(bf)
    for g in range(GH):
        lanes = slice(g * PD, (g + 1) * PD)
        for k in range(K):
            blockh = wh_h[:, (g * K + k) * PD:(g * K + k + 1) * PD]
            np.fill_diagonal(blockh, hw_[k, lanes].astype(bf))
        for j in range(3):
            blockw = wi_h[:, (g * 3 + j) * PD:(g * 3 + j + 1) * PD]
            np.fill_diagonal(blockw, wi[j, lanes].astype(bf))
            blockz = wid_h[:, (g * 3 + j) * PD:(g * 3 + j + 1) * PD]
            np.fill_diagonal(blockz, wz[j, lanes].astype(bf))

    nc = _get_program()
    in_maps = []
    for c in range(NCORES):
        Ic = I_Ca[c * BPC:(c + 1) * BPC]
        Ic = Ic.reshape(BPC, T, GH, PD).transpose(0, 2, 3, 1)
        in_maps.append({
            "i_ca": np.ascontiguousarray(Ic.reshape(NLB, PD, T)).astype(bf),
            "par": par_core,
            "wh": wh_h, "wi": wi_h, "wid": wid_h,
            "wie": wie_h, "cpr": cpr_h,
        })

    res = run_bass_kernel_spmd(nc, in_maps, core_ids=list(range(NCORES)))
    global LAST_RESULTS
    LAST_RESULTS = res
    nblk = T // TB
    out = np.empty((B, T, H), np.float32)
    for c in range(NCORES):
        Oc = np.stack([
            np.concatenate([res.results[c][f"epsc_{lb}_{blk}"].astype(np.float32)
                            for blk in range(nblk)], axis=1)
            for lb in range(NLB)])
        Oc = Oc.reshape(BPC, GH, PD, T)
        out[c * BPC:(c + 1) * BPC] = Oc.transpose(0, 3, 1, 2).reshape(BPC, T, H)
    return out


# revision 6
# speedup vs baseline: 1.0412x; 1.0214x over previous
"""Trainium2 Bass kernel for the FD synapse layer — v3 (engine-rebalanced).

Math (per lane h, substeps s = 4t+k, dt = 1/4):
    y_{s+1} = c1*y_s + I_t          y = (Ca-mu)/(dt*alpha), y_0 = 0
    sig_s   = sigmoid(SSC*y_s)      SSC = dt*alpha/sigma
    P_s     = cp + V'_t*sig_s       V' = -dt*Prm*I - dt*k_delta, cp = 1-dt*k_min
    Q_s     = QM*sig_s + QA         QM = dt*k_delta, QA = dt*k_min
    R_{s+1} = P_s*R_s + Q_s         R_0 = 1
    sacc_t  = sum_k e1^{3-k} sig_{t,k} R_{t,k}
    E_{t+1} = e1^4*E_t + W2_t*sacc_t    W2 = -dt*beta*Prm*I, E_0 = 0

Engine assignment (per (lb, blk) unit, tb=512 timesteps, S=2048 substeps):
    DVE : y-scan at substep granularity (raw 3-D-AP scan, d1 = I broadcast
          over k), R-scan (d1 = Q in PSUM), sr = sig*R (bf16 2x), EPSC scan
    ACT : the single sigmoid over S, V'/W2 affines of I, P = vsig + cp,
          sacc PSUM->SBUF evacuation
    Pool: vsig = sig * V'-broadcast, racc = W2*sacc
    PE  : Q = diag(QM)@sig + QA x ones -> PSUM, Horner sacc = sum_k
          diag(e1^{3-k}) @ sr-plane-k -> PSUM (accumulating diag matmuls)

I/O is bf16 (host converts); output EPSC returned as f32.
Sharding: batch 32 -> 4 samples/core, pure data parallel on 8 cores.
"""

import numpy as np
import ml_dtypes
from contextlib import ExitStack

import concourse.bass as bass
import concourse.mybir as mybir
import concourse.tile as tile
from concourse.bass_utils import run_bass_kernel_spmd

f32 = mybir.dt.float32
bf16 = mybir.dt.bfloat16
AF = mybir.ActivationFunctionType
OP = mybir.AluOpType

B, T, H = 32, 2048, 512
K = 4
NCORES = 8
BPC = B // NCORES     # 4 samples per core
GH = H // 128         # 4 h-groups
NLB = BPC * GH        # 16 lane batches per core
PD = 128
TB = int(__import__('os').environ.get('V3_TB', '512'))  # timesteps per block
S = K * TB            # substeps per block
NP = 13               # param columns per lb

(C1, SSC, VC, VA, W2S, CP, E14, QMC, QAC, SG0, SG1, SG2, SG3) = range(NP)


def _raw_scan(eng, out3, d0, initial, d1):
    """tensor_tensor_scan with multi-free-dim APs (verified on HW): the
    recurrence chains across slice boundaries in AP iteration order."""
    nc = eng.bass
    return eng.add_instruction(
        mybir.InstTensorScalarPtr(
            name=nc.get_next_instruction_name(),
            is_tensor_tensor_scan=True,
            is_scalar_tensor_tensor=True,
            op0=OP.mult,
            op1=OP.add,
            ins=[eng.lower_ap(d0), eng.lower_ap_or_imm(initial),
                 eng.lower_ap(d1)],
            outs=[eng.lower_ap(out3)],
        )
    )


BUFS = int(__import__('os').environ.get('V3_BUFS', '3'))
SKEW = [int(x) for x in __import__('os').environ.get('V3_SKEW', '1,2,3,4,6,10').split(',')]
QMODE = __import__('os').environ.get('V3_QMODE', 'pe')   # 'pe' | 'dve'


def build_program(Tn=T, tb=TB, nlb=NLB, n_devices=NCORES):
    nblk = Tn // tb
    s_ = K * tb
    nc = bass.Bass("TRN2", target_bir_lowering=False, debug=False,
                   num_devices=n_devices)
    I_d = nc.dram_tensor("i_ca", [nlb, PD, Tn], bf16, kind="ExternalInput").ap()
    par_d = nc.dram_tensor("par", [PD, nlb * NP], f32,
                           kind="ExternalInput").ap()
    wh_d = nc.dram_tensor("wh", [PD, GH * K * PD], bf16,
                          kind="ExternalInput").ap()
    wi_d = nc.dram_tensor("wi", [PD, GH * 3 * PD], bf16,
                          kind="ExternalInput").ap()
    wid_d = nc.dram_tensor("wid", [PD, GH * 3 * PD], bf16,
                           kind="ExternalInput").ap()
    wie_d = nc.dram_tensor("wie", [PD, PD], bf16, kind="ExternalInput").ap()
    cpr_d = nc.dram_tensor("cpr", [1, GH * PD], bf16, kind="ExternalInput").ap()
    O_d = [[nc.dram_tensor(f"epsc_{lb}_{blk}", [PD, tb], bf16,
                           kind="ExternalOutput").ap()
            for blk in range(nblk)] for lb in range(nlb)]

    with ExitStack() as ctx:
        tc = ctx.enter_context(tile.TileContext(nc))
        import os as _os
        TAGB = {"zsh": 5, "sig": 5, "vp": 6, "w2": 6, "vsig": 5, "pt": 5,
                "qts": 5, "rsh": 5, "sr": 5, "saccs": 6, "racc": 6,
                "etile": 5, "plane": 3, "ptps": 2, "sacc": 1}
        for kv in _os.environ.get('V3_TAGB', '').split(';'):
            if kv:
                k, v = kv.split('='); TAGB[k] = int(v)
        apool = ctx.enter_context(tc.tile_pool(name="asig", bufs=BUFS))
        bpool = ctx.enter_context(tc.tile_pool(name="bmid", bufs=BUFS))
        cpool = ctx.enter_context(tc.tile_pool(name="ccar", bufs=BUFS))
        spool = ctx.enter_context(tc.tile_pool(name="small", bufs=BUFS + 1))
        ipool = ctx.enter_context(tc.tile_pool(name="inp", bufs=2))
        ppool = ctx.enter_context(tc.tile_pool(name="par", bufs=1))
        plpool = ctx.enter_context(tc.psum_pool(name="plps", bufs=1))
        ptpool = ctx.enter_context(tc.psum_pool(name="ptps", bufs=1))
        hpool = ctx.enter_context(tc.psum_pool(name="hps", bufs=2))

        par = ppool.tile([PD, nlb * NP], f32, tag="par")
        wh = ppool.tile([PD, GH * K * PD], bf16, tag="wh")
        wi = ppool.tile([PD, GH * 3 * PD], bf16, tag="wi")
        wid = ppool.tile([PD, GH * 3 * PD], bf16, tag="wid")
        wie = ppool.tile([PD, PD], bf16, tag="wie")
        cpr = ppool.tile([1, GH * PD], bf16, tag="cpr")
        ones = ppool.tile([1, 512], bf16, tag="ones")
        nc.vector.memset(ones[:], 1.0)
        nc.sync.dma_start(wie[:], wie_d)
        nc.sync.dma_start(cpr[:], cpr_d)
        nc.sync.dma_start(par[:], par_d)

        itile_lbs = {}
        prev_z = {}
        prev_rsh = {}
        prev_e = {}

        def pcol_of(lb):
            return lambda i: par[:, lb * NP + i:lb * NP + i + 1]

        def stage_a0(lb, blk):
            """input-chunk prefetch DMA."""
            t0 = blk * tb
            if blk == 0:
                itile_lb = ipool.tile([PD, Tn], bf16, tag="itile")
                itile_lbs[lb] = itile_lb
            nc.sync.dma_start(itile_lbs[lb][:, t0:t0 + tb],
                              I_d[lb][:, t0:t0 + tb])

        def stage_a(lb, blk):
            """z-scan (timestep granularity), V', W2 on DVE."""
            pcol = pcol_of(lb)
            t0 = blk * tb
            it = itile_lbs[lb][:, t0:t0 + tb]

            zsh = apool.tile([PD, tb + 1], bf16, tag="zsh", bufs=TAGB["zsh"])
            init = 0.0 if blk == 0 else prev_z[lb][:, tb:tb + 1]
            nc.vector.tensor_tensor_scan(
                zsh[:, 1:tb + 1], pcol(C1).to_broadcast((PD, tb)), it,
                init, OP.mult, OP.add)
            CARRY_ACT = _os.environ.get('V3_CARRY', 'dve') == 'act'
            if blk == 0:
                if CARRY_ACT:
                    nc.scalar.mul(zsh[:, 0:1], zsh[:, 1:2], 0.0)
                else:
                    nc.vector.memset(zsh[:, 0:1], 0.0)
            elif CARRY_ACT:
                nc.scalar.copy(zsh[:, 0:1], prev_z[lb][:, tb:tb + 1])
            else:
                nc.vector.tensor_copy(zsh[:, 0:1], prev_z[lb][:, tb:tb + 1])
            prev_z[lb] = zsh

            vp = spool.tile([PD, tb], bf16, tag="vp", bufs=TAGB["vp"])
            nc.vector.tensor_scalar(vp[:], it, pcol(VC), pcol(VA),
                                    OP.mult, OP.add)
            w2 = spool.tile([PD, tb], bf16, tag="w2", bufs=TAGB["w2"])
            nc.vector.tensor_scalar(w2[:], it, pcol(W2S), 0.0,
                                    OP.mult, OP.add)
            return zsh, it, vp, w2

        def stage_a2(lb, blk, zsh, it, vp, w2):
            """sigmoid-argument planes k=1..3 on PE -> PSUM."""
            g = lb % GH
            planes = []
            for j in range(3):
                pl = plpool.tile([PD, tb], f32, tag="plane", name="pl",
                                 bufs=TAGB["plane"])
                for c0 in range(0, tb, 512):
                    nc.tensor.matmul(pl[:, c0:c0 + 512],
                                     wi[:, (g * 3 + j) * PD:
                                        (g * 3 + j + 1) * PD],
                                     it[:, c0:c0 + 512],
                                     start=True, stop=False)
                    nc.tensor.matmul(pl[:, c0:c0 + 512],
                                     wid[:, (g * 3 + j) * PD:
                                         (g * 3 + j + 1) * PD],
                                     zsh[:, c0:c0 + 512],
                                     start=False, stop=True)
                planes.append(pl)
            return zsh, planes, vp, w2

        def stage_a3(lb, blk, zsh, planes, vp, w2):
            """sigmoids: k=0 from z (SBUF), k=1..3 from PE planes (scales
            folded into the PE diags)."""
            pcol = pcol_of(lb)
            sig = apool.tile([PD, s_], bf16, tag="sig", bufs=TAGB["sig"])
            sig3 = sig[:].rearrange("p (t k) -> p t k", k=K)
            nc.scalar.activation(sig3[:, :, 0], zsh[:, 0:tb], AF.Sigmoid,
                                 bias=0.0, scale=pcol(SG0))
            for j, k in enumerate((1, 2, 3)):
                nc.scalar.activation(sig3[:, :, k], planes[j][:], AF.Sigmoid,
                                     bias=0.0, scale=1.0)
            return sig, vp, w2

        def stage_b1(lb, blk, sig, vp, w2):
            """vsig halves (Pool), P halves (ACT), Q (DVE ts 4x)."""
            pcol = pcol_of(lb)
            vsig = bpool.tile([PD, s_], bf16, tag="vsig", bufs=TAGB["vsig"])
            pt = bpool.tile([PD, s_], bf16, tag="pt", bufs=TAGB["pt"])
            NSPL = int(_os.environ.get('V3_VSPL', '1'))
            th = tb // NSPL
            sh = s_ // NSPL
            PTQ = _os.environ.get('V3_PTQ', 'pe')
            for h in range(NSPL):
                nc.gpsimd.tensor_mul(
                    vsig[:, h * sh:(h + 1) * sh].rearrange(
                        "p (t k) -> p t k", k=K),
                    sig[:, h * sh:(h + 1) * sh].rearrange(
                        "p (t k) -> p t k", k=K),
                    vp[:, h * th:(h + 1) * th].unsqueeze(2).broadcast_to(
                        (PD, th, K)))
                if PTQ == 'act_dve':
                    nc.scalar.activation(pt[:, h * sh:(h + 1) * sh],
                                         vsig[:, h * sh:(h + 1) * sh],
                                         AF.Identity, bias=pcol(CP), scale=1.0)
            qt = bpool.tile([PD, s_], bf16, tag="qts", bufs=TAGB["qts"])
            if PTQ == 'act_dve':
                nc.vector.tensor_scalar(qt[:], sig[:], pcol(QMC), pcol(QAC),
                                        OP.mult, OP.add)
            elif PTQ == 'pe':
                g_ = lb % GH
                pth = []
                for h in range(2):
                    ph = ptpool.tile([PD, s_ // 2], f32, tag="ptps",
                                     name="ph", bufs=TAGB.get("ptps", 2))
                    for c0 in range(0, s_ // 2, 512):
                        o0 = h * (s_ // 2) + c0
                        nc.tensor.matmul(ph[:, c0:c0 + 512], wie[:],
                                         vsig[:, o0:o0 + 512],
                                         start=True, stop=False)
                        nc.tensor.matmul(ph[:, c0:c0 + 512],
                                         cpr[:, g_ * PD:(g_ + 1) * PD],
                                         ones[:], start=False, stop=True)
                    pth.append(ph)
                pt = pth
                nc.scalar.activation(qt[:], sig[:], AF.Identity,
                                     bias=pcol(QAC), scale=pcol(QMC))
            else:
                nc.vector.tensor_scalar(pt[:], vsig[:], 1.0, pcol(CP),
                                        OP.mult, OP.add)
                nc.scalar.activation(qt[:], sig[:], AF.Identity,
                                     bias=pcol(QAC), scale=pcol(QMC))
            return sig, pt, qt, w2

        def stage_b2(lb, blk, sig, pt, qt, w2):
            """R-scan (all-SBUF operands), sr."""
            rsh = cpool.tile([PD, s_ + 1], bf16, tag="rsh", bufs=TAGB["rsh"])
            init = 1.0 if blk == 0 else prev_rsh[lb][:, s_:s_ + 1]
            if isinstance(pt, list):
                h_ = s_ // 2
                nc.vector.tensor_tensor_scan(rsh[:, 1:h_ + 1], pt[0][:],
                                             qt[:, 0:h_], init,
                                             OP.mult, OP.add)
                nc.vector.tensor_tensor_scan(rsh[:, h_ + 1:s_ + 1], pt[1][:],
                                             qt[:, h_:s_], rsh[:, h_:h_ + 1],
                                             OP.mult, OP.add)
            else:
                nc.vector.tensor_tensor_scan(rsh[:, 1:s_ + 1], pt[:], qt[:],
                                             init, OP.mult, OP.add)
            CARRY_ACT = _os.environ.get('V3_CARRY', 'dve') == 'act'
            if blk == 0:
                if CARRY_ACT:
                    nc.scalar.activation(rsh[:, 0:1], rsh[:, 1:2], AF.Identity,
                                         bias=1.0, scale=0.0)
                else:
                    nc.vector.memset(rsh[:, 0:1], 1.0)
            elif CARRY_ACT:
                nc.scalar.copy(rsh[:, 0:1], prev_rsh[lb][:, s_:s_ + 1])
            else:
                nc.vector.tensor_copy(rsh[:, 0:1], prev_rsh[lb][:, s_:s_ + 1])
            prev_rsh[lb] = rsh
            sr = bpool.tile([PD, s_], bf16, tag="sr", bufs=TAGB["sr"])
            nc.vector.tensor_mul(sr[:], sig[:], rsh[:, 0:s_])
            return sr, w2

        def stage_c1(lb, blk, sr, w2):
            """Horner on PE (4 accumulating diag matmuls), evacuate."""
            g = lb % GH
            srk = sr[:].rearrange("p (t k) -> p t k", k=K)
            sacc = hpool.tile([PD, tb], f32, tag="sacc", bufs=TAGB.get("sacc", 2))
            for c0 in range(0, tb, 512):
                for k in range(K):
                    w = wh[:, (g * K + k) * PD:(g * K + k + 1) * PD]
                    nc.tensor.matmul(sacc[:, c0:c0 + 512], w,
                                     srk[:, c0:c0 + 512, k],
                                     start=(k == 0), stop=(k == K - 1))
            saccs = spool.tile([PD, tb], bf16, tag="saccs",
                               bufs=TAGB["saccs"])
            nc.scalar.copy(saccs[:], sacc[:])
            return saccs, w2

        rc_i = [0]

        def stage_c2(lb, blk, saccs, w2):
            """racc."""
            racc = spool.tile([PD, tb], bf16, tag="racc", bufs=TAGB["racc"])
            mode = _os.environ.get('V3_RACC', 'pool')
            if mode == 'alt':
                reng = nc.gpsimd if rc_i[0] % 2 == 0 else nc.vector
                rc_i[0] += 1
            elif mode == 'pool':
                reng = nc.gpsimd
            else:
                reng = nc.vector
            reng.tensor_mul(racc[:], w2[:], saccs[:])
            return racc

        def stage_c3(lb, blk, racc):
            """EPSC scan, out DMA."""
            pcol = pcol_of(lb)
            etile = cpool.tile([PD, tb], bf16, tag="etile",
                               bufs=TAGB["etile"])
            einit = 0.0 if blk == 0 else prev_e[lb][:, tb - 1:tb]
            nc.vector.tensor_tensor_scan(
                etile[:], pcol(E14).to_broadcast((PD, tb)), racc[:],
                einit, OP.mult, OP.add)
            prev_e[lb] = etile
            nc.sync.dma_start(O_d[lb][blk][:], etile[:])

        units = [(lb, blk) for lb in range(nlb) for blk in range(nblk)]
        n = len(units)
        sa2, sa3, sb1, sb2, sc1, sc2 = SKEW
        a_out, a2_out, a3_out, b1_out, b2_out, c1_out, c2_out = \
            {}, {}, {}, {}, {}, {}, {}
        for i in range(n + sc2):
            if sc2 <= i < n + sc2:
                c2_out[i - sc2] = stage_c2(*units[i - sc2],
                                           *c1_out.pop(i - sc2))
            if sb2 <= i < n + sb2:
                b2_out[i - sb2] = stage_b2(*units[i - sb2],
                                           *b1_out.pop(i - sb2))
            if sb1 <= i < n + sb1:
                b1_out[i - sb1] = stage_b1(*units[i - sb1],
                                           *a3_out.pop(i - sb1))
            if sc1 <= i < n + sc1:
                c1_out[i - sc1] = stage_c1(*units[i - sc1],
                                           *b2_out.pop(i - sc1))
            if sa3 <= i < n + sa3:
                a3_out[i - sa3] = stage_a3(*units[i - sa3],
                                           *a2_out.pop(i - sa3))
            if sa2 <= i < n + sa2:
                a2_out[i - sa2] = stage_a2(*units[i - sa2],
                                           *a_out.pop(i - sa2))
            if i == 0:
                stage_a0(*units[0])
                nc.sync.dma_start(wi[:], wi_d)
                nc.sync.dma_start(wid[:], wid_d)
            if i + 1 < n:
                stage_a0(*units[i + 1])
            if i < n:
                a_out[i] = stage_a(*units[i])
            if i == 0:
                nc.sync.dma_start(wh[:], wh_d)
            if sc2 <= i < n + sc2:
                stage_c3(*units[i - sc2], c2_out.pop(i - sc2))

    import bass_rust
    bass_rust.generate_event_semaphores(nc)
    return nc


def derive_params(log_Ca_mu, log_Ca_sigma, log_tau_Ca, log_alpha, log_tau_EPSC,
                  log_beta, presigmoid_P_rel_max, log_k_recov_min,
                  log_k_recov_delta, ode_steps):
    d = np.float64
    dt = 1.0 / int(ode_steps)
    sigma = np.exp(log_Ca_sigma.astype(d))
    tau_Ca = np.exp(log_tau_Ca.astype(d))
    alpha = np.exp(log_alpha.astype(d))
    tau_E = np.exp(log_tau_EPSC.astype(d))
    beta = np.exp(log_beta.astype(d))
    Prm = 1.0 / (1.0 + np.exp(-presigmoid_P_rel_max.astype(d)))
    k_min = np.exp(log_k_recov_min.astype(d))
    k_delta = np.exp(log_k_recov_delta.astype(d))

    e1 = 1.0 - dt / tau_E
    c1 = 1.0 - dt / tau_Ca
    S_k = np.stack([np.zeros_like(c1), np.ones_like(c1), 1.0 + c1,
                    1.0 + c1 + c1 ** 2], 0)          # [K, H]
    S4 = S_k[3] + c1 ** 3
    n = log_Ca_mu.shape[0]
    par = np.zeros((n, NP), np.float64)
    par[:, C1] = c1 ** 4                 # z-scan coefficient (timesteps)
    par[:, SSC] = dt * alpha / sigma
    par[:, VC] = -dt * Prm
    par[:, VA] = -dt * k_delta
    par[:, W2S] = -dt * beta * Prm
    par[:, CP] = 1.0 - dt * k_min
    par[:, E14] = e1 ** 4
    par[:, QMC] = dt * k_delta
    par[:, QAC] = dt * k_min
    # sig_0 = sigmoid(SG0*z); k>=1: plane_k = SGk*(z + (S_k/(c1^k S4)) I) on
    # PE with the scale folded into both diags; sig_k = sigmoid(plane_k)
    ssc = dt * alpha / sigma
    for k in range(K):
        par[:, SG0 + k] = ssc * (c1 ** k) * S4
    sg = np.stack([ssc * (c1 ** k) * S4 for k in range(K)], 0)      # [K,H]
    wi = np.stack([sg[k] * S_k[k] / ((c1 ** k) * S4)
                   for k in range(1, K)], 0)                        # [3,H]
    wz = sg[1:4]                                                    # [3,H]
    qm = dt * k_delta
    qa = dt * k_min
    hw_ = np.stack([e1 ** (3 - k) for k in range(K)], 0)   # [K, H]
    return par.astype(np.float32), qm.astype(np.float32), \
        qa.astype(np.float32), hw_.astype(np.float32), \
        wi.astype(np.float32), wz.astype(np.float32)


_PROG = None
LAST_RESULTS = None


def _get_program():
    global _PROG
    if _PROG is None:
        _PROG = build_program()
    return _PROG


def kernel(I_Ca, log_Ca_mu, log_Ca_sigma, log_tau_Ca, log_alpha, log_tau_EPSC,
           log_beta, presigmoid_P_rel_max, log_k_recov_min, log_k_recov_delta,
           ode_steps):
    assert int(ode_steps) == K
    I_Ca = np.asarray(I_Ca, np.float32)
    assert I_Ca.shape == (B, T, H)

    par_h, qm, qa, hw_, wi, wz = derive_params(
        np.asarray(log_Ca_mu), np.asarray(log_Ca_sigma), np.asarray(log_tau_Ca),
        np.asarray(log_alpha), np.asarray(log_tau_EPSC), np.asarray(log_beta),
        np.asarray(presigmoid_P_rel_max), np.asarray(log_k_recov_min),
        np.asarray(log_k_recov_delta), ode_steps)

    # params: lane-batch lb = b_local*GH + g holds lanes h = g*128 + p
    par_lb = par_h.reshape(GH, PD, NP)
    par_core = np.ascontiguousarray(
        np.broadcast_to(par_lb[None], (BPC, GH, PD, NP)).reshape(
            NLB, PD, NP).transpose(1, 0, 2).reshape(PD, NLB * NP))

    # PE weights per h-group g
    bf = ml_dtypes.bfloat16
    wh_h = np.zeros((PD, GH * K * PD), bf)
    wi_h = np.zeros((PD, GH * 3 * PD), bf)
    wid_h = np.zeros((PD, GH * 3 * PD), bf)
    wie_h = np.zeros((PD, PD), bf)
    np.fill_diagonal(wie_h, np.ones(PD, bf))
    cp_full = (1.0 - (1.0 / K) * np.exp(np.asarray(log_k_recov_min,
                                                   np.float64)))
    cpr_h = np.zeros((1, GH * PD), bf)
    cpr_h[0, :] = cp_full.astype(bf)
    for g in range(GH):
        lanes = slice(g * PD, (g + 1) * PD)
        for k in range(K):
            blockh = wh_h[:, (g * K + k) * PD:(g * K + k + 1) * PD]
            np.fill_diagonal(blockh, hw_[k, lanes].astype(bf))
        for j in range(3):
            blockw = wi_h[:, (g * 3 + j) * PD:(g * 3 + j + 1) * PD]
            np.fill_diagonal(blockw, wi[j, lanes].astype(bf))
            blockz = wid_h[:, (g * 3 + j) * PD:(g * 3 + j + 1) * PD]
            np.fill_diagonal(blockz, wz[j, lanes].astype(bf))

    nc = _get_program()
    in_maps = []
    for c in range(NCORES):
        Ic = I_Ca[c * BPC:(c + 1) * BPC]
        Ic = Ic.reshape(BPC, T, GH, PD).transpose(0, 2, 3, 1)
        in_maps.append({
            "i_ca": np.ascontiguousarray(Ic.reshape(NLB, PD, T)).astype(bf),
            "par": par_core,
            "wh": wh_h, "wi": wi_h, "wid": wid_h,
            "wie": wie_h, "cpr": cpr_h,
        })

    res = run_bass_kernel_spmd(nc, in_maps, core_ids=list(range(NCORES)))
    global LAST_RESULTS
    LAST_RESULTS = res
    nblk = T // TB
    out = np.empty((B, T, H), np.float32)
    for c in range(NCORES):
        Oc = np.stack([
            np.concatenate([res.results[c][f"epsc_{lb}_{blk}"].astype(np.float32)
                            for blk in range(nblk)], axis=1)
            for lb in range(NLB)])
        Oc = Oc.reshape(BPC, GH, PD, T)
        out[c * BPC:(c + 1) * BPC] = Oc.transpose(0, 3, 1, 2).reshape(BPC, T, H)
    return out


# revision 7
# speedup vs baseline: 1.0445x; 1.0032x over previous
"""Trainium2 Bass kernel for the FD synapse layer — v3 (engine-rebalanced).

Math (per lane h, substeps s = 4t+k, dt = 1/4):
    y_{s+1} = c1*y_s + I_t          y = (Ca-mu)/(dt*alpha), y_0 = 0
    sig_s   = sigmoid(SSC*y_s)      SSC = dt*alpha/sigma
    P_s     = cp + V'_t*sig_s       V' = -dt*Prm*I - dt*k_delta, cp = 1-dt*k_min
    Q_s     = QM*sig_s + QA         QM = dt*k_delta, QA = dt*k_min
    R_{s+1} = P_s*R_s + Q_s         R_0 = 1
    sacc_t  = sum_k e1^{3-k} sig_{t,k} R_{t,k}
    E_{t+1} = e1^4*E_t + W2_t*sacc_t    W2 = -dt*beta*Prm*I, E_0 = 0

Engine assignment (per (lb, blk) unit, tb=512 timesteps, S=2048 substeps):
    DVE : y-scan at substep granularity (raw 3-D-AP scan, d1 = I broadcast
          over k), R-scan (d1 = Q in PSUM), sr = sig*R (bf16 2x), EPSC scan
    ACT : the single sigmoid over S, V'/W2 affines of I, P = vsig + cp,
          sacc PSUM->SBUF evacuation
    Pool: vsig = sig * V'-broadcast, racc = W2*sacc
    PE  : Q = diag(QM)@sig + QA x ones -> PSUM, Horner sacc = sum_k
          diag(e1^{3-k}) @ sr-plane-k -> PSUM (accumulating diag matmuls)

I/O is bf16 (host converts); output EPSC returned as f32.
Sharding: batch 32 -> 4 samples/core, pure data parallel on 8 cores.
"""

import numpy as np
import ml_dtypes
from contextlib import ExitStack

import concourse.bass as bass
import concourse.mybir as mybir
import concourse.tile as tile
from concourse.bass_utils import run_bass_kernel_spmd

f32 = mybir.dt.float32
bf16 = mybir.dt.bfloat16
AF = mybir.ActivationFunctionType
OP = mybir.AluOpType

B, T, H = 32, 2048, 512
K = 4
NCORES = 8
BPC = B // NCORES     # 4 samples per core
GH = H // 128         # 4 h-groups
NLB = BPC * GH        # 16 lane batches per core
PD = 128
TB = int(__import__('os').environ.get('V3_TB', '512'))  # timesteps per block
S = K * TB            # substeps per block
NP = 13               # param columns per lb

(C1, SSC, VC, VA, W2S, CP, E14, QMC, QAC, SG0, SG1, SG2, SG3) = range(NP)


def _raw_scan(eng, out3, d0, initial, d1):
    """tensor_tensor_scan with multi-free-dim APs (verified on HW): the
    recurrence chains across slice boundaries in AP iteration order."""
    nc = eng.bass
    return eng.add_instruction(
        mybir.InstTensorScalarPtr(
            name=nc.get_next_instruction_name(),
            is_tensor_tensor_scan=True,
            is_scalar_tensor_tensor=True,
            op0=OP.mult,
            op1=OP.add,
            ins=[eng.lower_ap(d0), eng.lower_ap_or_imm(initial),
                 eng.lower_ap(d1)],
            outs=[eng.lower_ap(out3)],
        )
    )


BUFS = int(__import__('os').environ.get('V3_BUFS', '3'))
SKEW = [int(x) for x in __import__('os').environ.get('V3_SKEW', '1,2,3,4,6,9').split(',')]
QMODE = __import__('os').environ.get('V3_QMODE', 'pe')   # 'pe' | 'dve'


def build_program(Tn=T, tb=TB, nlb=NLB, n_devices=NCORES):
    nblk = Tn // tb
    s_ = K * tb
    nc = bass.Bass("TRN2", target_bir_lowering=False, debug=False,
                   num_devices=n_devices)
    I_d = nc.dram_tensor("i_ca", [nlb, PD, Tn], bf16, kind="ExternalInput").ap()
    par_d = nc.dram_tensor("par", [PD, nlb * NP], f32,
                           kind="ExternalInput").ap()
    wh_d = nc.dram_tensor("wh", [PD, GH * K * PD], bf16,
                          kind="ExternalInput").ap()
    wi_d = nc.dram_tensor("wi", [PD, GH * 3 * PD], bf16,
                          kind="ExternalInput").ap()
    wid_d = nc.dram_tensor("wid", [PD, GH * 3 * PD], bf16,
                           kind="ExternalInput").ap()
    wie_d = nc.dram_tensor("wie", [PD, PD], bf16, kind="ExternalInput").ap()
    cpr_d = nc.dram_tensor("cpr", [1, GH * PD], bf16, kind="ExternalInput").ap()
    O_d = [[nc.dram_tensor(f"epsc_{lb}_{blk}", [PD, tb], bf16,
                           kind="ExternalOutput").ap()
            for blk in range(nblk)] for lb in range(nlb)]

    with ExitStack() as ctx:
        tc = ctx.enter_context(tile.TileContext(nc))
        import os as _os
        TAGB = {"zsh": 5, "sig": 5, "vp": 6, "w2": 6, "vsig": 5, "pt": 5,
                "qts": 5, "rsh": 5, "sr": 5, "saccs": 6, "racc": 6,
                "etile": 5, "plane": 3, "ptps": 2, "sacc": 1}
        for kv in _os.environ.get('V3_TAGB', '').split(';'):
            if kv:
                k, v = kv.split('='); TAGB[k] = int(v)
        apool = ctx.enter_context(tc.tile_pool(name="asig", bufs=BUFS))
        bpool = ctx.enter_context(tc.tile_pool(name="bmid", bufs=BUFS))
        cpool = ctx.enter_context(tc.tile_pool(name="ccar", bufs=BUFS))
        spool = ctx.enter_context(tc.tile_pool(name="small", bufs=BUFS + 1))
        ipool = ctx.enter_context(tc.tile_pool(name="inp", bufs=2))
        ppool = ctx.enter_context(tc.tile_pool(name="par", bufs=1))
        plpool = ctx.enter_context(tc.psum_pool(name="plps", bufs=1))
        ptpool = ctx.enter_context(tc.psum_pool(name="ptps", bufs=1))
        hpool = ctx.enter_context(tc.psum_pool(name="hps", bufs=2))

        par = ppool.tile([PD, nlb * NP], f32, tag="par")
        wh = ppool.tile([PD, GH * K * PD], bf16, tag="wh")
        wi = ppool.tile([PD, GH * 3 * PD], bf16, tag="wi")
        wid = ppool.tile([PD, GH * 3 * PD], bf16, tag="wid")
        wie = ppool.tile([PD, PD], bf16, tag="wie")
        cpr = ppool.tile([1, GH * PD], bf16, tag="cpr")
        ones = ppool.tile([1, 512], bf16, tag="ones")
        nc.vector.memset(ones[:], 1.0)
        nc.sync.dma_start(wie[:], wie_d)
        nc.sync.dma_start(cpr[:], cpr_d)
        nc.sync.dma_start(par[:], par_d)

        itile_lbs = {}
        prev_z = {}
        prev_rsh = {}
        prev_e = {}

        def pcol_of(lb):
            return lambda i: par[:, lb * NP + i:lb * NP + i + 1]

        def stage_a0(lb, blk):
            """input-chunk prefetch DMA."""
            t0 = blk * tb
            if blk == 0:
                itile_lb = ipool.tile([PD, Tn], bf16, tag="itile")
                itile_lbs[lb] = itile_lb
            nc.sync.dma_start(itile_lbs[lb][:, t0:t0 + tb],
                              I_d[lb][:, t0:t0 + tb])

        def stage_a(lb, blk):
            """z-scan (timestep granularity), V', W2 on DVE."""
            pcol = pcol_of(lb)
            t0 = blk * tb
            it = itile_lbs[lb][:, t0:t0 + tb]

            zsh = apool.tile([PD, tb + 1], bf16, tag="zsh", bufs=TAGB["zsh"])
            init = 0.0 if blk == 0 else prev_z[lb][:, tb:tb + 1]
            nc.vector.tensor_tensor_scan(
                zsh[:, 1:tb + 1], pcol(C1).to_broadcast((PD, tb)), it,
                init, OP.mult, OP.add)
            CARRY_ACT = _os.environ.get('V3_CARRY', 'dve') == 'act'
            if blk == 0:
                if CARRY_ACT:
                    nc.scalar.mul(zsh[:, 0:1], zsh[:, 1:2], 0.0)
                else:
                    nc.vector.memset(zsh[:, 0:1], 0.0)
            elif CARRY_ACT:
                nc.scalar.copy(zsh[:, 0:1], prev_z[lb][:, tb:tb + 1])
            else:
                nc.vector.tensor_copy(zsh[:, 0:1], prev_z[lb][:, tb:tb + 1])
            prev_z[lb] = zsh

            vp = spool.tile([PD, tb], bf16, tag="vp", bufs=TAGB["vp"])
            nc.vector.tensor_scalar(vp[:], it, pcol(VC), pcol(VA),
                                    OP.mult, OP.add)
            w2 = spool.tile([PD, tb], bf16, tag="w2", bufs=TAGB["w2"])
            nc.vector.tensor_scalar(w2[:], it, pcol(W2S), 0.0,
                                    OP.mult, OP.add)
            return zsh, it, vp, w2

        def stage_a2(lb, blk, zsh, it, vp, w2):
            """sigmoid-argument planes k=1..3 on PE -> PSUM."""
            g = lb % GH
            planes = []
            for j in range(3):
                pl = plpool.tile([PD, tb], f32, tag="plane", name="pl",
                                 bufs=TAGB["plane"])
                for c0 in range(0, tb, 512):
                    nc.tensor.matmul(pl[:, c0:c0 + 512],
                                     wi[:, (g * 3 + j) * PD:
                                        (g * 3 + j + 1) * PD],
                                     it[:, c0:c0 + 512],
                                     start=True, stop=False)
                    nc.tensor.matmul(pl[:, c0:c0 + 512],
                                     wid[:, (g * 3 + j) * PD:
                                         (g * 3 + j + 1) * PD],
                                     zsh[:, c0:c0 + 512],
                                     start=False, stop=True)
                planes.append(pl)
            return zsh, planes, vp, w2

        def stage_a3(lb, blk, zsh, planes, vp, w2):
            """sigmoids: k=0 from z (SBUF), k=1..3 from PE planes (scales
            folded into the PE diags)."""
            pcol = pcol_of(lb)
            sig = apool.tile([PD, s_], bf16, tag="sig", bufs=TAGB["sig"])
            sig3 = sig[:].rearrange("p (t k) -> p t k", k=K)
            nc.scalar.activation(sig3[:, :, 0], zsh[:, 0:tb], AF.Sigmoid,
                                 bias=0.0, scale=pcol(SG0))
            for j, k in enumerate((1, 2, 3)):
                nc.scalar.activation(sig3[:, :, k], planes[j][:], AF.Sigmoid,
                                     bias=0.0, scale=1.0)
            return sig, vp, w2

        def stage_b1(lb, blk, sig, vp, w2):
            """vsig halves (Pool), P halves (ACT), Q (DVE ts 4x)."""
            pcol = pcol_of(lb)
            vsig = bpool.tile([PD, s_], bf16, tag="vsig", bufs=TAGB["vsig"])
            pt = bpool.tile([PD, s_], bf16, tag="pt", bufs=TAGB["pt"])
            NSPL = int(_os.environ.get('V3_VSPL', '1'))
            th = tb // NSPL
            sh = s_ // NSPL
            PTQ = _os.environ.get('V3_PTQ', 'pe')
            vt_i[0] += 1
            vten = nc.vector if vt_i[0] > n - int(
                _os.environ.get('V3_VTAIL', '0')) else nc.gpsimd
            for h in range(NSPL):
                vten.tensor_mul(
                    vsig[:, h * sh:(h + 1) * sh].rearrange(
                        "p (t k) -> p t k", k=K),
                    sig[:, h * sh:(h + 1) * sh].rearrange(
                        "p (t k) -> p t k", k=K),
                    vp[:, h * th:(h + 1) * th].unsqueeze(2).broadcast_to(
                        (PD, th, K)))
                if PTQ == 'act_dve':
                    nc.scalar.activation(pt[:, h * sh:(h + 1) * sh],
                                         vsig[:, h * sh:(h + 1) * sh],
                                         AF.Identity, bias=pcol(CP), scale=1.0)
            qt = bpool.tile([PD, s_], bf16, tag="qts", bufs=TAGB["qts"])
            if PTQ == 'act_dve':
                nc.vector.tensor_scalar(qt[:], sig[:], pcol(QMC), pcol(QAC),
                                        OP.mult, OP.add)
            elif PTQ == 'pe':
                g_ = lb % GH
                pth = []
                for h in range(2):
                    ph = ptpool.tile([PD, s_ // 2], f32, tag="ptps",
                                     name="ph", bufs=TAGB.get("ptps", 2))
                    for c0 in range(0, s_ // 2, 512):
                        o0 = h * (s_ // 2) + c0
                        nc.tensor.matmul(ph[:, c0:c0 + 512], wie[:],
                                         vsig[:, o0:o0 + 512],
                                         start=True, stop=False)
                        nc.tensor.matmul(ph[:, c0:c0 + 512],
                                         cpr[:, g_ * PD:(g_ + 1) * PD],
                                         ones[:], start=False, stop=True)
                    pth.append(ph)
                pt = pth
                nc.scalar.activation(qt[:], sig[:], AF.Identity,
                                     bias=pcol(QAC), scale=pcol(QMC))
            else:
                nc.vector.tensor_scalar(pt[:], vsig[:], 1.0, pcol(CP),
                                        OP.mult, OP.add)
                nc.scalar.activation(qt[:], sig[:], AF.Identity,
                                     bias=pcol(QAC), scale=pcol(QMC))
            return sig, pt, qt, w2

        def stage_b2(lb, blk, sig, pt, qt, w2):
            """R-scan (all-SBUF operands), sr."""
            rsh = cpool.tile([PD, s_ + 1], bf16, tag="rsh", bufs=TAGB["rsh"])
            init = 1.0 if blk == 0 else prev_rsh[lb][:, s_:s_ + 1]
            if isinstance(pt, list):
                h_ = s_ // 2
                nc.vector.tensor_tensor_scan(rsh[:, 1:h_ + 1], pt[0][:],
                                             qt[:, 0:h_], init,
                                             OP.mult, OP.add)
                nc.vector.tensor_tensor_scan(rsh[:, h_ + 1:s_ + 1], pt[1][:],
                                             qt[:, h_:s_], rsh[:, h_:h_ + 1],
                                             OP.mult, OP.add)
            else:
                nc.vector.tensor_tensor_scan(rsh[:, 1:s_ + 1], pt[:], qt[:],
                                             init, OP.mult, OP.add)
            CARRY_ACT = _os.environ.get('V3_CARRY', 'dve') == 'act'
            if blk == 0:
                if CARRY_ACT:
                    nc.scalar.activation(rsh[:, 0:1], rsh[:, 1:2], AF.Identity,
                                         bias=1.0, scale=0.0)
                else:
                    nc.vector.memset(rsh[:, 0:1], 1.0)
            elif CARRY_ACT:
                nc.scalar.copy(rsh[:, 0:1], prev_rsh[lb][:, s_:s_ + 1])
            else:
                nc.vector.tensor_copy(rsh[:, 0:1], prev_rsh[lb][:, s_:s_ + 1])
            prev_rsh[lb] = rsh
            sr = bpool.tile([PD, s_], bf16, tag="sr", bufs=TAGB["sr"])
            nc.vector.tensor_mul(sr[:], sig[:], rsh[:, 0:s_])
            return sr, w2

        def stage_c1(lb, blk, sr, w2):
            """Horner on PE (4 accumulating diag matmuls), evacuate."""
            g = lb % GH
            srk = sr[:].rearrange("p (t k) -> p t k", k=K)
            sacc = hpool.tile([PD, tb], f32, tag="sacc", bufs=TAGB.get("sacc", 2))
            for c0 in range(0, tb, 512):
                for k in range(K):
                    w = wh[:, (g * K + k) * PD:(g * K + k + 1) * PD]
                    nc.tensor.matmul(sacc[:, c0:c0 + 512], w,
                                     srk[:, c0:c0 + 512, k],
                                     start=(k == 0), stop=(k == K - 1))
            saccs = spool.tile([PD, tb], bf16, tag="saccs",
                               bufs=TAGB["saccs"])
            nc.scalar.copy(saccs[:], sacc[:])
            return saccs, w2

        rc_i = [0]
        vt_i = [0]

        def stage_c2(lb, blk, saccs, w2):
            """racc."""
            racc = spool.tile([PD, tb], bf16, tag="racc", bufs=TAGB["racc"])
            mode = _os.environ.get('V3_RACC', 'tail')
            if mode == 'alt':
                reng = nc.gpsimd if rc_i[0] % 2 == 0 else nc.vector
                rc_i[0] += 1
            elif mode == 'lbalt':
                reng = nc.gpsimd if lb % 2 == 0 else nc.vector
            elif mode == 'lbalt4':
                reng = nc.gpsimd if lb % 4 != 0 else nc.vector
            elif mode == 'tail':
                ntail = int(_os.environ.get('V3_NTAIL', '4'))
                reng = nc.vector if rc_i[0] >= n - ntail else nc.gpsimd
                rc_i[0] += 1
            elif mode == 'pool':
                reng = nc.gpsimd
            else:
                reng = nc.vector
            reng.tensor_mul(racc[:], w2[:], saccs[:])
            return racc

        def stage_c3(lb, blk, racc):
            """EPSC scan, out DMA."""
            pcol = pcol_of(lb)
            etile = cpool.tile([PD, tb], bf16, tag="etile",
                               bufs=TAGB["etile"])
            einit = 0.0 if blk == 0 else prev_e[lb][:, tb - 1:tb]
            nc.vector.tensor_tensor_scan(
                etile[:], pcol(E14).to_broadcast((PD, tb)), racc[:],
                einit, OP.mult, OP.add)
            prev_e[lb] = etile
            nc.sync.dma_start(O_d[lb][blk][:], etile[:])

        units = [(lb, blk) for lb in range(nlb) for blk in range(nblk)]
        n = len(units)
        sa2, sa3, sb1, sb2, sc1, sc2 = SKEW
        a_out, a2_out, a3_out, b1_out, b2_out, c1_out, c2_out = \
            {}, {}, {}, {}, {}, {}, {}
        for i in range(n + sc2):
            if sc2 <= i < n + sc2:
                c2_out[i - sc2] = stage_c2(*units[i - sc2],
                                           *c1_out.pop(i - sc2))
            if sb2 <= i < n + sb2:
                b2_out[i - sb2] = stage_b2(*units[i - sb2],
                                           *b1_out.pop(i - sb2))
            if sb1 <= i < n + sb1:
                b1_out[i - sb1] = stage_b1(*units[i - sb1],
                                           *a3_out.pop(i - sb1))
            if sc1 <= i < n + sc1:
                c1_out[i - sc1] = stage_c1(*units[i - sc1],
                                           *b2_out.pop(i - sc1))
            if sa3 <= i < n + sa3:
                a3_out[i - sa3] = stage_a3(*units[i - sa3],
                                           *a2_out.pop(i - sa3))
            if sa2 <= i < n + sa2:
                a2_out[i - sa2] = stage_a2(*units[i - sa2],
                                           *a_out.pop(i - sa2))
            if i == 0:
                stage_a0(*units[0])
                nc.sync.dma_start(wi[:], wi_d)
                nc.sync.dma_start(wid[:], wid_d)
            if i + 1 < n:
                stage_a0(*units[i + 1])
            if i < n:
                a_out[i] = stage_a(*units[i])
            if i == 0:
                nc.sync.dma_start(wh[:], wh_d)
            if sc2 <= i < n + sc2:
                stage_c3(*units[i - sc2], c2_out.pop(i - sc2))

    import bass_rust
    bass_rust.generate_event_semaphores(nc)
    return nc


def derive_params(log_Ca_mu, log_Ca_sigma, log_tau_Ca, log_alpha, log_tau_EPSC,
                  log_beta, presigmoid_P_rel_max, log_k_recov_min,
                  log_k_recov_delta, ode_steps):
    d = np.float64
    dt = 1.0 / int(ode_steps)
    sigma = np.exp(log_Ca_sigma.astype(d))
    tau_Ca = np.exp(log_tau_Ca.astype(d))
    alpha = np.exp(log_alpha.astype(d))
    tau_E = np.exp(log_tau_EPSC.astype(d))
    beta = np.exp(log_beta.astype(d))
    Prm = 1.0 / (1.0 + np.exp(-presigmoid_P_rel_max.astype(d)))
    k_min = np.exp(log_k_recov_min.astype(d))
    k_delta = np.exp(log_k_recov_delta.astype(d))

    e1 = 1.0 - dt / tau_E
    c1 = 1.0 - dt / tau_Ca
    S_k = np.stack([np.zeros_like(c1), np.ones_like(c1), 1.0 + c1,
                    1.0 + c1 + c1 ** 2], 0)          # [K, H]
    S4 = S_k[3] + c1 ** 3
    n = log_Ca_mu.shape[0]
    par = np.zeros((n, NP), np.float64)
    par[:, C1] = c1 ** 4                 # z-scan coefficient (timesteps)
    par[:, SSC] = dt * alpha / sigma
    par[:, VC] = -dt * Prm
    par[:, VA] = -dt * k_delta
    par[:, W2S] = -dt * beta * Prm
    par[:, CP] = 1.0 - dt * k_min
    par[:, E14] = e1 ** 4
    par[:, QMC] = dt * k_delta
    par[:, QAC] = dt * k_min
    # sig_0 = sigmoid(SG0*z); k>=1: plane_k = SGk*(z + (S_k/(c1^k S4)) I) on
    # PE with the scale folded into both diags; sig_k = sigmoid(plane_k)
    ssc = dt * alpha / sigma
    for k in range(K):
        par[:, SG0 + k] = ssc * (c1 ** k) * S4
    sg = np.stack([ssc * (c1 ** k) * S4 for k in range(K)], 0)      # [K,H]
    wi = np.stack([sg[k] * S_k[k] / ((c1 ** k) * S4)
                   for k in range(1, K)], 0)                        # [3,H]
    wz = sg[1:4]                                                    # [3,H]
    qm = dt * k_delta
    qa = dt * k_min
    hw_ = np.stack([e1 ** (3 - k) for k in range(K)], 0)   # [K, H]
    return par.astype(np.float32), qm.astype(np.float32), \
        qa.astype(np.float32), hw_.astype(np.float32), \
        wi.astype(np.float32), wz.astype(np.float32)


_PROG = None
LAST_RESULTS = None


def _get_program():
    global _PROG
    if _PROG is None:
        _PROG = build_program()
    return _PROG


def kernel(I_Ca, log_Ca_mu, log_Ca_sigma, log_tau_Ca, log_alpha, log_tau_EPSC,
           log_beta, presigmoid_P_rel_max, log_k_recov_min, log_k_recov_delta,
           ode_steps):
    assert int(ode_steps) == K
    I_Ca = np.asarray(I_Ca, np.float32)
    assert I_Ca.shape == (B, T, H)

    par_h, qm, qa, hw_, wi, wz = derive_params(
        np.asarray(log_Ca_mu), np.asarray(log_Ca_sigma), np.asarray(log_tau_Ca),
        np.asarray(log_alpha), np.asarray(log_tau_EPSC), np.asarray(log_beta),
        np.asarray(presigmoid_P_rel_max), np.asarray(log_k_recov_min),
        np.asarray(log_k_recov_delta), ode_steps)

    # params: lane-batch lb = b_local*GH + g holds lanes h = g*128 + p
    par_lb = par_h.reshape(GH, PD, NP)
    par_core = np.ascontiguousarray(
        np.broadcast_to(par_lb[None], (BPC, GH, PD, NP)).reshape(
            NLB, PD, NP).transpose(1, 0, 2).reshape(PD, NLB * NP))

    # PE weights per h-group g
    bf = ml_dtypes.bfloat16
    wh_h = np.zeros((PD, GH * K * PD), bf)
    wi_h = np.zeros((PD, GH * 3 * PD), bf)
    wid_h = np.zeros((PD, GH * 3 * PD), bf)
    wie_h = np.zeros((PD, PD), bf)
    np.fill_diagonal(wie_h, np.ones(PD, bf))
    cp_full = (1.0 - (1.0 / K) * np.exp(np.asarray(log_k_recov_min,
                                                   np.float64)))
    cpr_h = np.zeros((1, GH * PD), bf)
    cpr_h[0, :] = cp_full.astype(bf)
    for g in range(GH):
        lanes = slice(g * PD, (g + 1) * PD)
        for k in range(K):
            blockh = wh_h[:, (g * K + k) * PD:(g * K + k + 1) * PD]
            np.fill_diagonal(blockh, hw_[k, lanes].astype(bf))
        for j in range(3):
            blockw = wi_h[:, (g * 3 + j) * PD:(g * 3 + j + 1) * PD]
            np.fill_diagonal(blockw, wi[j, lanes].astype(bf))
            blockz = wid_h[:, (g * 3 + j) * PD:(g * 3 + j + 1) * PD]
            np.fill_diagonal(blockz, wz[j, lanes].astype(bf))

    nc = _get_program()
    in_maps = []
    for c in range(NCORES):
        Ic = I_Ca[c * BPC:(c + 1) * BPC]
        Ic = Ic.reshape(BPC, T, GH, PD).transpose(0, 2, 3, 1)
        in_maps.append({
            "i_ca": np.ascontiguousarray(Ic.reshape(NLB, PD, T)).astype(bf),
            "par": par_core,
            "wh": wh_h, "wi": wi_h, "wid": wid_h,
            "wie": wie_h, "cpr": cpr_h,
        })

    res = run_bass_kernel_spmd(nc, in_maps, core_ids=list(range(NCORES)))
    global LAST_RESULTS
    LAST_RESULTS = res
    nblk = T // TB
    out = np.empty((B, T, H), np.float32)
    for c in range(NCORES):
        Oc = np.stack([
            np.concatenate([res.results[c][f"epsc_{lb}_{blk}"].astype(np.float32)
                            for blk in range(nblk)], axis=1)
            for lb in range(NLB)])
        Oc = Oc.reshape(BPC, GH, PD, T)
        out[c * BPC:(c + 1) * BPC] = Oc.transpose(0, 3, 1, 2).reshape(BPC, T, H)
    return out


# revision 8
# speedup vs baseline: 1.0474x; 1.0028x over previous
"""Trainium2 Bass kernel for the FD synapse layer — v3 (engine-rebalanced).

Math (per lane h, substeps s = 4t+k, dt = 1/4):
    y_{s+1} = c1*y_s + I_t          y = (Ca-mu)/(dt*alpha), y_0 = 0
    sig_s   = sigmoid(SSC*y_s)      SSC = dt*alpha/sigma
    P_s     = cp + V'_t*sig_s       V' = -dt*Prm*I - dt*k_delta, cp = 1-dt*k_min
    Q_s     = QM*sig_s + QA         QM = dt*k_delta, QA = dt*k_min
    R_{s+1} = P_s*R_s + Q_s         R_0 = 1
    sacc_t  = sum_k e1^{3-k} sig_{t,k} R_{t,k}
    E_{t+1} = e1^4*E_t + W2_t*sacc_t    W2 = -dt*beta*Prm*I, E_0 = 0

Engine assignment (per (lb, blk) unit, tb=512 timesteps, S=2048 substeps):
    DVE : y-scan at substep granularity (raw 3-D-AP scan, d1 = I broadcast
          over k), R-scan (d1 = Q in PSUM), sr = sig*R (bf16 2x), EPSC scan
    ACT : the single sigmoid over S, V'/W2 affines of I, P = vsig + cp,
          sacc PSUM->SBUF evacuation
    Pool: vsig = sig * V'-broadcast, racc = W2*sacc
    PE  : Q = diag(QM)@sig + QA x ones -> PSUM, Horner sacc = sum_k
          diag(e1^{3-k}) @ sr-plane-k -> PSUM (accumulating diag matmuls)

I/O is bf16 (host converts); output EPSC returned as f32.
Sharding: batch 32 -> 4 samples/core, pure data parallel on 8 cores.
"""

import numpy as np
import ml_dtypes
from contextlib import ExitStack

import concourse.bass as bass
import concourse.mybir as mybir
import concourse.tile as tile
from concourse.bass_utils import run_bass_kernel_spmd

f32 = mybir.dt.float32
bf16 = mybir.dt.bfloat16
AF = mybir.ActivationFunctionType
OP = mybir.AluOpType

B, T, H = 32, 2048, 512
K = 4
NCORES = 8
BPC = B // NCORES     # 4 samples per core
GH = H // 128         # 4 h-groups
NLB = BPC * GH        # 16 lane batches per core
PD = 128
TB = int(__import__('os').environ.get('V3_TB', '512'))  # timesteps per block
S = K * TB            # substeps per block
NP = 13               # param columns per lb

(C1, SSC, VC, VA, W2S, CP, E14, QMC, QAC, SG0, SG1, SG2, SG3) = range(NP)


def _raw_scan(eng, out3, d0, initial, d1):
    """tensor_tensor_scan with multi-free-dim APs (verified on HW): the
    recurrence chains across slice boundaries in AP iteration order."""
    nc = eng.bass
    return eng.add_instruction(
        mybir.InstTensorScalarPtr(
            name=nc.get_next_instruction_name(),
            is_tensor_tensor_scan=True,
            is_scalar_tensor_tensor=True,
            op0=OP.mult,
            op1=OP.add,
            ins=[eng.lower_ap(d0), eng.lower_ap_or_imm(initial),
                 eng.lower_ap(d1)],
            outs=[eng.lower_ap(out3)],
        )
    )


BUFS = int(__import__('os').environ.get('V3_BUFS', '3'))
SKEW = [int(x) for x in __import__('os').environ.get('V3_SKEW', '1,2,3,4,6,9').split(',')]
QMODE = __import__('os').environ.get('V3_QMODE', 'pe')   # 'pe' | 'dve'


def build_program(Tn=T, tb=TB, nlb=NLB, n_devices=NCORES):
    nblk = Tn // tb
    s_ = K * tb
    nc = bass.Bass("TRN2", target_bir_lowering=False, debug=False,
                   num_devices=n_devices)
    I_d = nc.dram_tensor("i_ca", [nlb, PD, Tn], bf16, kind="ExternalInput").ap()
    par_d = nc.dram_tensor("par", [PD, nlb * NP], f32,
                           kind="ExternalInput").ap()
    wh_d = nc.dram_tensor("wh", [PD, GH * K * PD], bf16,
                          kind="ExternalInput").ap()
    wi_d = nc.dram_tensor("wi", [PD, GH * 3 * PD], bf16,
                          kind="ExternalInput").ap()
    wid_d = nc.dram_tensor("wid", [PD, GH * 3 * PD], bf16,
                           kind="ExternalInput").ap()
    wie_d = nc.dram_tensor("wie", [PD, PD], bf16, kind="ExternalInput").ap()
    cpr_d = nc.dram_tensor("cpr", [1, GH * PD], bf16, kind="ExternalInput").ap()
    O_d = [[nc.dram_tensor(f"epsc_{lb}_{blk}", [PD, tb], bf16,
                           kind="ExternalOutput").ap()
            for blk in range(nblk)] for lb in range(nlb)]

    with ExitStack() as ctx:
        tc = ctx.enter_context(tile.TileContext(nc))
        import os as _os
        TAGB = {"zsh": 5, "sig": 5, "vp": 6, "w2": 6, "vsig": 5, "pt": 5,
                "qts": 5, "rsh": 5, "sr": 5, "saccs": 6, "racc": 6,
                "etile": 5, "plane": 3, "ptps": 2, "sacc": 1}
        for kv in _os.environ.get('V3_TAGB', '').split(';'):
            if kv:
                k, v = kv.split('='); TAGB[k] = int(v)
        apool = ctx.enter_context(tc.tile_pool(name="asig", bufs=BUFS))
        bpool = ctx.enter_context(tc.tile_pool(name="bmid", bufs=BUFS))
        cpool = ctx.enter_context(tc.tile_pool(name="ccar", bufs=BUFS))
        spool = ctx.enter_context(tc.tile_pool(name="small", bufs=BUFS + 1))
        ipool = ctx.enter_context(tc.tile_pool(name="inp", bufs=2))
        ppool = ctx.enter_context(tc.tile_pool(name="par", bufs=1))
        plpool = ctx.enter_context(tc.psum_pool(name="plps", bufs=1))
        ptpool = ctx.enter_context(tc.psum_pool(name="ptps", bufs=1))
        hpool = ctx.enter_context(tc.psum_pool(name="hps", bufs=2))

        par = ppool.tile([PD, nlb * NP], f32, tag="par")
        wh = ppool.tile([PD, GH * K * PD], bf16, tag="wh")
        wi = ppool.tile([PD, GH * 3 * PD], bf16, tag="wi")
        wid = ppool.tile([PD, GH * 3 * PD], bf16, tag="wid")
        wie = ppool.tile([PD, PD], bf16, tag="wie")
        cpr = ppool.tile([1, GH * PD], bf16, tag="cpr")
        ones = ppool.tile([1, 512], bf16, tag="ones")
        nc.vector.memset(ones[:], 1.0)
        nc.sync.dma_start(wie[:], wie_d)
        nc.sync.dma_start(cpr[:], cpr_d)
        nc.sync.dma_start(par[:], par_d)

        itile_lbs = {}
        prev_z = {}
        prev_rsh = {}
        prev_e = {}

        def pcol_of(lb):
            return lambda i: par[:, lb * NP + i:lb * NP + i + 1]

        def stage_a0(lb, blk):
            """input-chunk prefetch DMA."""
            t0 = blk * tb
            if blk == 0:
                itile_lb = ipool.tile([PD, Tn], bf16, tag="itile")
                itile_lbs[lb] = itile_lb
            nc.sync.dma_start(itile_lbs[lb][:, t0:t0 + tb],
                              I_d[lb][:, t0:t0 + tb])

        def stage_a(lb, blk):
            """z-scan (timestep granularity), V', W2 on DVE."""
            pcol = pcol_of(lb)
            t0 = blk * tb
            it = itile_lbs[lb][:, t0:t0 + tb]

            zsh = apool.tile([PD, tb + 1], bf16, tag="zsh", bufs=TAGB["zsh"])
            init = 0.0 if blk == 0 else prev_z[lb][:, tb:tb + 1]
            nc.vector.tensor_tensor_scan(
                zsh[:, 1:tb + 1], pcol(C1).to_broadcast((PD, tb)), it,
                init, OP.mult, OP.add)
            CARRY_ACT = _os.environ.get('V3_CARRY', 'dve') == 'act'
            if blk == 0:
                if CARRY_ACT:
                    nc.scalar.mul(zsh[:, 0:1], zsh[:, 1:2], 0.0)
                else:
                    nc.vector.memset(zsh[:, 0:1], 0.0)
            elif CARRY_ACT:
                nc.scalar.copy(zsh[:, 0:1], prev_z[lb][:, tb:tb + 1])
            else:
                nc.vector.tensor_copy(zsh[:, 0:1], prev_z[lb][:, tb:tb + 1])
            prev_z[lb] = zsh

            vp = spool.tile([PD, tb], bf16, tag="vp", bufs=TAGB["vp"])
            nc.vector.tensor_scalar(vp[:], it, pcol(VC), pcol(VA),
                                    OP.mult, OP.add)
            w2 = spool.tile([PD, tb], bf16, tag="w2", bufs=TAGB["w2"])
            nc.vector.tensor_scalar(w2[:], it, pcol(W2S), 0.0,
                                    OP.mult, OP.add)
            return zsh, it, vp, w2

        def stage_a2(lb, blk, zsh, it, vp, w2):
            """sigmoid-argument planes k=1..3 on PE -> PSUM."""
            g = lb % GH
            planes = []
            for j in range(3):
                pl = plpool.tile([PD, tb], f32, tag="plane", name="pl",
                                 bufs=TAGB["plane"])
                for c0 in range(0, tb, 512):
                    nc.tensor.matmul(pl[:, c0:c0 + 512],
                                     wi[:, (g * 3 + j) * PD:
                                        (g * 3 + j + 1) * PD],
                                     it[:, c0:c0 + 512],
                                     start=True, stop=False)
                    nc.tensor.matmul(pl[:, c0:c0 + 512],
                                     wid[:, (g * 3 + j) * PD:
                                         (g * 3 + j + 1) * PD],
                                     zsh[:, c0:c0 + 512],
                                     start=False, stop=True)
                planes.append(pl)
            return zsh, planes, vp, w2

        def stage_a3(lb, blk, zsh, planes, vp, w2):
            """sigmoids: k=0 from z (SBUF), k=1..3 from PE planes (scales
            folded into the PE diags)."""
            pcol = pcol_of(lb)
            sig = apool.tile([PD, s_], bf16, tag="sig", bufs=TAGB["sig"])
            sig3 = sig[:].rearrange("p (t k) -> p t k", k=K)
            nc.scalar.activation(sig3[:, :, 0], zsh[:, 0:tb], AF.Sigmoid,
                                 bias=0.0, scale=pcol(SG0))
            for j, k in enumerate((1, 2, 3)):
                nc.scalar.activation(sig3[:, :, k], planes[j][:], AF.Sigmoid,
                                     bias=0.0, scale=1.0)
            return sig, vp, w2

        def stage_b1(lb, blk, sig, vp, w2):
            """vsig halves (Pool), P halves (ACT), Q (DVE ts 4x)."""
            pcol = pcol_of(lb)
            vsig = bpool.tile([PD, s_], bf16, tag="vsig", bufs=TAGB["vsig"])
            pt = bpool.tile([PD, s_], bf16, tag="pt", bufs=TAGB["pt"])
            NSPL = int(_os.environ.get('V3_VSPL', '1'))
            if (vt_i[0] >= n - int(_os.environ.get('V3_VFINE', '2'))
                    or vt_i[0] <= int(_os.environ.get('V3_VHEAD', '0'))):
                NSPL = 4
            th = tb // NSPL
            sh = s_ // NSPL
            PTQ = _os.environ.get('V3_PTQ', 'pe')
            vt_i[0] += 1
            vten = nc.vector if vt_i[0] > n - int(
                _os.environ.get('V3_VTAIL', '0')) else nc.gpsimd
            for h in range(NSPL):
                vten.tensor_mul(
                    vsig[:, h * sh:(h + 1) * sh].rearrange(
                        "p (t k) -> p t k", k=K),
                    sig[:, h * sh:(h + 1) * sh].rearrange(
                        "p (t k) -> p t k", k=K),
                    vp[:, h * th:(h + 1) * th].unsqueeze(2).broadcast_to(
                        (PD, th, K)))
                if PTQ == 'act_dve':
                    nc.scalar.activation(pt[:, h * sh:(h + 1) * sh],
                                         vsig[:, h * sh:(h + 1) * sh],
                                         AF.Identity, bias=pcol(CP), scale=1.0)
            qt = bpool.tile([PD, s_], bf16, tag="qts", bufs=TAGB["qts"])
            if PTQ == 'act_dve':
                nc.vector.tensor_scalar(qt[:], sig[:], pcol(QMC), pcol(QAC),
                                        OP.mult, OP.add)
            elif PTQ == 'pe':
                g_ = lb % GH
                pth = []
                for h in range(2):
                    ph = ptpool.tile([PD, s_ // 2], f32, tag="ptps",
                                     name="ph", bufs=TAGB.get("ptps", 2))
                    for c0 in range(0, s_ // 2, 512):
                        o0 = h * (s_ // 2) + c0
                        nc.tensor.matmul(ph[:, c0:c0 + 512], wie[:],
                                         vsig[:, o0:o0 + 512],
                                         start=True, stop=False)
                        nc.tensor.matmul(ph[:, c0:c0 + 512],
                                         cpr[:, g_ * PD:(g_ + 1) * PD],
                                         ones[:], start=False, stop=True)
                    pth.append(ph)
                pt = pth
                nc.scalar.activation(qt[:], sig[:], AF.Identity,
                                     bias=pcol(QAC), scale=pcol(QMC))
            else:
                nc.vector.tensor_scalar(pt[:], vsig[:], 1.0, pcol(CP),
                                        OP.mult, OP.add)
                nc.scalar.activation(qt[:], sig[:], AF.Identity,
                                     bias=pcol(QAC), scale=pcol(QMC))
            return sig, pt, qt, w2

        def stage_b2(lb, blk, sig, pt, qt, w2):
            """R-scan (all-SBUF operands), sr."""
            rsh = cpool.tile([PD, s_ + 1], bf16, tag="rsh", bufs=TAGB["rsh"])
            init = 1.0 if blk == 0 else prev_rsh[lb][:, s_:s_ + 1]
            if isinstance(pt, list):
                h_ = s_ // 2
                nc.vector.tensor_tensor_scan(rsh[:, 1:h_ + 1], pt[0][:],
                                             qt[:, 0:h_], init,
                                             OP.mult, OP.add)
                nc.vector.tensor_tensor_scan(rsh[:, h_ + 1:s_ + 1], pt[1][:],
                                             qt[:, h_:s_], rsh[:, h_:h_ + 1],
                                             OP.mult, OP.add)
            else:
                nc.vector.tensor_tensor_scan(rsh[:, 1:s_ + 1], pt[:], qt[:],
                                             init, OP.mult, OP.add)
            CARRY_ACT = _os.environ.get('V3_CARRY', 'dve') == 'act'
            if blk == 0:
                if CARRY_ACT:
                    nc.scalar.activation(rsh[:, 0:1], rsh[:, 1:2], AF.Identity,
                                         bias=1.0, scale=0.0)
                else:
                    nc.vector.memset(rsh[:, 0:1], 1.0)
            elif CARRY_ACT:
                nc.scalar.copy(rsh[:, 0:1], prev_rsh[lb][:, s_:s_ + 1])
            else:
                nc.vector.tensor_copy(rsh[:, 0:1], prev_rsh[lb][:, s_:s_ + 1])
            prev_rsh[lb] = rsh
            sr = bpool.tile([PD, s_], bf16, tag="sr", bufs=TAGB["sr"])
            sm = _os.environ.get('V3_SRSPL', 'tail')
            fine = sm == 'all' or (sm == 'tail' and sr_i[0] >= n - 2)
            sr_i[0] += 1
            if fine:
                h_ = s_ // 2
                nc.vector.tensor_mul(sr[:, 0:h_], sig[:, 0:h_], rsh[:, 0:h_])
                nc.vector.tensor_mul(sr[:, h_:s_], sig[:, h_:s_],
                                     rsh[:, h_:s_])
            else:
                nc.vector.tensor_mul(sr[:], sig[:], rsh[:, 0:s_])
            return sr, w2

        def stage_c1(lb, blk, sr, w2):
            """Horner on PE (4 accumulating diag matmuls), evacuate."""
            g = lb % GH
            srk = sr[:].rearrange("p (t k) -> p t k", k=K)
            sacc = hpool.tile([PD, tb], f32, tag="sacc", bufs=TAGB.get("sacc", 2))
            for c0 in range(0, tb, 512):
                for k in range(K):
                    w = wh[:, (g * K + k) * PD:(g * K + k + 1) * PD]
                    nc.tensor.matmul(sacc[:, c0:c0 + 512], w,
                                     srk[:, c0:c0 + 512, k],
                                     start=(k == 0), stop=(k == K - 1))
            saccs = spool.tile([PD, tb], bf16, tag="saccs",
                               bufs=TAGB["saccs"])
            nc.scalar.copy(saccs[:], sacc[:])
            return saccs, w2

        rc_i = [0]
        vt_i = [0]
        sr_i = [0]

        def stage_c2(lb, blk, saccs, w2):
            """racc."""
            racc = spool.tile([PD, tb], bf16, tag="racc", bufs=TAGB["racc"])
            mode = _os.environ.get('V3_RACC', 'tail')
            if mode == 'alt':
                reng = nc.gpsimd if rc_i[0] % 2 == 0 else nc.vector
                rc_i[0] += 1
            elif mode == 'lbalt':
                reng = nc.gpsimd if lb % 2 == 0 else nc.vector
            elif mode == 'lbalt4':
                reng = nc.gpsimd if lb % 4 != 0 else nc.vector
            elif mode == 'tail':
                ntail = int(_os.environ.get('V3_NTAIL', '4'))
                reng = nc.vector if rc_i[0] >= n - ntail else nc.gpsimd
                rc_i[0] += 1
            elif mode == 'pool':
                reng = nc.gpsimd
            else:
                reng = nc.vector
            reng.tensor_mul(racc[:], w2[:], saccs[:])
            return racc

        def stage_c3(lb, blk, racc):
            """EPSC scan, out DMA."""
            pcol = pcol_of(lb)
            etile = cpool.tile([PD, tb], bf16, tag="etile",
                               bufs=TAGB["etile"])
            einit = 0.0 if blk == 0 else prev_e[lb][:, tb - 1:tb]
            nc.vector.tensor_tensor_scan(
                etile[:], pcol(E14).to_broadcast((PD, tb)), racc[:],
                einit, OP.mult, OP.add)
            prev_e[lb] = etile
            nc.sync.dma_start(O_d[lb][blk][:], etile[:])

        units = [(lb, blk) for lb in range(nlb) for blk in range(nblk)]
        n = len(units)
        sa2, sa3, sb1, sb2, sc1, sc2 = SKEW
        a_out, a2_out, a3_out, b1_out, b2_out, c1_out, c2_out = \
            {}, {}, {}, {}, {}, {}, {}
        for i in range(n + sc2):
            if sc2 <= i < n + sc2:
                c2_out[i - sc2] = stage_c2(*units[i - sc2],
                                           *c1_out.pop(i - sc2))
            if sb2 <= i < n + sb2:
                b2_out[i - sb2] = stage_b2(*units[i - sb2],
                                           *b1_out.pop(i - sb2))
            if sb1 <= i < n + sb1:
                b1_out[i - sb1] = stage_b1(*units[i - sb1],
                                           *a3_out.pop(i - sb1))
            if sc1 <= i < n + sc1:
                c1_out[i - sc1] = stage_c1(*units[i - sc1],
                                           *b2_out.pop(i - sc1))
            if sa3 <= i < n + sa3:
                a3_out[i - sa3] = stage_a3(*units[i - sa3],
                                           *a2_out.pop(i - sa3))
            if sa2 <= i < n + sa2:
                a2_out[i - sa2] = stage_a2(*units[i - sa2],
                                           *a_out.pop(i - sa2))
            if i == 0:
                stage_a0(*units[0])
                nc.sync.dma_start(wi[:], wi_d)
                nc.sync.dma_start(wid[:], wid_d)
            if i + 1 < n:
                stage_a0(*units[i + 1])
            if i < n:
                a_out[i] = stage_a(*units[i])
            if i == 0:
                nc.sync.dma_start(wh[:], wh_d)
            if sc2 <= i < n + sc2:
                stage_c3(*units[i - sc2], c2_out.pop(i - sc2))

    import bass_rust
    bass_rust.generate_event_semaphores(nc)
    return nc


def derive_params(log_Ca_mu, log_Ca_sigma, log_tau_Ca, log_alpha, log_tau_EPSC,
                  log_beta, presigmoid_P_rel_max, log_k_recov_min,
                  log_k_recov_delta, ode_steps):
    d = np.float64
    dt = 1.0 / int(ode_steps)
    sigma = np.exp(log_Ca_sigma.astype(d))
    tau_Ca = np.exp(log_tau_Ca.astype(d))
    alpha = np.exp(log_alpha.astype(d))
    tau_E = np.exp(log_tau_EPSC.astype(d))
    beta = np.exp(log_beta.astype(d))
    Prm = 1.0 / (1.0 + np.exp(-presigmoid_P_rel_max.astype(d)))
    k_min = np.exp(log_k_recov_min.astype(d))
    k_delta = np.exp(log_k_recov_delta.astype(d))

    e1 = 1.0 - dt / tau_E
    c1 = 1.0 - dt / tau_Ca
    S_k = np.stack([np.zeros_like(c1), np.ones_like(c1), 1.0 + c1,
                    1.0 + c1 + c1 ** 2], 0)          # [K, H]
    S4 = S_k[3] + c1 ** 3
    n = log_Ca_mu.shape[0]
    par = np.zeros((n, NP), np.float64)
    par[:, C1] = c1 ** 4                 # z-scan coefficient (timesteps)
    par[:, SSC] = dt * alpha / sigma
    par[:, VC] = -dt * Prm
    par[:, VA] = -dt * k_delta
    par[:, W2S] = -dt * beta * Prm
    par[:, CP] = 1.0 - dt * k_min
    par[:, E14] = e1 ** 4
    par[:, QMC] = dt * k_delta
    par[:, QAC] = dt * k_min
    # sig_0 = sigmoid(SG0*z); k>=1: plane_k = SGk*(z + (S_k/(c1^k S4)) I) on
    # PE with the scale folded into both diags; sig_k = sigmoid(plane_k)
    ssc = dt * alpha / sigma
    for k in range(K):
        par[:, SG0 + k] = ssc * (c1 ** k) * S4
    sg = np.stack([ssc * (c1 ** k) * S4 for k in range(K)], 0)      # [K,H]
    wi = np.stack([sg[k] * S_k[k] / ((c1 ** k) * S4)
                   for k in range(1, K)], 0)                        # [3,H]
    wz = sg[1:4]                                                    # [3,H]
    qm = dt * k_delta
    qa = dt * k_min
    hw_ = np.stack([e1 ** (3 - k) for k in range(K)], 0)   # [K, H]
    return par.astype(np.float32), qm.astype(np.float32), \
        qa.astype(np.float32), hw_.astype(np.float32), \
        wi.astype(np.float32), wz.astype(np.float32)


_PROG = None
LAST_RESULTS = None


def _get_program():
    global _PROG
    if _PROG is None:
        _PROG = build_program()
    return _PROG


def kernel(I_Ca, log_Ca_mu, log_Ca_sigma, log_tau_Ca, log_alpha, log_tau_EPSC,
           log_beta, presigmoid_P_rel_max, log_k_recov_min, log_k_recov_delta,
           ode_steps):
    assert int(ode_steps) == K
    I_Ca = np.asarray(I_Ca, np.float32)
    assert I_Ca.shape == (B, T, H)

    par_h, qm, qa, hw_, wi, wz = derive_params(
        np.asarray(log_Ca_mu), np.asarray(log_Ca_sigma), np.asarray(log_tau_Ca),
        np.asarray(log_alpha), np.asarray(log_tau_EPSC), np.asarray(log_beta),
        np.asarray(presigmoid_P_rel_max), np.asarray(log_k_recov_min),
        np.asarray(log_k_recov_delta), ode_steps)

    # params: lane-batch lb = b_local*GH + g holds lanes h = g*128 + p
    par_lb = par_h.reshape(GH, PD, NP)
    par_core = np.ascontiguousarray(
        np.broadcast_to(par_lb[None], (BPC, GH, PD, NP)).reshape(
            NLB, PD, NP).transpose(1, 0, 2).reshape(PD, NLB * NP))

    # PE weights per h-group g
    bf = ml_dtypes.bfloat16
    wh_h = np.zeros((PD, GH * K * PD), bf)
    wi_h = np.zeros((PD, GH * 3 * PD), bf)
    wid_h = np.zeros((PD, GH * 3 * PD), bf)
    wie_h = np.zeros((PD, PD), bf)
    np.fill_diagonal(wie_h, np.ones(PD, bf))
    cp_full = (1.0 - (1.0 / K) * np.exp(np.asarray(log_k_recov_min,
                                                   np.float64)))
    cpr_h = np.zeros((1, GH * PD), bf)
    cpr_h[0, :] = cp_full.astype(bf)
    for g in range(GH):
        lanes = slice(g * PD, (g + 1) * PD)
        for k in range(K):
            blockh = wh_h[:, (g * K + k) * PD:(g * K + k + 1) * PD]
            np.fill_diagonal(blockh, hw_[k, lanes].astype(bf))
        for j in range(3):
            blockw = wi_h[:, (g * 3 + j) * PD:(g * 3 + j + 1) * PD]
            np.fill_diagonal(blockw, wi[j, lanes].astype(bf))
            blockz = wid_h[:, (g * 3 + j) * PD:(g * 3 + j + 1) * PD]
            np.fill_diagonal(blockz, wz[j, lanes].astype(bf))

    nc = _get_program()
    in_maps = []
    for c in range(NCORES):
        Ic = I_Ca[c * BPC:(c + 1) * BPC]
        Ic = Ic.reshape(BPC, T, GH, PD).transpose(0, 2, 3, 1)
        in_maps.append({
            "i_ca": np.ascontiguousarray(Ic.reshape(NLB, PD, T)).astype(bf),
            "par": par_core,
            "wh": wh_h, "wi": wi_h, "wid": wid_h,
            "wie": wie_h, "cpr": cpr_h,
        })

    res = run_bass_kernel_spmd(nc, in_maps, core_ids=list(range(NCORES)))
    global LAST_RESULTS
    LAST_RESULTS = res
    nblk = T // TB
    out = np.empty((B, T, H), np.float32)
    for c in range(NCORES):
        Oc = np.stack([
            np.concatenate([res.results[c][f"epsc_{lb}_{blk}"].astype(np.float32)
                            for blk in range(nblk)], axis=1)
            for lb in range(NLB)])
        Oc = Oc.reshape(BPC, GH, PD, T)
        out[c * BPC:(c + 1) * BPC] = Oc.transpose(0, 3, 1, 2).reshape(BPC, T, H)
    return out


# revision 9
# speedup vs baseline: 1.0571x; 1.0092x over previous
"""Trainium2 Bass kernel for the FD synapse layer — v3 (engine-rebalanced).

Math (per lane h, substeps s = 4t+k, dt = 1/4):
    y_{s+1} = c1*y_s + I_t          y = (Ca-mu)/(dt*alpha), y_0 = 0
    sig_s   = sigmoid(SSC*y_s)      SSC = dt*alpha/sigma
    P_s     = cp + V'_t*sig_s       V' = -dt*Prm*I - dt*k_delta, cp = 1-dt*k_min
    Q_s     = QM*sig_s + QA         QM = dt*k_delta, QA = dt*k_min
    R_{s+1} = P_s*R_s + Q_s         R_0 = 1
    sacc_t  = sum_k e1^{3-k} sig_{t,k} R_{t,k}
    E_{t+1} = e1^4*E_t + W2_t*sacc_t    W2 = -dt*beta*Prm*I, E_0 = 0

Engine assignment (per (lb, blk) unit, tb=512 timesteps, S=2048 substeps):
    DVE : y-scan at substep granularity (raw 3-D-AP scan, d1 = I broadcast
          over k), R-scan (d1 = Q in PSUM), sr = sig*R (bf16 2x), EPSC scan
    ACT : the single sigmoid over S, V'/W2 affines of I, P = vsig + cp,
          sacc PSUM->SBUF evacuation
    Pool: vsig = sig * V'-broadcast, racc = W2*sacc
    PE  : Q = diag(QM)@sig + QA x ones -> PSUM, Horner sacc = sum_k
          diag(e1^{3-k}) @ sr-plane-k -> PSUM (accumulating diag matmuls)

I/O is bf16 (host converts); output EPSC returned as f32.
Sharding: batch 32 -> 4 samples/core, pure data parallel on 8 cores.
"""

import numpy as np
import ml_dtypes
from contextlib import ExitStack

import concourse.bass as bass
import concourse.mybir as mybir
import concourse.tile as tile
from concourse.bass_utils import run_bass_kernel_spmd

f32 = mybir.dt.float32
bf16 = mybir.dt.bfloat16
AF = mybir.ActivationFunctionType
OP = mybir.AluOpType

B, T, H = 32, 2048, 512
K = 4
NCORES = 8
BPC = B // NCORES     # 4 samples per core
GH = H // 128         # 4 h-groups
NLB = BPC * GH        # 16 lane batches per core
PD = 128
TB = int(__import__('os').environ.get('V3_TB', '512'))  # timesteps per block
S = K * TB            # substeps per block
NP = 13               # param columns per lb

(C1, SSC, VC, VA, W2S, CP, E14, QMC, QAC, SG0, SG1, SG2, SG3) = range(NP)


def _raw_scan(eng, out3, d0, initial, d1):
    """tensor_tensor_scan with multi-free-dim APs (verified on HW): the
    recurrence chains across slice boundaries in AP iteration order."""
    nc = eng.bass
    return eng.add_instruction(
        mybir.InstTensorScalarPtr(
            name=nc.get_next_instruction_name(),
            is_tensor_tensor_scan=True,
            is_scalar_tensor_tensor=True,
            op0=OP.mult,
            op1=OP.add,
            ins=[eng.lower_ap(d0), eng.lower_ap_or_imm(initial),
                 eng.lower_ap(d1)],
            outs=[eng.lower_ap(out3)],
        )
    )


BUFS = int(__import__('os').environ.get('V3_BUFS', '3'))
SKEW = [int(x) for x in __import__('os').environ.get('V3_SKEW', '1,2,3,4,6,9').split(',')]
QMODE = __import__('os').environ.get('V3_QMODE', 'pe')   # 'pe' | 'dve'


def build_program(Tn=T, tb=TB, nlb=NLB, n_devices=NCORES):
    nblk = Tn // tb
    s_ = K * tb
    nc = bass.Bass("TRN2", target_bir_lowering=False, debug=False,
                   num_devices=n_devices)
    I_d = nc.dram_tensor("i_ca", [nlb, PD, Tn], bf16, kind="ExternalInput").ap()
    par_d = nc.dram_tensor("par", [PD, nlb * NP], f32,
                           kind="ExternalInput").ap()
    wh_d = nc.dram_tensor("wh", [PD, GH * K * PD], bf16,
                          kind="ExternalInput").ap()
    wi_d = nc.dram_tensor("wi", [PD, GH * 3 * PD], bf16,
                          kind="ExternalInput").ap()
    wid_d = nc.dram_tensor("wid", [PD, GH * 3 * PD], bf16,
                           kind="ExternalInput").ap()
    wie_d = nc.dram_tensor("wie", [PD, PD], bf16, kind="ExternalInput").ap()
    cpr_d = nc.dram_tensor("cpr", [1, GH * PD], bf16, kind="ExternalInput").ap()
    O_d = [[nc.dram_tensor(f"epsc_{lb}_{blk}", [PD, tb], bf16,
                           kind="ExternalOutput").ap()
            for blk in range(nblk)] for lb in range(nlb)]

    with ExitStack() as ctx:
        tc = ctx.enter_context(tile.TileContext(nc))
        import os as _os
        TAGB = {"zsh": 5, "sig": 5, "vp": 6, "w2": 6, "vsig": 5, "pt": 5,
                "qts": 5, "rsh": 5, "sr": 5, "saccs": 6, "racc": 6,
                "etile": 5, "plane": 3, "ptps": 2, "sacc": 1}
        for kv in _os.environ.get('V3_TAGB', '').split(';'):
            if kv:
                k, v = kv.split('='); TAGB[k] = int(v)
        apool = ctx.enter_context(tc.tile_pool(name="asig", bufs=BUFS))
        bpool = ctx.enter_context(tc.tile_pool(name="bmid", bufs=BUFS))
        cpool = ctx.enter_context(tc.tile_pool(name="ccar", bufs=BUFS))
        spool = ctx.enter_context(tc.tile_pool(name="small", bufs=BUFS + 1))
        ipool = ctx.enter_context(tc.tile_pool(name="inp", bufs=2))
        ppool = ctx.enter_context(tc.tile_pool(name="par", bufs=1))
        plpool = ctx.enter_context(tc.psum_pool(name="plps", bufs=1))
        ptpool = ctx.enter_context(tc.psum_pool(name="ptps", bufs=1))
        hpool = ctx.enter_context(tc.psum_pool(name="hps", bufs=2))

        par = ppool.tile([PD, nlb * NP], f32, tag="par")
        wh = ppool.tile([PD, GH * K * PD], bf16, tag="wh")
        wi = ppool.tile([PD, GH * 3 * PD], bf16, tag="wi")
        wid = ppool.tile([PD, GH * 3 * PD], bf16, tag="wid")
        wie = ppool.tile([PD, PD], bf16, tag="wie")
        cpr = ppool.tile([1, GH * PD], bf16, tag="cpr")
        ones = ppool.tile([1, 512], bf16, tag="ones")
        nc.vector.memset(ones[:], 1.0)
        nc.sync.dma_start(wie[:], wie_d)
        nc.sync.dma_start(cpr[:], cpr_d)
        nc.sync.dma_start(par[:], par_d)

        itile_lbs = {}
        prev_z = {}
        prev_rsh = {}
        prev_e = {}

        def pcol_of(lb):
            return lambda i: par[:, lb * NP + i:lb * NP + i + 1]

        def stage_a0(lb, blk):
            """input-chunk prefetch DMA."""
            t0 = blk * tb
            if blk == 0:
                itile_lb = ipool.tile([PD, Tn], bf16, tag="itile")
                itile_lbs[lb] = itile_lb
            nc.sync.dma_start(itile_lbs[lb][:, t0:t0 + tb],
                              I_d[lb][:, t0:t0 + tb])

        def stage_a(lb, blk):
            """z-scan (timestep granularity), V', W2 on DVE."""
            pcol = pcol_of(lb)
            t0 = blk * tb
            it = itile_lbs[lb][:, t0:t0 + tb]

            zsh = apool.tile([PD, tb + 1], bf16, tag="zsh", bufs=TAGB["zsh"])
            init = 0.0 if blk == 0 else prev_z[lb][:, tb:tb + 1]
            nc.vector.tensor_tensor_scan(
                zsh[:, 1:tb + 1], pcol(C1).to_broadcast((PD, tb)), it,
                init, OP.mult, OP.add)
            CARRY_ACT = _os.environ.get('V3_CARRY', 'dve') == 'act'
            if blk == 0:
                if CARRY_ACT:
                    nc.scalar.mul(zsh[:, 0:1], zsh[:, 1:2], 0.0)
                else:
                    nc.vector.memset(zsh[:, 0:1], 0.0)
            elif CARRY_ACT:
                nc.scalar.copy(zsh[:, 0:1], prev_z[lb][:, tb:tb + 1])
            else:
                nc.vector.tensor_copy(zsh[:, 0:1], prev_z[lb][:, tb:tb + 1])
            prev_z[lb] = zsh

            vp = spool.tile([PD, tb], bf16, tag="vp", bufs=TAGB["vp"])
            nc.vector.tensor_scalar(vp[:], it, pcol(VC), pcol(VA),
                                    OP.mult, OP.add)
            w2 = spool.tile([PD, tb], bf16, tag="w2", bufs=TAGB["w2"])
            nc.vector.tensor_scalar(w2[:], it, pcol(W2S), 0.0,
                                    OP.mult, OP.add)
            return zsh, it, vp, w2

        def stage_a2(lb, blk, zsh, it, vp, w2):
            """sigmoid-argument planes k=1..3 on PE -> PSUM."""
            g = lb % GH
            planes = []
            for j in range(3):
                pl = plpool.tile([PD, tb], f32, tag="plane", name="pl",
                                 bufs=TAGB["plane"])
                for c0 in range(0, tb, 512):
                    nc.tensor.matmul(pl[:, c0:c0 + 512],
                                     wi[:, (g * 3 + j) * PD:
                                        (g * 3 + j + 1) * PD],
                                     it[:, c0:c0 + 512],
                                     start=True, stop=False)
                    nc.tensor.matmul(pl[:, c0:c0 + 512],
                                     wid[:, (g * 3 + j) * PD:
                                         (g * 3 + j + 1) * PD],
                                     zsh[:, c0:c0 + 512],
                                     start=False, stop=True)
                planes.append(pl)
            return zsh, planes, vp, w2

        def stage_a3(lb, blk, zsh, planes, vp, w2):
            """sigmoids: k=0 from z (SBUF), k=1..3 from PE planes (scales
            folded into the PE diags)."""
            pcol = pcol_of(lb)
            sig = apool.tile([PD, s_], bf16, tag="sig", bufs=TAGB["sig"])
            sig3 = sig[:].rearrange("p (t k) -> p t k", k=K)
            nc.scalar.activation(sig3[:, :, 0], zsh[:, 0:tb], AF.Sigmoid,
                                 bias=0.0, scale=pcol(SG0))
            for j, k in enumerate((1, 2, 3)):
                nc.scalar.activation(sig3[:, :, k], planes[j][:], AF.Sigmoid,
                                     bias=0.0, scale=1.0)
            return sig, vp, w2

        def stage_b1(lb, blk, sig, vp, w2):
            """vsig halves (Pool), P halves (ACT), Q (DVE ts 4x)."""
            pcol = pcol_of(lb)
            vsig = bpool.tile([PD, s_], bf16, tag="vsig", bufs=TAGB["vsig"])
            pt = bpool.tile([PD, s_], bf16, tag="pt", bufs=TAGB["pt"])
            NSPL = int(_os.environ.get('V3_VSPL', '1'))
            if (vt_i[0] >= n - int(_os.environ.get('V3_VFINE', '2'))
                    or vt_i[0] <= int(_os.environ.get('V3_VHEAD', '0'))):
                NSPL = 4
            th = tb // NSPL
            sh = s_ // NSPL
            PTQ = _os.environ.get('V3_PTQ', 'pe')
            vt_i[0] += 1
            vten = nc.vector if vt_i[0] > n - int(
                _os.environ.get('V3_VTAIL', '0')) else nc.gpsimd
            TE = int(_os.environ.get('V3_VDVE', '16'))
            if NSPL == 1 and TE > 0:
                nc.vector.tensor_mul(
                    vsig[:, 0:TE * K].rearrange("p (t k) -> p t k", k=K),
                    sig[:, 0:TE * K].rearrange("p (t k) -> p t k", k=K),
                    vp[:, 0:TE].unsqueeze(2).broadcast_to((PD, TE, K)))
                nc.gpsimd.tensor_mul(
                    vsig[:, TE * K:s_].rearrange("p (t k) -> p t k", k=K),
                    sig[:, TE * K:s_].rearrange("p (t k) -> p t k", k=K),
                    vp[:, TE:tb].unsqueeze(2).broadcast_to(
                        (PD, tb - TE, K)))
            else:
                for h in range(NSPL):
                    vten.tensor_mul(
                        vsig[:, h * sh:(h + 1) * sh].rearrange(
                            "p (t k) -> p t k", k=K),
                        sig[:, h * sh:(h + 1) * sh].rearrange(
                            "p (t k) -> p t k", k=K),
                        vp[:, h * th:(h + 1) * th].unsqueeze(2).broadcast_to(
                            (PD, th, K)))
                if PTQ == 'act_dve':
                    nc.scalar.activation(pt[:, h * sh:(h + 1) * sh],
                                         vsig[:, h * sh:(h + 1) * sh],
                                         AF.Identity, bias=pcol(CP), scale=1.0)
            qt = bpool.tile([PD, s_], bf16, tag="qts", bufs=TAGB["qts"])
            if PTQ == 'act_dve':
                nc.vector.tensor_scalar(qt[:], sig[:], pcol(QMC), pcol(QAC),
                                        OP.mult, OP.add)
            elif PTQ == 'pe':
                g_ = lb % GH
                pth = []
                for h in range(2):
                    ph = ptpool.tile([PD, s_ // 2], f32, tag="ptps",
                                     name="ph", bufs=TAGB.get("ptps", 2))
                    for c0 in range(0, s_ // 2, 512):
                        o0 = h * (s_ // 2) + c0
                        nc.tensor.matmul(ph[:, c0:c0 + 512], wie[:],
                                         vsig[:, o0:o0 + 512],
                                         start=True, stop=False)
                        nc.tensor.matmul(ph[:, c0:c0 + 512],
                                         cpr[:, g_ * PD:(g_ + 1) * PD],
                                         ones[:], start=False, stop=True)
                    pth.append(ph)
                pt = pth
                nc.scalar.activation(qt[:], sig[:], AF.Identity,
                                     bias=pcol(QAC), scale=pcol(QMC))
            else:
                nc.vector.tensor_scalar(pt[:], vsig[:], 1.0, pcol(CP),
                                        OP.mult, OP.add)
                nc.scalar.activation(qt[:], sig[:], AF.Identity,
                                     bias=pcol(QAC), scale=pcol(QMC))
            return sig, pt, qt, w2

        def stage_b2(lb, blk, sig, pt, qt, w2):
            """R-scan (all-SBUF operands), sr."""
            rsh = cpool.tile([PD, s_ + 1], bf16, tag="rsh", bufs=TAGB["rsh"])
            init = 1.0 if blk == 0 else prev_rsh[lb][:, s_:s_ + 1]
            if isinstance(pt, list):
                h_ = s_ // 2
                nc.vector.tensor_tensor_scan(rsh[:, 1:h_ + 1], pt[0][:],
                                             qt[:, 0:h_], init,
                                             OP.mult, OP.add)
                nc.vector.tensor_tensor_scan(rsh[:, h_ + 1:s_ + 1], pt[1][:],
                                             qt[:, h_:s_], rsh[:, h_:h_ + 1],
                                             OP.mult, OP.add)
            else:
                nc.vector.tensor_tensor_scan(rsh[:, 1:s_ + 1], pt[:], qt[:],
                                             init, OP.mult, OP.add)
            CARRY_ACT = _os.environ.get('V3_CARRY', 'dve') == 'act'
            if blk == 0:
                if CARRY_ACT:
                    nc.scalar.activation(rsh[:, 0:1], rsh[:, 1:2], AF.Identity,
                                         bias=1.0, scale=0.0)
                else:
                    nc.vector.memset(rsh[:, 0:1], 1.0)
            elif CARRY_ACT:
                nc.scalar.copy(rsh[:, 0:1], prev_rsh[lb][:, s_:s_ + 1])
            else:
                nc.vector.tensor_copy(rsh[:, 0:1], prev_rsh[lb][:, s_:s_ + 1])
            prev_rsh[lb] = rsh
            sr = bpool.tile([PD, s_], bf16, tag="sr", bufs=TAGB["sr"])
            sm = _os.environ.get('V3_SRSPL', 'tail')
            fine = sm == 'all' or (sm == 'tail' and sr_i[0] >= n - 2)
            sr_i[0] += 1
            if fine:
                h_ = s_ // 2
                nc.vector.tensor_mul(sr[:, 0:h_], sig[:, 0:h_], rsh[:, 0:h_])
                nc.vector.tensor_mul(sr[:, h_:s_], sig[:, h_:s_],
                                     rsh[:, h_:s_])
            else:
                nc.vector.tensor_mul(sr[:], sig[:], rsh[:, 0:s_])
            return sr, w2

        def stage_c1(lb, blk, sr, w2):
            """Horner on PE (4 accumulating diag matmuls), evacuate."""
            g = lb % GH
            srk = sr[:].rearrange("p (t k) -> p t k", k=K)
            sacc = hpool.tile([PD, tb], f32, tag="sacc", bufs=TAGB.get("sacc", 2))
            for c0 in range(0, tb, 512):
                for k in range(K):
                    w = wh[:, (g * K + k) * PD:(g * K + k + 1) * PD]
                    nc.tensor.matmul(sacc[:, c0:c0 + 512], w,
                                     srk[:, c0:c0 + 512, k],
                                     start=(k == 0), stop=(k == K - 1))
            saccs = spool.tile([PD, tb], bf16, tag="saccs",
                               bufs=TAGB["saccs"])
            ev_i[0] += 1
            if ev_i[0] > n - int(_os.environ.get('V3_EVTAIL', '0')):
                nc.vector.tensor_copy(saccs[:], sacc[:])
            else:
                nc.scalar.copy(saccs[:], sacc[:])
            return saccs, w2

        rc_i = [0]
        vt_i = [0]
        sr_i = [0]
        ev_i = [0]

        def stage_c2(lb, blk, saccs, w2):
            """racc."""
            racc = spool.tile([PD, tb], bf16, tag="racc", bufs=TAGB["racc"])
            mode = _os.environ.get('V3_RACC', 'tail')
            if mode == 'alt':
                reng = nc.gpsimd if rc_i[0] % 2 == 0 else nc.vector
                rc_i[0] += 1
            elif mode == 'lbalt':
                reng = nc.gpsimd if lb % 2 == 0 else nc.vector
            elif mode == 'lbalt4':
                reng = nc.gpsimd if lb % 4 != 0 else nc.vector
            elif mode == 'tail':
                ntail = int(_os.environ.get('V3_NTAIL', '4'))
                reng = nc.vector if rc_i[0] >= n - ntail else nc.gpsimd
                rc_i[0] += 1
            elif mode == 'pool':
                reng = nc.gpsimd
            else:
                reng = nc.vector
            reng.tensor_mul(racc[:], w2[:], saccs[:])
            return racc

        def stage_c3(lb, blk, racc):
            """EPSC scan, out DMA."""
            pcol = pcol_of(lb)
            etile = cpool.tile([PD, tb], bf16, tag="etile",
                               bufs=TAGB["etile"])
            einit = 0.0 if blk == 0 else prev_e[lb][:, tb - 1:tb]
            nc.vector.tensor_tensor_scan(
                etile[:], pcol(E14).to_broadcast((PD, tb)), racc[:],
                einit, OP.mult, OP.add)
            prev_e[lb] = etile
            nc.sync.dma_start(O_d[lb][blk][:], etile[:])

        units = [(lb, blk) for lb in range(nlb) for blk in range(nblk)]
        n = len(units)
        sa2, sa3, sb1, sb2, sc1, sc2 = SKEW
        a_out, a2_out, a3_out, b1_out, b2_out, c1_out, c2_out = \
            {}, {}, {}, {}, {}, {}, {}
        for i in range(n + sc2):
            if sc2 <= i < n + sc2:
                c2_out[i - sc2] = stage_c2(*units[i - sc2],
                                           *c1_out.pop(i - sc2))
            if sb2 <= i < n + sb2:
                b2_out[i - sb2] = stage_b2(*units[i - sb2],
                                           *b1_out.pop(i - sb2))
            if sb1 <= i < n + sb1:
                b1_out[i - sb1] = stage_b1(*units[i - sb1],
                                           *a3_out.pop(i - sb1))
            if sc1 <= i < n + sc1:
                c1_out[i - sc1] = stage_c1(*units[i - sc1],
                                           *b2_out.pop(i - sc1))
            if sa3 <= i < n + sa3:
                a3_out[i - sa3] = stage_a3(*units[i - sa3],
                                           *a2_out.pop(i - sa3))
            if sa2 <= i < n + sa2:
                a2_out[i - sa2] = stage_a2(*units[i - sa2],
                                           *a_out.pop(i - sa2))
            if i == 0:
                stage_a0(*units[0])
                nc.sync.dma_start(wi[:], wi_d)
                nc.sync.dma_start(wid[:], wid_d)
            if i + 1 < n:
                stage_a0(*units[i + 1])
            if i < n:
                a_out[i] = stage_a(*units[i])
            if i == 0:
                nc.sync.dma_start(wh[:], wh_d)
            if sc2 <= i < n + sc2:
                stage_c3(*units[i - sc2], c2_out.pop(i - sc2))

    import bass_rust
    bass_rust.generate_event_semaphores(nc)
    return nc


def derive_params(log_Ca_mu, log_Ca_sigma, log_tau_Ca, log_alpha, log_tau_EPSC,
                  log_beta, presigmoid_P_rel_max, log_k_recov_min,
                  log_k_recov_delta, ode_steps):
    d = np.float64
    dt = 1.0 / int(ode_steps)
    sigma = np.exp(log_Ca_sigma.astype(d))
    tau_Ca = np.exp(log_tau_Ca.astype(d))
    alpha = np.exp(log_alpha.astype(d))
    tau_E = np.exp(log_tau_EPSC.astype(d))
    beta = np.exp(log_beta.astype(d))
    Prm = 1.0 / (1.0 + np.exp(-presigmoid_P_rel_max.astype(d)))
    k_min = np.exp(log_k_recov_min.astype(d))
    k_delta = np.exp(log_k_recov_delta.astype(d))

    e1 = 1.0 - dt / tau_E
    c1 = 1.0 - dt / tau_Ca
    S_k = np.stack([np.zeros_like(c1), np.ones_like(c1), 1.0 + c1,
                    1.0 + c1 + c1 ** 2], 0)          # [K, H]
    S4 = S_k[3] + c1 ** 3
    n = log_Ca_mu.shape[0]
    par = np.zeros((n, NP), np.float64)
    par[:, C1] = c1 ** 4                 # z-scan coefficient (timesteps)
    par[:, SSC] = dt * alpha / sigma
    par[:, VC] = -dt * Prm
    par[:, VA] = -dt * k_delta
    par[:, W2S] = -dt * beta * Prm
    par[:, CP] = 1.0 - dt * k_min
    par[:, E14] = e1 ** 4
    par[:, QMC] = dt * k_delta
    par[:, QAC] = dt * k_min
    # sig_0 = sigmoid(SG0*z); k>=1: plane_k = SGk*(z + (S_k/(c1^k S4)) I) on
    # PE with the scale folded into both diags; sig_k = sigmoid(plane_k)
    ssc = dt * alpha / sigma
    for k in range(K):
        par[:, SG0 + k] = ssc * (c1 ** k) * S4
    sg = np.stack([ssc * (c1 ** k) * S4 for k in range(K)], 0)      # [K,H]
    wi = np.stack([sg[k] * S_k[k] / ((c1 ** k) * S4)
                   for k in range(1, K)], 0)                        # [3,H]
    wz = sg[1:4]                                                    # [3,H]
    qm = dt * k_delta
    qa = dt * k_min
    hw_ = np.stack([e1 ** (3 - k) for k in range(K)], 0)   # [K, H]
    return par.astype(np.float32), qm.astype(np.float32), \
        qa.astype(np.float32), hw_.astype(np.float32), \
        wi.astype(np.float32), wz.astype(np.float32)


_PROG = None
LAST_RESULTS = None


def _get_program():
    global _PROG
    if _PROG is None:
        _PROG = build_program()
    return _PROG


def kernel(I_Ca, log_Ca_mu, log_Ca_sigma, log_tau_Ca, log_alpha, log_tau_EPSC,
           log_beta, presigmoid_P_rel_max, log_k_recov_min, log_k_recov_delta,
           ode_steps):
    assert int(ode_steps) == K
    I_Ca = np.asarray(I_Ca, np.float32)
    assert I_Ca.shape == (B, T, H)

    par_h, qm, qa, hw_, wi, wz = derive_params(
        np.asarray(log_Ca_mu), np.asarray(log_Ca_sigma), np.asarray(log_tau_Ca),
        np.asarray(log_alpha), np.asarray(log_tau_EPSC), np.asarray(log_beta),
        np.asarray(presigmoid_P_rel_max), np.asarray(log_k_recov_min),
        np.asarray(log_k_recov_delta), ode_steps)

    # params: lane-batch lb = b_local*GH + g holds lanes h = g*128 + p
    par_lb = par_h.reshape(GH, PD, NP)
    par_core = np.ascontiguousarray(
        np.broadcast_to(par_lb[None], (BPC, GH, PD, NP)).reshape(
            NLB, PD, NP).transpose(1, 0, 2).reshape(PD, NLB * NP))

    # PE weights per h-group g
    bf = ml_dtypes.bfloat16
    wh_h = np.zeros((PD, GH * K * PD), bf)
    wi_h = np.zeros((PD, GH * 3 * PD), bf)
    wid_h = np.zeros((PD, GH * 3 * PD), bf)
    wie_h = np.zeros((PD, PD), bf)
    np.fill_diagonal(wie_h, np.ones(PD, bf))
    cp_full = (1.0 - (1.0 / K) * np.exp(np.asarray(log_k_recov_min,
                                                   np.float64)))
    cpr_h = np.zeros((1, GH * PD), bf)
    cpr_h[0, :] = cp_full.astype(bf)
    for g in range(GH):
        lanes = slice(g * PD, (g + 1) * PD)
        for k in range(K):
            blockh = wh_h[:, (g * K + k) * PD:(g * K + k + 1) * PD]
            np.fill_diagonal(blockh, hw_[k, lanes].astype(bf))
        for j in range(3):
            blockw = wi_h[:, (g * 3 + j) * PD:(g * 3 + j + 1) * PD]
            np.fill_diagonal(blockw, wi[j, lanes].astype(bf))
            blockz = wid_h[:, (g * 3 + j) * PD:(g * 3 + j + 1) * PD]
            np.fill_diagonal(blockz, wz[j, lanes].astype(bf))

    nc = _get_program()
    in_maps = []
    for c in range(NCORES):
        Ic = I_Ca[c * BPC:(c + 1) * BPC]
        Ic = Ic.reshape(BPC, T, GH, PD).transpose(0, 2, 3, 1)
        in_maps.append({
            "i_ca": np.ascontiguousarray(Ic.reshape(NLB, PD, T)).astype(bf),
            "par": par_core,
            "wh": wh_h, "wi": wi_h, "wid": wid_h,
            "wie": wie_h, "cpr": cpr_h,
        })

    res = run_bass_kernel_spmd(nc, in_maps, core_ids=list(range(NCORES)))
    global LAST_RESULTS
    LAST_RESULTS = res
    nblk = T // TB
    out = np.empty((B, T, H), np.float32)
    for c in range(NCORES):
        Oc = np.stack([
            np.concatenate([res.results[c][f"epsc_{lb}_{blk}"].astype(np.float32)
                            for blk in range(nblk)], axis=1)
            for lb in range(NLB)])
        Oc = Oc.reshape(BPC, GH, PD, T)
        out[c * BPC:(c + 1) * BPC] = Oc.transpose(0, 3, 1, 2).reshape(BPC, T, H)
    return out


# revision 10
# speedup vs baseline: 1.0578x; 1.0007x over previous
"""Trainium2 Bass kernel for the FD synapse layer — v3 (engine-rebalanced).

Math (per lane h, substeps s = 4t+k, dt = 1/4):
    y_{s+1} = c1*y_s + I_t          y = (Ca-mu)/(dt*alpha), y_0 = 0
    sig_s   = sigmoid(SSC*y_s)      SSC = dt*alpha/sigma
    P_s     = cp + V'_t*sig_s       V' = -dt*Prm*I - dt*k_delta, cp = 1-dt*k_min
    Q_s     = QM*sig_s + QA         QM = dt*k_delta, QA = dt*k_min
    R_{s+1} = P_s*R_s + Q_s         R_0 = 1
    sacc_t  = sum_k e1^{3-k} sig_{t,k} R_{t,k}
    E_{t+1} = e1^4*E_t + W2_t*sacc_t    W2 = -dt*beta*Prm*I, E_0 = 0

Engine assignment (per (lb, blk) unit, tb=512 timesteps, S=2048 substeps):
    DVE : y-scan at substep granularity (raw 3-D-AP scan, d1 = I broadcast
          over k), R-scan (d1 = Q in PSUM), sr = sig*R (bf16 2x), EPSC scan
    ACT : the single sigmoid over S, V'/W2 affines of I, P = vsig + cp,
          sacc PSUM->SBUF evacuation
    Pool: vsig = sig * V'-broadcast, racc = W2*sacc
    PE  : Q = diag(QM)@sig + QA x ones -> PSUM, Horner sacc = sum_k
          diag(e1^{3-k}) @ sr-plane-k -> PSUM (accumulating diag matmuls)

I/O is bf16 (host converts); output EPSC returned as f32.
Sharding: batch 32 -> 4 samples/core, pure data parallel on 8 cores.
"""

import numpy as np
import ml_dtypes
from contextlib import ExitStack

import concourse.bass as bass
import concourse.mybir as mybir
import concourse.tile as tile
from concourse.bass_utils import run_bass_kernel_spmd

f32 = mybir.dt.float32
bf16 = mybir.dt.bfloat16
AF = mybir.ActivationFunctionType
OP = mybir.AluOpType

B, T, H = 32, 2048, 512
K = 4
NCORES = 8
BPC = B // NCORES     # 4 samples per core
GH = H // 128         # 4 h-groups
NLB = BPC * GH        # 16 lane batches per core
PD = 128
TB = int(__import__('os').environ.get('V3_TB', '512'))  # timesteps per block
S = K * TB            # substeps per block
NP = 13               # param columns per lb

(C1, SSC, VC, VA, W2S, CP, E14, QMC, QAC, SG0, SG1, SG2, SG3) = range(NP)


def _raw_scan(eng, out3, d0, initial, d1):
    """tensor_tensor_scan with multi-free-dim APs (verified on HW): the
    recurrence chains across slice boundaries in AP iteration order."""
    nc = eng.bass
    return eng.add_instruction(
        mybir.InstTensorScalarPtr(
            name=nc.get_next_instruction_name(),
            is_tensor_tensor_scan=True,
            is_scalar_tensor_tensor=True,
            op0=OP.mult,
            op1=OP.add,
            ins=[eng.lower_ap(d0), eng.lower_ap_or_imm(initial),
                 eng.lower_ap(d1)],
            outs=[eng.lower_ap(out3)],
        )
    )


BUFS = int(__import__('os').environ.get('V3_BUFS', '3'))
SKEW = [int(x) for x in __import__('os').environ.get('V3_SKEW', '1,2,3,4,6,9').split(',')]
QMODE = __import__('os').environ.get('V3_QMODE', 'pe')   # 'pe' | 'dve'


def build_program(Tn=T, tb=TB, nlb=NLB, n_devices=NCORES):
    nblk = Tn // tb
    s_ = K * tb
    nc = bass.Bass("TRN2", target_bir_lowering=False, debug=False,
                   num_devices=n_devices)
    I_d = nc.dram_tensor("i_ca", [nlb, PD, Tn], bf16, kind="ExternalInput").ap()
    par_d = nc.dram_tensor("par", [PD, nlb * NP], f32,
                           kind="ExternalInput").ap()
    wh_d = nc.dram_tensor("wh", [PD, GH * K * PD], bf16,
                          kind="ExternalInput").ap()
    wi_d = nc.dram_tensor("wi", [PD, GH * 3 * PD], bf16,
                          kind="ExternalInput").ap()
    wid_d = nc.dram_tensor("wid", [PD, GH * 3 * PD], bf16,
                           kind="ExternalInput").ap()
    wie_d = nc.dram_tensor("wie", [PD, PD], bf16, kind="ExternalInput").ap()
    cpr_d = nc.dram_tensor("cpr", [1, GH * PD], bf16, kind="ExternalInput").ap()
    O_d = [[nc.dram_tensor(f"epsc_{lb}_{blk}", [PD, tb], bf16,
                           kind="ExternalOutput").ap()
            for blk in range(nblk)] for lb in range(nlb)]

    with ExitStack() as ctx:
        tc = ctx.enter_context(tile.TileContext(nc))
        import os as _os
        TAGB = {"zsh": 5, "sig": 5, "vp": 6, "w2": 6, "vsig": 5, "pt": 5,
                "qts": 5, "rsh": 5, "sr": 5, "saccs": 6, "racc": 6,
                "etile": 5, "plane": 3, "ptps": 2, "sacc": 1}
        for kv in _os.environ.get('V3_TAGB', '').split(';'):
            if kv:
                k, v = kv.split('='); TAGB[k] = int(v)
        apool = ctx.enter_context(tc.tile_pool(name="asig", bufs=BUFS))
        bpool = ctx.enter_context(tc.tile_pool(name="bmid", bufs=BUFS))
        cpool = ctx.enter_context(tc.tile_pool(name="ccar", bufs=BUFS))
        spool = ctx.enter_context(tc.tile_pool(name="small", bufs=BUFS + 1))
        ipool = ctx.enter_context(tc.tile_pool(name="inp", bufs=2))
        ppool = ctx.enter_context(tc.tile_pool(name="par", bufs=1))
        plpool = ctx.enter_context(tc.psum_pool(name="plps", bufs=1))
        ptpool = ctx.enter_context(tc.psum_pool(name="ptps", bufs=1))
        hpool = ctx.enter_context(tc.psum_pool(name="hps", bufs=2))

        par = ppool.tile([PD, nlb * NP], f32, tag="par")
        wh = ppool.tile([PD, GH * K * PD], bf16, tag="wh")
        wi = ppool.tile([PD, GH * 3 * PD], bf16, tag="wi")
        wid = ppool.tile([PD, GH * 3 * PD], bf16, tag="wid")
        wie = ppool.tile([PD, PD], bf16, tag="wie")
        cpr = ppool.tile([1, GH * PD], bf16, tag="cpr")
        ones = ppool.tile([1, 512], bf16, tag="ones")
        nc.vector.memset(ones[:], 1.0)
        nc.sync.dma_start(wie[:], wie_d)
        nc.sync.dma_start(cpr[:], cpr_d)
        nc.sync.dma_start(par[:], par_d)

        itile_lbs = {}
        prev_z = {}
        prev_rsh = {}
        prev_e = {}

        def pcol_of(lb):
            return lambda i: par[:, lb * NP + i:lb * NP + i + 1]

        def stage_a0(lb, blk):
            """input-chunk prefetch DMA."""
            t0 = blk * tb
            if blk == 0:
                itile_lb = ipool.tile([PD, Tn], bf16, tag="itile")
                itile_lbs[lb] = itile_lb
            nc.sync.dma_start(itile_lbs[lb][:, t0:t0 + tb],
                              I_d[lb][:, t0:t0 + tb])

        def stage_a(lb, blk):
            """z-scan (timestep granularity), V', W2 on DVE."""
            pcol = pcol_of(lb)
            t0 = blk * tb
            it = itile_lbs[lb][:, t0:t0 + tb]

            zsh = apool.tile([PD, tb + 1], bf16, tag="zsh", bufs=TAGB["zsh"])
            init = 0.0 if blk == 0 else prev_z[lb][:, tb:tb + 1]
            nc.vector.tensor_tensor_scan(
                zsh[:, 1:tb + 1], pcol(C1).to_broadcast((PD, tb)), it,
                init, OP.mult, OP.add)
            CARRY_ACT = _os.environ.get('V3_CARRY', 'dve') == 'act'
            if blk == 0:
                if CARRY_ACT:
                    nc.scalar.mul(zsh[:, 0:1], zsh[:, 1:2], 0.0)
                else:
                    nc.vector.memset(zsh[:, 0:1], 0.0)
            elif CARRY_ACT:
                nc.scalar.copy(zsh[:, 0:1], prev_z[lb][:, tb:tb + 1])
            else:
                nc.vector.tensor_copy(zsh[:, 0:1], prev_z[lb][:, tb:tb + 1])
            prev_z[lb] = zsh

            vp = spool.tile([PD, tb], bf16, tag="vp", bufs=TAGB["vp"])
            nc.vector.tensor_scalar(vp[:], it, pcol(VC), pcol(VA),
                                    OP.mult, OP.add)
            w2 = spool.tile([PD, tb], bf16, tag="w2", bufs=TAGB["w2"])
            nc.vector.tensor_scalar(w2[:], it, pcol(W2S), 0.0,
                                    OP.mult, OP.add)
            return zsh, it, vp, w2

        def stage_a2(lb, blk, zsh, it, vp, w2):
            """sigmoid-argument planes k=1..3 on PE -> PSUM."""
            g = lb % GH
            planes = []
            for j in range(3):
                pl = plpool.tile([PD, tb], f32, tag="plane", name="pl",
                                 bufs=TAGB["plane"])
                for c0 in range(0, tb, 512):
                    nc.tensor.matmul(pl[:, c0:c0 + 512],
                                     wi[:, (g * 3 + j) * PD:
                                        (g * 3 + j + 1) * PD],
                                     it[:, c0:c0 + 512],
                                     start=True, stop=False)
                    nc.tensor.matmul(pl[:, c0:c0 + 512],
                                     wid[:, (g * 3 + j) * PD:
                                         (g * 3 + j + 1) * PD],
                                     zsh[:, c0:c0 + 512],
                                     start=False, stop=True)
                planes.append(pl)
            return zsh, planes, vp, w2

        def stage_a3(lb, blk, zsh, planes, vp, w2):
            """sigmoids: k=0 from z (SBUF), k=1..3 from PE planes (scales
            folded into the PE diags)."""
            pcol = pcol_of(lb)
            sig = apool.tile([PD, s_], bf16, tag="sig", bufs=TAGB["sig"])
            sig3 = sig[:].rearrange("p (t k) -> p t k", k=K)
            nc.scalar.activation(sig3[:, :, 0], zsh[:, 0:tb], AF.Sigmoid,
                                 bias=0.0, scale=pcol(SG0))
            for j, k in enumerate((1, 2, 3)):
                nc.scalar.activation(sig3[:, :, k], planes[j][:], AF.Sigmoid,
                                     bias=0.0, scale=1.0)
            return sig, vp, w2

        def stage_b1(lb, blk, sig, vp, w2):
            """vsig halves (Pool), P halves (ACT), Q (DVE ts 4x)."""
            pcol = pcol_of(lb)
            vsig = bpool.tile([PD, s_], bf16, tag="vsig", bufs=TAGB["vsig"])
            pt = bpool.tile([PD, s_], bf16, tag="pt", bufs=TAGB["pt"])
            NSPL = int(_os.environ.get('V3_VSPL', '1'))
            if (vt_i[0] >= n - int(_os.environ.get('V3_VFINE', '2'))
                    or vt_i[0] <= int(_os.environ.get('V3_VHEAD', '0'))):
                NSPL = 4
            th = tb // NSPL
            sh = s_ // NSPL
            PTQ = _os.environ.get('V3_PTQ', 'pe')
            vt_i[0] += 1
            vten = nc.vector if vt_i[0] > n - int(
                _os.environ.get('V3_VTAIL', '0')) else nc.gpsimd
            TE = int(_os.environ.get('V3_VDVE', '16'))
            if NSPL == 1 and TE > 0:
                nc.vector.tensor_mul(
                    vsig[:, 0:TE * K].rearrange("p (t k) -> p t k", k=K),
                    sig[:, 0:TE * K].rearrange("p (t k) -> p t k", k=K),
                    vp[:, 0:TE].unsqueeze(2).broadcast_to((PD, TE, K)))
                nc.gpsimd.tensor_mul(
                    vsig[:, TE * K:s_].rearrange("p (t k) -> p t k", k=K),
                    sig[:, TE * K:s_].rearrange("p (t k) -> p t k", k=K),
                    vp[:, TE:tb].unsqueeze(2).broadcast_to(
                        (PD, tb - TE, K)))
            else:
                for h in range(NSPL):
                    vten.tensor_mul(
                        vsig[:, h * sh:(h + 1) * sh].rearrange(
                            "p (t k) -> p t k", k=K),
                        sig[:, h * sh:(h + 1) * sh].rearrange(
                            "p (t k) -> p t k", k=K),
                        vp[:, h * th:(h + 1) * th].unsqueeze(2).broadcast_to(
                            (PD, th, K)))
                if PTQ == 'act_dve':
                    nc.scalar.activation(pt[:, h * sh:(h + 1) * sh],
                                         vsig[:, h * sh:(h + 1) * sh],
                                         AF.Identity, bias=pcol(CP), scale=1.0)
            qt = bpool.tile([PD, s_], bf16, tag="qts", bufs=TAGB["qts"])
            if PTQ == 'act_dve':
                nc.vector.tensor_scalar(qt[:], sig[:], pcol(QMC), pcol(QAC),
                                        OP.mult, OP.add)
            elif PTQ == 'pe':
                g_ = lb % GH
                pth = []
                for h in range(2):
                    ph = ptpool.tile([PD, s_ // 2], f32, tag="ptps",
                                     name="ph", bufs=TAGB.get("ptps", 2))
                    for c0 in range(0, s_ // 2, 512):
                        o0 = h * (s_ // 2) + c0
                        nc.tensor.matmul(ph[:, c0:c0 + 512], wie[:],
                                         vsig[:, o0:o0 + 512],
                                         start=True, stop=False)
                        nc.tensor.matmul(ph[:, c0:c0 + 512],
                                         cpr[:, g_ * PD:(g_ + 1) * PD],
                                         ones[:], start=False, stop=True)
                    pth.append(ph)
                pt = pth
                nc.scalar.activation(qt[:], sig[:], AF.Identity,
                                     bias=pcol(QAC), scale=pcol(QMC))
            else:
                nc.vector.tensor_scalar(pt[:], vsig[:], 1.0, pcol(CP),
                                        OP.mult, OP.add)
                nc.scalar.activation(qt[:], sig[:], AF.Identity,
                                     bias=pcol(QAC), scale=pcol(QMC))
            return sig, pt, qt, w2

        def stage_b2(lb, blk, sig, pt, qt, w2):
            """R-scan (all-SBUF operands), sr."""
            rsh = cpool.tile([PD, s_ + 1], bf16, tag="rsh", bufs=TAGB["rsh"])
            init = 1.0 if blk == 0 else prev_rsh[lb][:, s_:s_ + 1]
            if isinstance(pt, list):
                h_ = s_ // 2
                nc.vector.tensor_tensor_scan(rsh[:, 1:h_ + 1], pt[0][:],
                                             qt[:, 0:h_], init,
                                             OP.mult, OP.add)
                nc.vector.tensor_tensor_scan(rsh[:, h_ + 1:s_ + 1], pt[1][:],
                                             qt[:, h_:s_], rsh[:, h_:h_ + 1],
                                             OP.mult, OP.add)
            else:
                nc.vector.tensor_tensor_scan(rsh[:, 1:s_ + 1], pt[:], qt[:],
                                             init, OP.mult, OP.add)
            CARRY_ACT = _os.environ.get('V3_CARRY', 'dve') == 'act'
            if blk == 0:
                if CARRY_ACT:
                    nc.scalar.activation(rsh[:, 0:1], rsh[:, 1:2], AF.Identity,
                                         bias=1.0, scale=0.0)
                else:
                    nc.vector.memset(rsh[:, 0:1], 1.0)
            elif CARRY_ACT:
                nc.scalar.copy(rsh[:, 0:1], prev_rsh[lb][:, s_:s_ + 1])
            else:
                nc.vector.tensor_copy(rsh[:, 0:1], prev_rsh[lb][:, s_:s_ + 1])
            prev_rsh[lb] = rsh
            sr = bpool.tile([PD, s_], bf16, tag="sr", bufs=TAGB["sr"])
            sm = _os.environ.get('V3_SRSPL', 'tail')
            fine = sm == 'all' or (sm == 'tail' and sr_i[0] >= n - 2)
            sr_i[0] += 1
            if fine:
                h_ = s_ // 2
                nc.vector.tensor_mul(sr[:, 0:h_], sig[:, 0:h_], rsh[:, 0:h_])
                nc.vector.tensor_mul(sr[:, h_:s_], sig[:, h_:s_],
                                     rsh[:, h_:s_])
            else:
                nc.vector.tensor_mul(sr[:], sig[:], rsh[:, 0:s_])
            return sr, w2

        def stage_c1(lb, blk, sr, w2):
            """Horner on PE (4 accumulating diag matmuls), evacuate."""
            g = lb % GH
            srk = sr[:].rearrange("p (t k) -> p t k", k=K)
            sacc = hpool.tile([PD, tb], f32, tag="sacc", bufs=TAGB.get("sacc", 2))
            for c0 in range(0, tb, 512):
                for k in range(K):
                    w = wh[:, (g * K + k) * PD:(g * K + k + 1) * PD]
                    nc.tensor.matmul(sacc[:, c0:c0 + 512], w,
                                     srk[:, c0:c0 + 512, k],
                                     start=(k == 0), stop=(k == K - 1))
            saccs = spool.tile([PD, tb], bf16, tag="saccs",
                               bufs=TAGB["saccs"])
            ev_i[0] += 1
            if ev_i[0] > n - int(_os.environ.get('V3_EVTAIL', '0')):
                nc.vector.tensor_copy(saccs[:], sacc[:])
            else:
                nc.scalar.copy(saccs[:], sacc[:])
            return saccs, w2

        rc_i = [0]
        vt_i = [0]
        sr_i = [0]
        ev_i = [0]

        def stage_c2(lb, blk, saccs, w2):
            """racc."""
            racc = spool.tile([PD, tb], bf16, tag="racc", bufs=TAGB["racc"])
            mode = _os.environ.get('V3_RACC', 'tail')
            if mode == 'split':
                E = int(_os.environ.get('V3_RDVE', '64'))
                rc_i[0] += 1
                if rc_i[0] > n - 4:
                    nc.vector.tensor_mul(racc[:], w2[:], saccs[:])
                else:
                    nc.vector.tensor_mul(racc[:, 0:E], w2[:, 0:E],
                                         saccs[:, 0:E])
                    nc.gpsimd.tensor_mul(racc[:, E:tb], w2[:, E:tb],
                                         saccs[:, E:tb])
                return racc
            if mode == 'alt':
                reng = nc.gpsimd if rc_i[0] % 2 == 0 else nc.vector
                rc_i[0] += 1
            elif mode == 'lbalt':
                reng = nc.gpsimd if lb % 2 == 0 else nc.vector
            elif mode == 'lbalt4':
                reng = nc.gpsimd if lb % 4 != 0 else nc.vector
            elif mode == 'tail':
                ntail = int(_os.environ.get('V3_NTAIL', '3'))
                reng = nc.vector if rc_i[0] >= n - ntail else nc.gpsimd
                rc_i[0] += 1
            elif mode == 'pool':
                reng = nc.gpsimd
            else:
                reng = nc.vector
            reng.tensor_mul(racc[:], w2[:], saccs[:])
            return racc

        def stage_c3(lb, blk, racc):
            """EPSC scan, out DMA."""
            pcol = pcol_of(lb)
            etile = cpool.tile([PD, tb], bf16, tag="etile",
                               bufs=TAGB["etile"])
            einit = 0.0 if blk == 0 else prev_e[lb][:, tb - 1:tb]
            nc.vector.tensor_tensor_scan(
                etile[:], pcol(E14).to_broadcast((PD, tb)), racc[:],
                einit, OP.mult, OP.add)
            prev_e[lb] = etile
            nc.sync.dma_start(O_d[lb][blk][:], etile[:])

        units = [(lb, blk) for lb in range(nlb) for blk in range(nblk)]
        n = len(units)
        sa2, sa3, sb1, sb2, sc1, sc2 = SKEW
        a_out, a2_out, a3_out, b1_out, b2_out, c1_out, c2_out = \
            {}, {}, {}, {}, {}, {}, {}
        for i in range(n + sc2):
            if sc2 <= i < n + sc2:
                c2_out[i - sc2] = stage_c2(*units[i - sc2],
                                           *c1_out.pop(i - sc2))
            if sb2 <= i < n + sb2:
                b2_out[i - sb2] = stage_b2(*units[i - sb2],
                                           *b1_out.pop(i - sb2))
            if sb1 <= i < n + sb1:
                b1_out[i - sb1] = stage_b1(*units[i - sb1],
                                           *a3_out.pop(i - sb1))
            if sc1 <= i < n + sc1:
                c1_out[i - sc1] = stage_c1(*units[i - sc1],
                                           *b2_out.pop(i - sc1))
            if sa3 <= i < n + sa3:
                a3_out[i - sa3] = stage_a3(*units[i - sa3],
                                           *a2_out.pop(i - sa3))
            if sa2 <= i < n + sa2:
                a2_out[i - sa2] = stage_a2(*units[i - sa2],
                                           *a_out.pop(i - sa2))
            if i == 0:
                stage_a0(*units[0])
                nc.sync.dma_start(wi[:], wi_d)
                nc.sync.dma_start(wid[:], wid_d)
            if i + 1 < n:
                stage_a0(*units[i + 1])
            if i < n:
                a_out[i] = stage_a(*units[i])
            if i == 0:
                nc.sync.dma_start(wh[:], wh_d)
            if sc2 <= i < n + sc2:
                stage_c3(*units[i - sc2], c2_out.pop(i - sc2))

    import bass_rust
    bass_rust.generate_event_semaphores(nc)
    return nc


def derive_params(log_Ca_mu, log_Ca_sigma, log_tau_Ca, log_alpha, log_tau_EPSC,
                  log_beta, presigmoid_P_rel_max, log_k_recov_min,
                  log_k_recov_delta, ode_steps):
    d = np.float64
    dt = 1.0 / int(ode_steps)
    sigma = np.exp(log_Ca_sigma.astype(d))
    tau_Ca = np.exp(log_tau_Ca.astype(d))
    alpha = np.exp(log_alpha.astype(d))
    tau_E = np.exp(log_tau_EPSC.astype(d))
    beta = np.exp(log_beta.astype(d))
    Prm = 1.0 / (1.0 + np.exp(-presigmoid_P_rel_max.astype(d)))
    k_min = np.exp(log_k_recov_min.astype(d))
    k_delta = np.exp(log_k_recov_delta.astype(d))

    e1 = 1.0 - dt / tau_E
    c1 = 1.0 - dt / tau_Ca
    S_k = np.stack([np.zeros_like(c1), np.ones_like(c1), 1.0 + c1,
                    1.0 + c1 + c1 ** 2], 0)          # [K, H]
    S4 = S_k[3] + c1 ** 3
    n = log_Ca_mu.shape[0]
    par = np.zeros((n, NP), np.float64)
    par[:, C1] = c1 ** 4                 # z-scan coefficient (timesteps)
    par[:, SSC] = dt * alpha / sigma
    par[:, VC] = -dt * Prm
    par[:, VA] = -dt * k_delta
    par[:, W2S] = -dt * beta * Prm
    par[:, CP] = 1.0 - dt * k_min
    par[:, E14] = e1 ** 4
    par[:, QMC] = dt * k_delta
    par[:, QAC] = dt * k_min
    # sig_0 = sigmoid(SG0*z); k>=1: plane_k = SGk*(z + (S_k/(c1^k S4)) I) on
    # PE with the scale folded into both diags; sig_k = sigmoid(plane_k)
    ssc = dt * alpha / sigma
    for k in range(K):
        par[:, SG0 + k] = ssc * (c1 ** k) * S4
    sg = np.stack([ssc * (c1 ** k) * S4 for k in range(K)], 0)      # [K,H]
    wi = np.stack([sg[k] * S_k[k] / ((c1 ** k) * S4)
                   for k in range(1, K)], 0)                        # [3,H]
    wz = sg[1:4]                                                    # [3,H]
    qm = dt * k_delta
    qa = dt * k_min
    hw_ = np.stack([e1 ** (3 - k) for k in range(K)], 0)   # [K, H]
    return par.astype(np.float32), qm.astype(np.float32), \
        qa.astype(np.float32), hw_.astype(np.float32), \
        wi.astype(np.float32), wz.astype(np.float32)


_PROG = None
LAST_RESULTS = None


def _get_program():
    global _PROG
    if _PROG is None:
        _PROG = build_program()
    return _PROG


def kernel(I_Ca, log_Ca_mu, log_Ca_sigma, log_tau_Ca, log_alpha, log_tau_EPSC,
           log_beta, presigmoid_P_rel_max, log_k_recov_min, log_k_recov_delta,
           ode_steps):
    assert int(ode_steps) == K
    I_Ca = np.asarray(I_Ca, np.float32)
    assert I_Ca.shape == (B, T, H)

    par_h, qm, qa, hw_, wi, wz = derive_params(
        np.asarray(log_Ca_mu), np.asarray(log_Ca_sigma), np.asarray(log_tau_Ca),
        np.asarray(log_alpha), np.asarray(log_tau_EPSC), np.asarray(log_beta),
        np.asarray(presigmoid_P_rel_max), np.asarray(log_k_recov_min),
        np.asarray(log_k_recov_delta), ode_steps)

    # params: lane-batch lb = b_local*GH + g holds lanes h = g*128 + p
    par_lb = par_h.reshape(GH, PD, NP)
    par_core = np.ascontiguousarray(
        np.broadcast_to(par_lb[None], (BPC, GH, PD, NP)).reshape(
            NLB, PD, NP).transpose(1, 0, 2).reshape(PD, NLB * NP))

    # PE weights per h-group g
    bf = ml_dtypes.bfloat16
    wh_h = np.zeros((PD, GH * K * PD), bf)
    wi_h = np.zeros((PD, GH * 3 * PD), bf)
    wid_h = np.zeros((PD, GH * 3 * PD), bf)
    wie_h = np.zeros((PD, PD), bf)
    np.fill_diagonal(wie_h, np.ones(PD, bf))
    cp_full = (1.0 - (1.0 / K) * np.exp(np.asarray(log_k_recov_min,
                                                   np.float64)))
    cpr_h = np.zeros((1, GH * PD), bf)
    cpr_h[0, :] = cp_full.astype(bf)
    for g in range(GH):
        lanes = slice(g * PD, (g + 1) * PD)
        for k in range(K):
            blockh = wh_h[:, (g * K + k) * PD:(g * K + k + 1) * PD]
            np.fill_diagonal(blockh, hw_[k, lanes].astype(bf))
        for j in range(3):
            blockw = wi_h[:, (g * 3 + j) * PD:(g * 3 + j + 1) * PD]
            np.fill_diagonal(blockw, wi[j, lanes].astype(bf))
            blockz = wid_h[:, (g * 3 + j) * PD:(g * 3 + j + 1) * PD]
            np.fill_diagonal(blockz, wz[j, lanes].astype(bf))

    nc = _get_program()
    in_maps = []
    for c in range(NCORES):
        Ic = I_Ca[c * BPC:(c + 1) * BPC]
        Ic = Ic.reshape(BPC, T, GH, PD).transpose(0, 2, 3, 1)
        in_maps.append({
            "i_ca": np.ascontiguousarray(Ic.reshape(NLB, PD, T)).astype(bf),
            "par": par_core,
            "wh": wh_h, "wi": wi_h, "wid": wid_h,
            "wie": wie_h, "cpr": cpr_h,
        })

    res = run_bass_kernel_spmd(nc, in_maps, core_ids=list(range(NCORES)))
    global LAST_RESULTS
    LAST_RESULTS = res
    nblk = T // TB
    out = np.empty((B, T, H), np.float32)
    for c in range(NCORES):
        Oc = np.stack([
            np.concatenate([res.results[c][f"epsc_{lb}_{blk}"].astype(np.float32)
                            for blk in range(nblk)], axis=1)
            for lb in range(NLB)])
        Oc = Oc.reshape(BPC, GH, PD, T)
        out[c * BPC:(c + 1) * BPC] = Oc.transpose(0, 3, 1, 2).reshape(BPC, T, H)
    return out
